# revision 15
# baseline (speedup 1.0000x reference)
"""Trainium2 Bass kernel for nn_AttentionManifold (SPD manifold attention).

For each of bs*m=2048 SPD matrices X (100x100): Q/K/V = W^T X W (64x64),
logQ/K/V = matrix log, log-Euclidean attention (Frobenius distances ->
scores -> softmax over K index), mixed = prob-weighted sum of logV,
out = matrix exp(mixed).

Matrix log via tuned Newton-Schulz sqrt chain (4 levels, R = (A/16)^(1/16),
log A = 16 log R + log16*I; the global log16*I terms cancel in the
attention distances and fold into a final *16 output scale), log R via a
degree-14 series (Paterson-Stockmeyer), exp via scaling-squaring (k=5,
degree-7 Taylor).  Q/K paths use fp16 matmuls (scores are insensitive);
V path, congruence mm1 and exp use fp32 matmuls.

Sharding: pure data parallelism, bs=32 -> 4 samples per NeuronCore.
"""
import numpy as np
from contextlib import ExitStack

C_NORM = 16.0
SCHED = [
    [(24.871321977, -35.245186442),
     (1.605560380, -0.024430481),
     (1.595838197, -0.060908024),
     (1.576384611, -0.143218467),
     (1.543497701, -0.291162661),
     (1.511244305, -0.443655343),
     (1.5, -0.5), (1.5, -0.5)],
    [(6.228647233, -6.864010667),
     (1.554009519, -0.242273245),
     (1.518749014, -0.406941447),
     (1.5, -0.5), (1.5, -0.5), (1.5, -0.5)],
    [(3.051424190, -2.460263319),
     (1.508484255, -0.457724181),
     (1.5, -0.5), (1.5, -0.5)],
    [(2.128257338, -1.230895381),
     (1.5, -0.5), (1.5, -0.5)],
]
EXP_DEG = 7
EXP_SQ = 5
DEBUG = False

BS, M, DIN, DOUT = 32, 64, 100, 64
NCORES = 8
NSAMP = BS // NCORES
NP_PAIR = M // 2
PAIR_BATCH = 8
NBATCH = NP_PAIR // PAIR_BATCH


def _flat_sched():
    out = []
    for steps in SCHED:
        for j, (a, b) in enumerate(steps):
            out.append((j == 0, a, b))
    return out


def emit_kernel(nc, tc, ctx, x_ap, wq_ap, wk_ap, wv_ap, out_ap, nsamp=NSAMP, taps=None):
    def tap(name, t):
        if taps is not None and name in taps:
            nc.sync.dma_start(out=taps[name], in_=t)
    import concourse.mybir as mybir
    from concourse.bass import ds, ts
    from concourse.masks import make_identity

    f32 = mybir.dt.float32
    f16 = mybir.dt.float16
    AX = mybir.AxisListType
    OP = mybir.AluOpType
    ACT = mybir.ActivationFunctionType
    WB = PAIR_BATCH * 64

    const = ctx.enter_context(tc.tile_pool(name="const", bufs=1))
    work = ctx.enter_context(tc.tile_pool(name="work", bufs=2))
    big = ctx.enter_context(tc.tile_pool(name="big", bufs=1))
    logs = ctx.enter_context(tc.tile_pool(name="logs", bufs=2))
    chain = ctx.enter_context(tc.tile_pool(name="chain", bufs=2))
    ser = ctx.enter_context(tc.tile_pool(name="ser", bufs=1))
    ps_small = ctx.enter_context(tc.tile_pool(name="ps_s", bufs=2, space="PSUM"))
    ps_big = ctx.enter_context(tc.tile_pool(name="ps_b", bufs=2, space="PSUM"))
    ps_mid = ctx.enter_context(tc.tile_pool(name="ps_m", bufs=1, space="PSUM"))

    # ---------------- constants ----------------
    W3 = const.tile([DIN, 3 * DOUT], f32)
    nc.sync.dma_start(out=W3[:, 0:DOUT], in_=wq_ap)
    nc.sync.dma_start(out=W3[:, DOUT:2 * DOUT], in_=wk_ap)
    nc.sync.dma_start(out=W3[:, 2 * DOUT:3 * DOUT], in_=wv_ap)
    WQh = const.tile([DIN, DOUT], f16)
    WKh = const.tile([DIN, DOUT], f16)
    nc.vector.tensor_copy(out=WQh, in_=W3[:, 0:DOUT])
    nc.vector.tensor_copy(out=WKh, in_=W3[:, DOUT:2 * DOUT])

    IREP16 = const.tile([128, 64], f16)
    IREP32 = const.tile([128, 64], f32)
    for t in (IREP16, IREP32):
        make_identity(nc, t[0:64, :])
        make_identity(nc, t[64:128, :])
    # widened identity / block-coefficient tiles [128, WB]
    IW = {}
    for dt_, rep, tag in ((f16, IREP16, "16"), (f32, IREP32, "32")):
        w = const.tile([128, WB], dt_, tag=f"IW{tag}")
        for p in range(PAIR_BATCH):
            nc.vector.tensor_copy(out=w[:, ts(p, 64)], in_=rep)
        IW[tag] = w
    cI = {}
    for tag in ("16", "32"):
        for k in (4, 8, 12):
            dt_ = f16 if tag == "16" else f32
            t = const.tile([128, WB], dt_, tag=f"cI{tag}_{k}")
            nc.vector.tensor_scalar_mul(t, IW[tag], 1.0 / k)
            cI[(tag, k)] = t

    ones_col = const.tile([64, 1], f32)
    nc.vector.memset(ones_col, 1.0)
    ones_col_h = const.tile([64, 1], f16)
    nc.vector.memset(ones_col_h, 32.0)      # folds the /32 exp prescale
    ones_row = const.tile([1, 64], f32)
    nc.vector.memset(ones_row, 1.0)
    bias_ln = const.tile([64, 1], f32)
    nc.vector.memset(bias_ln, 1.0 + 64e-6)
    bias_one = const.tile([64, 1], f32)
    nc.vector.memset(bias_one, 1.0)

    FS = _flat_sched()

    def mm_pairs(out_ps, lhs_t, rhs_t, ncols=64):
        for p in range(PAIR_BATCH):
            for h in (0, 1):
                nc.tensor.matmul(
                    out_ps[h * 64:h * 64 + 64, ts(p, ncols)],
                    lhs_t[h * 64:h * 64 + 64, ts(p, 64)],
                    rhs_t[h * 64:h * 64 + 64, ts(p, ncols)],
                    start=True, stop=True)

    def chain_and_series(init_t, dt_, tag, flat_t):
        irep = IW["16" if dt_ == f16 else "32"]
        ctag = "16" if dt_ == f16 else "32"
        tag = ctag
        for b in range(NBATCH):
            cs = ds(b * WB, WB)
            # state quad [Y | Yt | Z | Zt] per pair, 256 cols each
            SQ = chain.tile([128, PAIR_BATCH * 256], dt_, tag=f"SQ{tag}")
            sq4 = SQ.rearrange("p (n f c) -> p n f c", f=4, c=64)
            iv = init_t[:, cs].rearrange("p (n c) -> p n c", c=64)
            nc.vector.tensor_copy(out=sq4[:, :, 0, :], in_=iv)
            nc.vector.tensor_copy(out=sq4[:, :, 1, :], in_=iv)
            ir3 = irep.rearrange("p (n c) -> p n c", c=64)
            nc.vector.tensor_copy(out=sq4[:, :, 2, :], in_=ir3)
            nc.vector.tensor_copy(out=sq4[:, :, 3, :], in_=ir3)

            def qmm(out_ps, oslice, lhs4, li, rhs4, ri):
                for p in range(PAIR_BATCH):
                    for h in (0, 1):
                        nc.tensor.matmul(
                            out_ps[h * 64:h * 64 + 64, p * oslice[1] + oslice[0] * 64:
                                   p * oslice[1] + oslice[0] * 64 + 64],
                            lhs4[h * 64:h * 64 + 64, p * 256 + li * 64:p * 256 + li * 64 + 64],
                            rhs4[h * 64:h * 64 + 64, p * 256 + ri * 64:p * 256 + ri * 64 + 64] if ri is not None
                            else rhs4[h * 64:h * 64 + 64, ts(p, 64)],
                            start=True, stop=True)

            for k_idx, (lvl_start, al, be) in enumerate(FS):
                if lvl_start and k_idx > 0:
                    nc.vector.tensor_copy(out=sq4[:, :, 2, :], in_=ir3)
                    nc.vector.tensor_copy(out=sq4[:, :, 3, :], in_=ir3)
                # W = Zt^T Y ; Wt = Y^T Zt
                psA = ps_big.tile([128, PAIR_BATCH * 128], mybir.dt.float32, tag="ps1")
                qmm(psA, (0, 128), SQ, 3, SQ, 0)
                qmm(psA, (1, 128), SQ, 0, SQ, 3)
                psAr = psA.rearrange("p (n f c) -> p n f c", f=2, c=64)
                Pb = chain.tile([128, 2 * WB], dt_, tag=f"Pb{tag}")
                pb3 = Pb.rearrange("p (n f c) -> p n f c", f=2, c=64)
                nc.vector.tensor_scalar_mul(pb3, psAr, be)
                nc.vector.scalar_tensor_tensor(
                    out=pb3[:, :, 0, :], in0=ir3, scalar=al,
                    in1=pb3[:, :, 0, :], op0=OP.mult, op1=OP.add)
                nc.vector.scalar_tensor_tensor(
                    out=pb3[:, :, 1, :], in0=ir3, scalar=al,
                    in1=pb3[:, :, 1, :], op0=OP.mult, op1=OP.add)
                # P = Pb[...,0], Pt = Pb[...,1]
                # Yn = Yt^T P ; Ytn = P^T Yt ; Zn = Pt^T Z ; Ztn = Z^T Pt
                psB = ps_big.tile([128, PAIR_BATCH * 128], mybir.dt.float32, tag="ps1")
                for p in range(PAIR_BATCH):
                    for h in (0, 1):
                        hs = slice(h * 64, h * 64 + 64)
                        yt = SQ[hs, p * 256 + 64:p * 256 + 128]
                        pp = Pb[hs, p * 128:p * 128 + 64]
                        nc.tensor.matmul(psB[hs, p * 128:p * 128 + 64], yt, pp,
                                         start=True, stop=True)
                        nc.tensor.matmul(psB[hs, p * 128 + 64:p * 128 + 128], pp, yt,
                                         start=True, stop=True)
                psC = ps_big.tile([128, PAIR_BATCH * 128], mybir.dt.float32, tag="ps1")
                for p in range(PAIR_BATCH):
                    for h in (0, 1):
                        hs = slice(h * 64, h * 64 + 64)
                        z = SQ[hs, p * 256 + 128:p * 256 + 192]
                        zt = SQ[hs, p * 256 + 192:p * 256 + 256]
                        pt = Pb[hs, p * 128 + 64:p * 128 + 128]
                        nc.tensor.matmul(psC[hs, p * 128:p * 128 + 64], pt, z,
                                         start=True, stop=True)
                        nc.tensor.matmul(psC[hs, p * 128 + 64:p * 128 + 128], z, pt,
                                         start=True, stop=True)
                SQ2 = chain.tile([128, PAIR_BATCH * 256], dt_, tag=f"SQ{tag}")
                sq24 = SQ2.rearrange("p (n f c) -> p n f c", f=4, c=64)
                psBr = psB.rearrange("p (n f c) -> p n f c", f=2, c=64)
                psCr = psC.rearrange("p (n f c) -> p n f c", f=2, c=64)
                nc.vector.tensor_copy(out=sq24[:, :, 0:2, :], in_=psBr)
                nc.vector.tensor_copy(out=sq24[:, :, 2:4, :], in_=psCr)
                SQ, sq4 = SQ2, sq24
            # R = (Y + Yt)/2 ; E = I - R
            E = ser.tile([128, WB], dt_, tag=f"E{tag}")
            e3 = E.rearrange("p (n c) -> p n c", c=64)
            nc.vector.tensor_add(e3, sq4[:, :, 0, :], sq4[:, :, 1, :])
            nc.vector.scalar_tensor_tensor(
                out=e3, in0=e3, scalar=-0.5,
                in1=ir3, op0=OP.mult, op1=OP.add)
            if ctag == "32" and b == 0:
                tap("serE", E)
            psE = ps_mid.tile([128, WB], mybir.dt.float32, tag="ps2")
            mm_pairs(psE, E, E)
            E2 = ser.tile([128, WB], dt_, tag=f"E2{tag}")
            nc.vector.tensor_copy(out=E2, in_=psE)
            psE3 = ps_mid.tile([128, WB], mybir.dt.float32, tag="ps2")
            mm_pairs(psE3, E2, E)
            E3 = ser.tile([128, WB], dt_, tag=f"E3{tag}")
            nc.vector.tensor_copy(out=E3, in_=psE3)
            psE4 = ps_mid.tile([128, WB], mybir.dt.float32, tag="ps2")
            mm_pairs(psE4, E2, E2)
            E4 = ser.tile([128, WB], dt_, tag=f"E4{tag}")
            nc.vector.tensor_copy(out=E4, in_=psE4)
            if ctag == "32" and b == 0:
                tap("serE4", E4)
            B = ser.tile([128, WB], dt_, tag=f"B{tag}")
            nc.vector.scalar_tensor_tensor(out=B, in0=E, scalar=1.0 / 13, in1=cI[(ctag, 12)], op0=OP.mult, op1=OP.add)
            nc.vector.scalar_tensor_tensor(out=B, in0=E2, scalar=1.0 / 14, in1=B, op0=OP.mult, op1=OP.add)
            psH = ps_mid.tile([128, WB], mybir.dt.float32, tag="ps2")
            mm_pairs(psH, E4, B)
            H = ser.tile([128, WB], dt_, tag=f"B{tag}")
            nc.vector.scalar_tensor_tensor(out=H, in0=E, scalar=1.0 / 9, in1=cI[(ctag, 8)], op0=OP.mult, op1=OP.add)
            nc.vector.scalar_tensor_tensor(out=H, in0=E2, scalar=1.0 / 10, in1=H, op0=OP.mult, op1=OP.add)
            nc.vector.scalar_tensor_tensor(out=H, in0=E3, scalar=1.0 / 11, in1=H, op0=OP.mult, op1=OP.add)
            Hs = ser.tile([128, WB], dt_, tag=f"Hs{tag}")
            nc.vector.tensor_copy(out=Hs, in_=psH)
            nc.vector.tensor_add(H, H, Hs)
            psH2 = ps_mid.tile([128, WB], mybir.dt.float32, tag="ps2")
            mm_pairs(psH2, E4, H)
            H2 = ser.tile([128, WB], dt_, tag=f"B{tag}")
            nc.vector.scalar_tensor_tensor(out=H2, in0=E, scalar=1.0 / 5, in1=cI[(ctag, 4)], op0=OP.mult, op1=OP.add)
            nc.vector.scalar_tensor_tensor(out=H2, in0=E2, scalar=1.0 / 6, in1=H2, op0=OP.mult, op1=OP.add)
            nc.vector.scalar_tensor_tensor(out=H2, in0=E3, scalar=1.0 / 7, in1=H2, op0=OP.mult, op1=OP.add)
            Hs2 = ser.tile([128, WB], dt_, tag=f"Hs{tag}")
            nc.vector.tensor_copy(out=Hs2, in_=psH2)
            nc.vector.tensor_add(H2, H2, Hs2)
            psH3 = ps_mid.tile([128, WB], mybir.dt.float32, tag="ps2")
            mm_pairs(psH3, E4, H2)
            B0 = ser.tile([128, WB], dt_, tag=f"B{tag}")
            nc.vector.tensor_scalar_mul(B0, E2, 0.5)
            nc.vector.scalar_tensor_tensor(out=B0, in0=E3, scalar=1.0 / 3, in1=B0, op0=OP.mult, op1=OP.add)
            nc.vector.tensor_add(B0, B0, E)
            Hs3 = ser.tile([128, WB], dt_, tag=f"Hs{tag}")
            nc.vector.tensor_copy(out=Hs3, in_=psH3)
            nc.vector.tensor_add(B0, B0, Hs3)
            if ctag == "32" and b == 0:
                tap("serB0", B0)
            LS = logs.tile([128, WB], dt_, tag=f"LS{tag}")
            nc.vector.tensor_scalar_mul(LS, B0, -16.0)
            flat3 = flat_t.rearrange("p (n two c) -> p n two c", two=2, c=64)
            nc.vector.tensor_copy(
                out=flat3[:, ds(b * PAIR_BATCH, PAIR_BATCH), 0, :],
                in_=LS[0:64, :].rearrange("p (n c) -> p n c", c=64))
            nc.gpsimd.dma_start(
                out=flat3[:, ds(b * PAIR_BATCH, PAIR_BATCH), 1, :],
                in_=LS[64:128, :].rearrange("p (n c) -> p n c", c=64))

    # ======================== per-sample pipeline ========================
    for s in range(nsamp):
        initQ = big.tile([128, NP_PAIR * 64], f16, tag="initQ")
        initK = big.tile([128, NP_PAIR * 64], f16, tag="initK")
        initV = big.tile([128, NP_PAIR * 64], f32, tag="initV")
        oddQ = big.tile([64, NP_PAIR * 64], f16, tag="scr8a")
        oddK = big.tile([64, NP_PAIR * 64], f16, tag="scr8b")
        oddV = big.tile([64, NP_PAIR * 64], f32, tag="scr8c")

        for it in range(M):
            if it % 16 == 0:
                xbuf = work.tile([DIN, 16 * DIN], f32, tag="xbuf")
                nc.sync.dma_start(
                    out=xbuf.rearrange("p (i c) -> p i c", c=DIN),
                    in_=x_ap[s, ds(it, 16)].rearrange("i p c -> p i c"))
            p1 = ps_mid.tile([DIN, 3 * DOUT], mybir.dt.float32, tag="ps2")
            nc.tensor.matmul(p1, xbuf[:, ts(it % 16, DIN)], W3, start=True, stop=True)
            P1qk = work.tile([DIN, 2 * DOUT], f16, tag="p1qk")
            nc.vector.tensor_copy(out=P1qk, in_=p1[:, 0:2 * DOUT])
            P1v = work.tile([DIN, DOUT], f32, tag="p1v")
            nc.vector.tensor_copy(out=P1v, in_=p1[:, 2 * DOUT:3 * DOUT])
            pqkv = ps_small.tile([64, 192], mybir.dt.float32, tag="small")
            nc.tensor.matmul(pqkv[:, 0:64], WQh, P1qk[:, 0:DOUT], start=True, stop=True)
            nc.tensor.matmul(pqkv[:, 64:128], WKh, P1qk[:, DOUT:2 * DOUT], start=True, stop=True)
            nc.tensor.matmul(pqkv[:, 128:192], W3[:, 2 * DOUT:3 * DOUT], P1v, start=True, stop=True)
            pr, h = it // 2, it % 2
            for ci, (init_t, odd_t) in enumerate(((initQ, oddQ), (initK, oddK), (initV, oddV))):
                src = pqkv[:, ci * 64:(ci + 1) * 64]
                if h == 0:
                    nc.vector.tensor_scalar_mul(init_t[0:64, ts(pr, 64)], src, 1.0 / C_NORM)
                else:
                    nc.vector.tensor_scalar_mul(odd_t[:, ts(pr, 64)], src, 1.0 / C_NORM)
        for init_t, odd_t in ((initQ, oddQ), (initK, oddK), (initV, oddV)):
            nc.gpsimd.dma_start(out=init_t[64:128, :], in_=odd_t)
        if s == 0:
            tap("irep32", IREP32)
            tap("initV", initV)

        flatQ = big.tile([64, M * 64], f16, tag="flatQ")
        flatK = big.tile([64, M * 64], f16, tag="flatK")
        flatV = big.tile([64, M * 64], f32, tag="f32scr")
        chain_and_series(initQ, f16, "q", flatQ)
        chain_and_series(initK, f16, "k", flatK)
        chain_and_series(initV, f32, "v", flatV)
        if s == 0:
            tap("flatV", flatV)
            tap("flatQ", flatQ)

        # ---------------- attention ----------------
        partQ = work.tile([64, M], f32, tag="partQ")
        partK = work.tile([64, M], f32, tag="partK")
        for flat_t, part_t in ((flatQ, partQ), (flatK, partK)):
            sq = big.tile([64, M * 64], f32, tag="VF")
            nc.vector.tensor_mul(sq, flat_t, flat_t)
            nc.vector.tensor_reduce(
                out=part_t, in_=sq.rearrange("p (i c) -> p i c", c=64),
                axis=AX.X, op=OP.add)
        ps_qn = ps_small.tile([1, 64], mybir.dt.float32, tag="small")
        nc.tensor.matmul(ps_qn, ones_col, partQ, start=True, stop=True)
        qn_row = work.tile([1, 64], f32, tag="qnrow_sb")
        nc.vector.tensor_copy(out=qn_row, in_=ps_qn)
        ps_kn = ps_small.tile([64, 1], mybir.dt.float32, tag="small")
        nc.tensor.matmul(ps_kn, partK, ones_col, start=True, stop=True)
        kn_col = work.tile([64, 1], f32, tag="kncol_sb")
        nc.vector.tensor_copy(out=kn_col, in_=ps_kn)
        ps_qrep = ps_small.tile([64, 64], mybir.dt.float32, tag="small")
        nc.tensor.matmul(ps_qrep, ones_row, qn_row, start=True, stop=True)
        qrep = work.tile([64, 64], f32, tag="qrep_sb")
        nc.vector.tensor_copy(out=qrep, in_=ps_qrep)

        ps_cross = ps_small.tile([64, 64], mybir.dt.float32, tag="small")
        fQ3 = flatQ.rearrange("p (i c) -> p c i", c=64)
        fK3 = flatK.rearrange("p (i c) -> p c i", c=64)
        for c in range(64):
            nc.tensor.matmul(ps_cross, fK3[:, c, :], fQ3[:, c, :],
                             start=(c == 0), stop=(c == 63))
        cross_sb = work.tile([64, 64], f32, tag="cross_sb")
        nc.vector.tensor_copy(out=cross_sb, in_=ps_cross)
        Et = work.tile([64, 64], f32, tag="Et")
        nc.vector.scalar_tensor_tensor(out=Et, in0=cross_sb, scalar=-2.0,
                                       in1=qrep, op0=OP.mult, op1=OP.add)
        nc.vector.tensor_scalar(out=Et, in0=Et, scalar1=kn_col, scalar2=0.0,
                                op0=OP.add, op1=OP.max)
        lnE = work.tile([64, 64], f32, tag="lnE")
        nc.scalar.activation(out=lnE, in_=Et, func=ACT.Ln,
                             bias=bias_ln, scale=1.0)
        ln1 = work.tile([64, 64], f32, tag="ln1")
        nc.vector.tensor_scalar_add(ln1, lnE, 1.0)
        sc = work.tile([64, 64], f32, tag="sc")
        nc.vector.reciprocal(out=sc, in_=ln1)
        expS = work.tile([64, 64], f16, tag="expS")
        nc.scalar.activation(out=expS, in_=sc, func=ACT.Exp, bias=0.0, scale=1.0)
        if s == 0:
            tap("Et", Et)
            tap("expS", expS)
        ps_cs = ps_small.tile([64, 1], mybir.dt.float32, tag="small")
        nc.tensor.matmul(ps_cs, expS, ones_col_h, start=True, stop=True)
        inv = work.tile([64, 1], f32, tag="inv")
        nc.vector.reciprocal(out=inv, in_=ps_cs)

        VF = big.tile([64, M * 64], f32, tag="VF")
        VF3 = VF.rearrange("p (r c) -> p r c", c=64)
        for r in range(64):
            nc.gpsimd.dma_start(
                out=VF3[:, r:r + 1, :],
                in_=flatV[r:r + 1, :].rearrange("p (i c) -> p i c", c=64))
        VFh = big.tile([64, M * 64], f16, tag="scr8a")
        VFl = big.tile([64, M * 64], f16, tag="scr8b")
        nc.vector.tensor_copy(out=VFh, in_=VF)
        nc.vector.tensor_sub(VFl, VF, VFh)

        M2 = big.tile([64, M * 64], f32, tag="f32scr")
        for ch in range(8):
            ps_m2 = ps_small.tile([64, 512], mybir.dt.float32, tag="small")
            nc.tensor.matmul(ps_m2, expS, VFh[:, ts(ch, 512)], start=True, stop=False)
            nc.tensor.matmul(ps_m2, expS, VFl[:, ts(ch, 512)], start=False, stop=True)
            nc.vector.tensor_scalar_mul(M2[:, ts(ch, 512)], ps_m2, inv)

        S1M = big.tile([128, NP_PAIR * 64], f32, tag="scr8c")
        for j in range(M):
            pr, h = j // 2, j % 2
            nc.gpsimd.dma_start(
                out=S1M[h * 64:h * 64 + 64, ts(pr, 64)].rearrange("p (o c) -> p o c", o=1),
                in_=M2[j:j + 1, :].rearrange("p (r c) -> p r c", c=64))

        if s == 0:
            tap("M2", M2)
            tap("S1M", S1M)
        # ---------------- exp: scaling-squaring ----------------
        outS1 = big.tile([128, NP_PAIR * 64], f32, tag="outS1")
        for b in range(NBATCH):
            cs = ds(b * WB, WB)
            X = S1M[:, cs]
            H = chain.tile([128, WB], f32, tag="expH")
            nc.vector.scalar_tensor_tensor(
                out=H, in0=X, scalar=1.0 / EXP_DEG, in1=IW["32"],
                op0=OP.mult, op1=OP.add)
            for k in range(EXP_DEG - 1, 0, -1):
                psx = ps_mid.tile([128, WB], mybir.dt.float32, tag="ps2")
                mm_pairs(psx, X, H)
                H2 = chain.tile([128, WB], f32, tag="expH")
                nc.vector.tensor_scalar_mul(H2, psx, 1.0 / k)
                nc.vector.tensor_add(H2, H2, IW["32"])
                H = H2
            for sq in range(EXP_SQ):
                psx = ps_mid.tile([128, WB], mybir.dt.float32, tag="ps2")
                mm_pairs(psx, H, H)
                if sq < EXP_SQ - 1:
                    H2 = chain.tile([128, WB], f32, tag="expH")
                    nc.vector.tensor_copy(out=H2, in_=psx)
                    H = H2
                else:
                    nc.vector.tensor_scalar_mul(outS1[:, cs], psx, C_NORM)

        o3 = out_ap[s].rearrange("(pr two) r c -> two r pr c", two=2)
        nc.sync.dma_start(
            out=o3[0], in_=outS1[0:64, :].rearrange("p (pr c) -> p pr c", c=64))
        nc.sync.dma_start(
            out=o3[1], in_=outS1[64:128, :].rearrange("p (pr c) -> p pr c", c=64))


def build(nsamp=NSAMP, num_devices=NCORES):
    import concourse.bacc as bacc
    import concourse.mybir as mybir
    import concourse.tile as tile

    nc = bacc.Bacc("TRN2", target_bir_lowering=False, debug=False,
                   num_devices=num_devices)
    f32 = mybir.dt.float32
    x_ap = nc.dram_tensor("x", [nsamp, M, DIN, DIN], f32, kind="ExternalInput").ap()
    wq = nc.dram_tensor("wq", [DIN, DOUT], f32, kind="ExternalInput").ap()
    wk = nc.dram_tensor("wk", [DIN, DOUT], f32, kind="ExternalInput").ap()
    wv = nc.dram_tensor("wv", [DIN, DOUT], f32, kind="ExternalInput").ap()
    out = nc.dram_tensor("out", [nsamp, M, DOUT, DOUT], f32, kind="ExternalOutput").ap()

    tapspec = {}
    if DEBUG:
        tapspec = {
            "irep32": [128, 64], "initV": [128, NP_PAIR * 64],
            "flatV": [64, M * 64], "flatQ": [64, M * 64],
            "Et": [64, 64], "expS": [64, 64],
            "M2": [64, M * 64], "S1M": [128, NP_PAIR * 64],
        }
        for k in (3, 5, 7, 8, 11, 14, 17, 20):
            tapspec[f"chainYW{k}"] = [128, PAIR_BATCH * 128]
        tapspec["serE"] = [128, PAIR_BATCH * 64]
        tapspec["serE4"] = [128, PAIR_BATCH * 64]
        tapspec["serB0"] = [128, PAIR_BATCH * 64]
    taps = {k: nc.dram_tensor("tap_" + k, v, f32 if k != "flatQ" and k != "expS" else mybir.dt.float16,
                              kind="ExternalOutput").ap()
            for k, v in tapspec.items()}
    with tile.TileContext(nc) as tc, ExitStack() as ctx:
        emit_kernel(nc, tc, ctx, x_ap, wq, wk, wv, out, nsamp=nsamp, taps=taps)
    nc.compile()
    return nc


_CACHED = {}


def _get_nc(nsamp):
    from concourse.bass_interp import get_hw_module
    if nsamp not in _CACHED:
        nc = build(nsamp=nsamp)
        nc.m = get_hw_module(nc.m)
        _CACHED[nsamp] = nc
    return _CACHED[nsamp]


def kernel(x, Wq, Wk, Wv):
    from concourse.bass_utils import run_bass_kernel_spmd

    bs = x.shape[0]
    nsamp = bs // NCORES
    nc = _get_nc(nsamp)
    in_maps = []
    for c in range(NCORES):
        in_maps.append({
            "x": np.ascontiguousarray(x[c * nsamp:(c + 1) * nsamp], dtype=np.float32),
            "wq": np.ascontiguousarray(Wq, dtype=np.float32),
            "wk": np.ascontiguousarray(Wk, dtype=np.float32),
            "wv": np.ascontiguousarray(Wv, dtype=np.float32),
        })
    res = run_bass_kernel_spmd(nc, in_maps, list(range(NCORES)))
    outs = [res.results[c]["out"] for c in range(NCORES)]
    full = np.concatenate(outs, axis=0)
    return full.reshape(bs * M, DOUT, DOUT).astype(np.float32)


# revision 18
# speedup vs baseline: 1.0044x; 1.0044x over previous
"""Trainium2 Bass kernel for nn_AttentionManifold (SPD manifold attention).

For each of bs*m=2048 SPD matrices X (100x100): Q/K/V = W^T X W (64x64),
logQ/K/V = matrix log, log-Euclidean attention (Frobenius distances ->
scores -> softmax over K index), mixed = prob-weighted sum of logV,
out = matrix exp(mixed).

Matrix log via tuned Newton-Schulz sqrt chain (4 levels, R = (A/16)^(1/16),
log A = 16 log R + log16*I; the global log16*I terms cancel in the
attention distances and fold into a final *16 output scale), log R via a
degree-14 series (Paterson-Stockmeyer), exp via scaling-squaring (k=5,
degree-7 Taylor).  Q/K paths use fp16 matmuls (scores are insensitive);
V path, congruence mm1 and exp use fp32 matmuls.

Sharding: pure data parallelism, bs=32 -> 4 samples per NeuronCore.
"""
import numpy as np
from contextlib import ExitStack

C_NORM = 16.0
SCHED = [
    [(24.871321977, -35.245186442),
     (1.605560380, -0.024430481),
     (1.595838197, -0.060908024),
     (1.576384611, -0.143218467),
     (1.543497701, -0.291162661),
     (1.511244305, -0.443655343),
     (1.5, -0.5), (1.5, -0.5)],
    [(6.228647233, -6.864010667),
     (1.554009519, -0.242273245),
     (1.518749014, -0.406941447),
     (1.5, -0.5), (1.5, -0.5), (1.5, -0.5)],
    [(3.051424190, -2.460263319),
     (1.508484255, -0.457724181),
     (1.5, -0.5), (1.5, -0.5)],
    [(2.128257338, -1.230895381),
     (1.5, -0.5), (1.5, -0.5)],
]
EXP_DEG = 7
EXP_SQ = 5
DEBUG = False

BS, M, DIN, DOUT = 32, 64, 100, 64
NCORES = 8
NSAMP = BS // NCORES
NP_PAIR = M // 2
PAIR_BATCH = 8
NBATCH = NP_PAIR // PAIR_BATCH


def _flat_sched():
    out = []
    for steps in SCHED:
        for j, (a, b) in enumerate(steps):
            out.append((j == 0, a, b))
    return out


def emit_kernel(nc, tc, ctx, x_ap, wq_ap, wk_ap, wv_ap, out_ap, nsamp=NSAMP, taps=None):
    def tap(name, t):
        if taps is not None and name in taps:
            nc.sync.dma_start(out=taps[name], in_=t)
    import concourse.mybir as mybir
    from concourse.bass import ds, ts
    from concourse.masks import make_identity

    f32 = mybir.dt.float32
    f16 = mybir.dt.float16
    AX = mybir.AxisListType
    OP = mybir.AluOpType
    ACT = mybir.ActivationFunctionType
    WB = PAIR_BATCH * 64

    const = ctx.enter_context(tc.tile_pool(name="const", bufs=1))
    work = ctx.enter_context(tc.tile_pool(name="work", bufs=2))
    big = ctx.enter_context(tc.tile_pool(name="big", bufs=1))
    logs = ctx.enter_context(tc.tile_pool(name="logs", bufs=2))
    chain = ctx.enter_context(tc.tile_pool(name="chain", bufs=3))
    ser = ctx.enter_context(tc.tile_pool(name="ser", bufs=1))
    chainP = ctx.enter_context(tc.tile_pool(name="chainP", bufs=2))
    ps_small = ctx.enter_context(tc.tile_pool(name="ps_s", bufs=1, space="PSUM"))
    ps_big = ctx.enter_context(tc.tile_pool(name="ps_b", bufs=3, space="PSUM"))
    ps_mid = ctx.enter_context(tc.tile_pool(name="ps_m", bufs=1, space="PSUM"))

    # ---------------- constants ----------------
    W3 = const.tile([DIN, 3 * DOUT], f32)
    nc.sync.dma_start(out=W3[:, 0:DOUT], in_=wq_ap)
    nc.sync.dma_start(out=W3[:, DOUT:2 * DOUT], in_=wk_ap)
    nc.sync.dma_start(out=W3[:, 2 * DOUT:3 * DOUT], in_=wv_ap)
    WQh = const.tile([DIN, DOUT], f16)
    WKh = const.tile([DIN, DOUT], f16)
    nc.vector.tensor_copy(out=WQh, in_=W3[:, 0:DOUT])
    nc.vector.tensor_copy(out=WKh, in_=W3[:, DOUT:2 * DOUT])

    IREP16 = const.tile([128, 64], f16)
    IREP32 = const.tile([128, 64], f32)
    for t in (IREP16, IREP32):
        make_identity(nc, t[0:64, :])
        make_identity(nc, t[64:128, :])
    # widened identity / block-coefficient tiles [128, WB]
    IW = {}
    for dt_, rep, tag in ((f16, IREP16, "16"), (f32, IREP32, "32")):
        w = const.tile([128, WB], dt_, tag=f"IW{tag}")
        for p in range(PAIR_BATCH):
            nc.vector.tensor_copy(out=w[:, ts(p, 64)], in_=rep)
        IW[tag] = w
    cI = {}
    for tag in ("16", "32"):
        for k in (4, 8, 12):
            dt_ = f16 if tag == "16" else f32
            t = const.tile([128, WB], dt_, tag=f"cI{tag}_{k}")
            nc.vector.tensor_scalar_mul(t, IW[tag], 1.0 / k)
            cI[(tag, k)] = t

    ones_col = const.tile([64, 1], f32)
    nc.vector.memset(ones_col, 1.0)
    ones_col_h = const.tile([64, 1], f16)
    nc.vector.memset(ones_col_h, 32.0)      # folds the /32 exp prescale
    ones_row = const.tile([1, 64], f32)
    nc.vector.memset(ones_row, 1.0)
    bias_ln = const.tile([64, 1], f32)
    nc.vector.memset(bias_ln, 1.0 + 64e-6)
    bias_one = const.tile([64, 1], f32)
    nc.vector.memset(bias_one, 1.0)

    FS = _flat_sched()

    def mm_pairs(out_ps, lhs_t, rhs_t, ncols=64):
        for p in range(PAIR_BATCH):
            for h in (0, 1):
                nc.tensor.matmul(
                    out_ps[h * 64:h * 64 + 64, ts(p, ncols)],
                    lhs_t[h * 64:h * 64 + 64, ts(p, 64)],
                    rhs_t[h * 64:h * 64 + 64, ts(p, ncols)],
                    start=True, stop=True)

    def chain_and_series(init_t, dt_, tag, flat_t, b):
        irep = IW["16" if dt_ == f16 else "32"]
        ctag = "16" if dt_ == f16 else "32"
        tag = ctag
        if True:
            cs = ds(b * WB, WB)
            # state quad [Y | Yt | Z | Zt] per pair, 256 cols each
            SQ = chain.tile([128, PAIR_BATCH * 256], dt_, tag=f"SQ{tag}")
            sq4 = SQ.rearrange("p (n f c) -> p n f c", f=4, c=64)
            iv = init_t[:, cs].rearrange("p (n c) -> p n c", c=64)
            nc.vector.tensor_copy(out=sq4[:, :, 0, :], in_=iv)
            nc.vector.tensor_copy(out=sq4[:, :, 1, :], in_=iv)
            ir3 = irep.rearrange("p (n c) -> p n c", c=64)
            nc.vector.tensor_copy(out=sq4[:, :, 2, :], in_=ir3)
            nc.vector.tensor_copy(out=sq4[:, :, 3, :], in_=ir3)

            def qmm(out_ps, oslice, lhs4, li, rhs4, ri):
                for p in range(PAIR_BATCH):
                    for h in (0, 1):
                        nc.tensor.matmul(
                            out_ps[h * 64:h * 64 + 64, p * oslice[1] + oslice[0] * 64:
                                   p * oslice[1] + oslice[0] * 64 + 64],
                            lhs4[h * 64:h * 64 + 64, p * 256 + li * 64:p * 256 + li * 64 + 64],
                            rhs4[h * 64:h * 64 + 64, p * 256 + ri * 64:p * 256 + ri * 64 + 64] if ri is not None
                            else rhs4[h * 64:h * 64 + 64, ts(p, 64)],
                            start=True, stop=True)

            for k_idx, (lvl_start, al, be) in enumerate(FS):
                if lvl_start and k_idx > 0:
                    nc.vector.tensor_copy(out=sq4[:, :, 2, :], in_=ir3)
                    nc.vector.tensor_copy(out=sq4[:, :, 3, :], in_=ir3)
                # W = Zt^T Y ; Wt = Y^T Zt
                psA = ps_big.tile([128, PAIR_BATCH * 128], mybir.dt.float32, tag="ps1")
                qmm(psA, (0, 128), SQ, 3, SQ, 0)
                qmm(psA, (1, 128), SQ, 0, SQ, 3)
                psAr = psA.rearrange("p (n f c) -> p n f c", f=2, c=64)
                Pb = chainP.tile([128, 2 * WB], dt_, tag=f"Pb{tag}")
                pb3 = Pb.rearrange("p (n f c) -> p n f c", f=2, c=64)
                nc.vector.tensor_scalar_mul(pb3, psAr, be)
                nc.vector.scalar_tensor_tensor(
                    out=pb3[:, :, 0, :], in0=ir3, scalar=al,
                    in1=pb3[:, :, 0, :], op0=OP.mult, op1=OP.add)
                nc.vector.scalar_tensor_tensor(
                    out=pb3[:, :, 1, :], in0=ir3, scalar=al,
                    in1=pb3[:, :, 1, :], op0=OP.mult, op1=OP.add)
                # P = Pb[...,0], Pt = Pb[...,1]
                # Yn = Yt^T P ; Ytn = P^T Yt ; Zn = Pt^T Z ; Ztn = Z^T Pt
                psB = ps_big.tile([128, PAIR_BATCH * 128], mybir.dt.float32, tag="ps1")
                for p in range(PAIR_BATCH):
                    for h in (0, 1):
                        hs = slice(h * 64, h * 64 + 64)
                        yt = SQ[hs, p * 256 + 64:p * 256 + 128]
                        pp = Pb[hs, p * 128:p * 128 + 64]
                        nc.tensor.matmul(psB[hs, p * 128:p * 128 + 64], yt, pp,
                                         start=True, stop=True)
                        nc.tensor.matmul(psB[hs, p * 128 + 64:p * 128 + 128], pp, yt,
                                         start=True, stop=True)
                psC = ps_big.tile([128, PAIR_BATCH * 128], mybir.dt.float32, tag="ps1")
                for p in range(PAIR_BATCH):
                    for h in (0, 1):
                        hs = slice(h * 64, h * 64 + 64)
                        z = SQ[hs, p * 256 + 128:p * 256 + 192]
                        zt = SQ[hs, p * 256 + 192:p * 256 + 256]
                        pt = Pb[hs, p * 128 + 64:p * 128 + 128]
                        nc.tensor.matmul(psC[hs, p * 128:p * 128 + 64], pt, z,
                                         start=True, stop=True)
                        nc.tensor.matmul(psC[hs, p * 128 + 64:p * 128 + 128], z, pt,
                                         start=True, stop=True)
                SQ2 = chain.tile([128, PAIR_BATCH * 256], dt_, tag=f"SQ{tag}")
                sq24 = SQ2.rearrange("p (n f c) -> p n f c", f=4, c=64)
                psBr = psB.rearrange("p (n f c) -> p n f c", f=2, c=64)
                psCr = psC.rearrange("p (n f c) -> p n f c", f=2, c=64)
                nc.vector.tensor_copy(out=sq24[:, :, 0:2, :], in_=psBr)
                nc.vector.tensor_copy(out=sq24[:, :, 2:4, :], in_=psCr)
                SQ, sq4 = SQ2, sq24
            # R = (Y + Yt)/2 ; E = I - R
            E = ser.tile([128, WB], dt_, tag=f"E{tag}")
            e3 = E.rearrange("p (n c) -> p n c", c=64)
            nc.vector.tensor_add(e3, sq4[:, :, 0, :], sq4[:, :, 1, :])
            nc.vector.scalar_tensor_tensor(
                out=e3, in0=e3, scalar=-0.5,
                in1=ir3, op0=OP.mult, op1=OP.add)
            if ctag == "32" and b == 0:
                tap("serE", E)
            psE = ps_mid.tile([128, WB], mybir.dt.float32, tag="ps2")
            mm_pairs(psE, E, E)
            E2 = ser.tile([128, WB], dt_, tag=f"E2{tag}")
            nc.vector.tensor_copy(out=E2, in_=psE)
            psE3 = ps_mid.tile([128, WB], mybir.dt.float32, tag="ps2")
            mm_pairs(psE3, E2, E)
            E3 = ser.tile([128, WB], dt_, tag=f"E3{tag}")
            nc.vector.tensor_copy(out=E3, in_=psE3)
            psE4 = ps_mid.tile([128, WB], mybir.dt.float32, tag="ps2")
            mm_pairs(psE4, E2, E2)
            E4 = ser.tile([128, WB], dt_, tag=f"E4{tag}")
            nc.vector.tensor_copy(out=E4, in_=psE4)
            if ctag == "32" and b == 0:
                tap("serE4", E4)
            B = ser.tile([128, WB], dt_, tag=f"B{tag}")
            nc.vector.scalar_tensor_tensor(out=B, in0=E, scalar=1.0 / 13, in1=cI[(ctag, 12)], op0=OP.mult, op1=OP.add)
            nc.vector.scalar_tensor_tensor(out=B, in0=E2, scalar=1.0 / 14, in1=B, op0=OP.mult, op1=OP.add)
            psH = ps_mid.tile([128, WB], mybir.dt.float32, tag="ps2")
            mm_pairs(psH, E4, B)
            H = ser.tile([128, WB], dt_, tag=f"B{tag}")
            nc.vector.scalar_tensor_tensor(out=H, in0=E, scalar=1.0 / 9, in1=cI[(ctag, 8)], op0=OP.mult, op1=OP.add)
            nc.vector.scalar_tensor_tensor(out=H, in0=E2, scalar=1.0 / 10, in1=H, op0=OP.mult, op1=OP.add)
            nc.vector.scalar_tensor_tensor(out=H, in0=E3, scalar=1.0 / 11, in1=H, op0=OP.mult, op1=OP.add)
            Hs = ser.tile([128, WB], dt_, tag=f"Hs{tag}")
            nc.vector.tensor_copy(out=Hs, in_=psH)
            nc.vector.tensor_add(H, H, Hs)
            psH2 = ps_mid.tile([128, WB], mybir.dt.float32, tag="ps2")
            mm_pairs(psH2, E4, H)
            H2 = ser.tile([128, WB], dt_, tag=f"B{tag}")
            nc.vector.scalar_tensor_tensor(out=H2, in0=E, scalar=1.0 / 5, in1=cI[(ctag, 4)], op0=OP.mult, op1=OP.add)
            nc.vector.scalar_tensor_tensor(out=H2, in0=E2, scalar=1.0 / 6, in1=H2, op0=OP.mult, op1=OP.add)
            nc.vector.scalar_tensor_tensor(out=H2, in0=E3, scalar=1.0 / 7, in1=H2, op0=OP.mult, op1=OP.add)
            Hs2 = ser.tile([128, WB], dt_, tag=f"Hs{tag}")
            nc.vector.tensor_copy(out=Hs2, in_=psH2)
            nc.vector.tensor_add(H2, H2, Hs2)
            psH3 = ps_mid.tile([128, WB], mybir.dt.float32, tag="ps2")
            mm_pairs(psH3, E4, H2)
            B0 = ser.tile([128, WB], dt_, tag=f"B{tag}")
            nc.vector.tensor_scalar_mul(B0, E2, 0.5)
            nc.vector.scalar_tensor_tensor(out=B0, in0=E3, scalar=1.0 / 3, in1=B0, op0=OP.mult, op1=OP.add)
            nc.vector.tensor_add(B0, B0, E)
            Hs3 = ser.tile([128, WB], dt_, tag=f"Hs{tag}")
            nc.vector.tensor_copy(out=Hs3, in_=psH3)
            nc.vector.tensor_add(B0, B0, Hs3)
            if ctag == "32" and b == 0:
                tap("serB0", B0)
            LS = logs.tile([128, WB], dt_, tag=f"LS{tag}")
            nc.vector.tensor_scalar_mul(LS, B0, -16.0)
            flat3 = flat_t.rearrange("p (n two c) -> p n two c", two=2, c=64)
            nc.vector.tensor_copy(
                out=flat3[:, ds(b * PAIR_BATCH, PAIR_BATCH), 0, :],
                in_=LS[0:64, :].rearrange("p (n c) -> p n c", c=64))
            nc.gpsimd.dma_start(
                out=flat3[:, ds(b * PAIR_BATCH, PAIR_BATCH), 1, :],
                in_=LS[64:128, :].rearrange("p (n c) -> p n c", c=64))

    # ======================== per-sample pipeline ========================
    for s in range(nsamp):
        initQ = big.tile([128, NP_PAIR * 64], f16, tag="initQ")
        initK = big.tile([128, NP_PAIR * 64], f16, tag="initK")
        initV = big.tile([128, NP_PAIR * 64], f32, tag="initV")
        oddQ = big.tile([64, NP_PAIR * 64], f16, tag="scr8a")
        oddK = big.tile([64, NP_PAIR * 64], f16, tag="scr8b")
        oddV = big.tile([64, NP_PAIR * 64], f32, tag="scr8c")

        for it in range(M):
            if it % 16 == 0:
                xbuf = work.tile([DIN, 16 * DIN], f32, tag="xbuf")
                nc.sync.dma_start(
                    out=xbuf.rearrange("p (i c) -> p i c", c=DIN),
                    in_=x_ap[s, ds(it, 16)].rearrange("i p c -> p i c"))
            p1 = ps_mid.tile([DIN, 3 * DOUT], mybir.dt.float32, tag="ps2")
            nc.tensor.matmul(p1, xbuf[:, ts(it % 16, DIN)], W3, start=True, stop=True)
            P1qk = work.tile([DIN, 2 * DOUT], f16, tag="p1qk")
            nc.vector.tensor_copy(out=P1qk, in_=p1[:, 0:2 * DOUT])
            P1v = work.tile([DIN, DOUT], f32, tag="p1v")
            nc.vector.tensor_copy(out=P1v, in_=p1[:, 2 * DOUT:3 * DOUT])
            pqkv = ps_small.tile([64, 192], mybir.dt.float32, tag="small")
            nc.tensor.matmul(pqkv[:, 0:64], WQh, P1qk[:, 0:DOUT], start=True, stop=True)
            nc.tensor.matmul(pqkv[:, 64:128], WKh, P1qk[:, DOUT:2 * DOUT], start=True, stop=True)
            nc.tensor.matmul(pqkv[:, 128:192], W3[:, 2 * DOUT:3 * DOUT], P1v, start=True, stop=True)
            pr, h = it // 2, it % 2
            for ci, (init_t, odd_t) in enumerate(((initQ, oddQ), (initK, oddK), (initV, oddV))):
                src = pqkv[:, ci * 64:(ci + 1) * 64]
                if h == 0:
                    nc.vector.tensor_scalar_mul(init_t[0:64, ts(pr, 64)], src, 1.0 / C_NORM)
                else:
                    nc.vector.tensor_scalar_mul(odd_t[:, ts(pr, 64)], src, 1.0 / C_NORM)
        for init_t, odd_t in ((initQ, oddQ), (initK, oddK), (initV, oddV)):
            nc.gpsimd.dma_start(out=init_t[64:128, :], in_=odd_t)
        if s == 0:
            tap("irep32", IREP32)
            tap("initV", initV)

        flatQ = big.tile([64, M * 64], f16, tag="flatQ")
        flatK = big.tile([64, M * 64], f16, tag="flatK")
        flatV = big.tile([64, M * 64], f32, tag="f32scr")
        for b in range(NBATCH):
            chain_and_series(initQ, f16, "q", flatQ, b)
            chain_and_series(initK, f16, "k", flatK, b)
            chain_and_series(initV, f32, "v", flatV, b)
        if s == 0:
            tap("flatV", flatV)
            tap("flatQ", flatQ)

        # ---------------- attention ----------------
        partQ = work.tile([64, M], f32, tag="partQ")
        partK = work.tile([64, M], f32, tag="partK")
        for flat_t, part_t in ((flatQ, partQ), (flatK, partK)):
            sq = big.tile([64, M * 64], f32, tag="VF")
            nc.vector.tensor_mul(sq, flat_t, flat_t)
            nc.vector.tensor_reduce(
                out=part_t, in_=sq.rearrange("p (i c) -> p i c", c=64),
                axis=AX.X, op=OP.add)
        ps_qn = ps_small.tile([1, 64], mybir.dt.float32, tag="small")
        nc.tensor.matmul(ps_qn, ones_col, partQ, start=True, stop=True)
        qn_row = work.tile([1, 64], f32, tag="qnrow_sb")
        nc.vector.tensor_copy(out=qn_row, in_=ps_qn)
        ps_kn = ps_small.tile([64, 1], mybir.dt.float32, tag="small")
        nc.tensor.matmul(ps_kn, partK, ones_col, start=True, stop=True)
        kn_col = work.tile([64, 1], f32, tag="kncol_sb")
        nc.vector.tensor_copy(out=kn_col, in_=ps_kn)
        ps_qrep = ps_small.tile([64, 64], mybir.dt.float32, tag="small")
        nc.tensor.matmul(ps_qrep, ones_row, qn_row, start=True, stop=True)
        qrep = work.tile([64, 64], f32, tag="qrep_sb")
        nc.vector.tensor_copy(out=qrep, in_=ps_qrep)

        ps_cross = ps_small.tile([64, 64], mybir.dt.float32, tag="small")
        fQ3 = flatQ.rearrange("p (i c) -> p c i", c=64)
        fK3 = flatK.rearrange("p (i c) -> p c i", c=64)
        for c in range(64):
            nc.tensor.matmul(ps_cross, fK3[:, c, :], fQ3[:, c, :],
                             start=(c == 0), stop=(c == 63))
        cross_sb = work.tile([64, 64], f32, tag="cross_sb")
        nc.vector.tensor_copy(out=cross_sb, in_=ps_cross)
        Et = work.tile([64, 64], f32, tag="Et")
        nc.vector.scalar_tensor_tensor(out=Et, in0=cross_sb, scalar=-2.0,
                                       in1=qrep, op0=OP.mult, op1=OP.add)
        nc.vector.tensor_scalar(out=Et, in0=Et, scalar1=kn_col, scalar2=0.0,
                                op0=OP.add, op1=OP.max)
        lnE = work.tile([64, 64], f32, tag="lnE")
        nc.scalar.activation(out=lnE, in_=Et, func=ACT.Ln,
                             bias=bias_ln, scale=1.0)
        ln1 = work.tile([64, 64], f32, tag="ln1")
        nc.vector.tensor_scalar_add(ln1, lnE, 1.0)
        sc = work.tile([64, 64], f32, tag="sc")
        nc.vector.reciprocal(out=sc, in_=ln1)
        expS = work.tile([64, 64], f16, tag="expS")
        nc.scalar.activation(out=expS, in_=sc, func=ACT.Exp, bias=0.0, scale=1.0)
        if s == 0:
            tap("Et", Et)
            tap("expS", expS)
        ps_cs = ps_small.tile([64, 1], mybir.dt.float32, tag="small")
        nc.tensor.matmul(ps_cs, expS, ones_col_h, start=True, stop=True)
        inv = work.tile([64, 1], f32, tag="inv")
        nc.vector.reciprocal(out=inv, in_=ps_cs)

        VF = big.tile([64, M * 64], f32, tag="VF")
        VF3 = VF.rearrange("p (r c) -> p r c", c=64)
        for r in range(64):
            nc.gpsimd.dma_start(
                out=VF3[:, r:r + 1, :],
                in_=flatV[r:r + 1, :].rearrange("p (i c) -> p i c", c=64))
        VFh = big.tile([64, M * 64], f16, tag="scr8a")
        VFl = big.tile([64, M * 64], f16, tag="scr8b")
        nc.vector.tensor_copy(out=VFh, in_=VF)
        nc.vector.tensor_sub(VFl, VF, VFh)

        M2 = big.tile([64, M * 64], f32, tag="f32scr")
        for ch in range(8):
            ps_m2 = ps_small.tile([64, 512], mybir.dt.float32, tag="small")
            nc.tensor.matmul(ps_m2, expS, VFh[:, ts(ch, 512)], start=True, stop=False)
            nc.tensor.matmul(ps_m2, expS, VFl[:, ts(ch, 512)], start=False, stop=True)
            nc.vector.tensor_scalar_mul(M2[:, ts(ch, 512)], ps_m2, inv)

        S1M = big.tile([128, NP_PAIR * 64], f32, tag="scr8c")
        for j in range(M):
            pr, h = j // 2, j % 2
            nc.gpsimd.dma_start(
                out=S1M[h * 64:h * 64 + 64, ts(pr, 64)].rearrange("p (o c) -> p o c", o=1),
                in_=M2[j:j + 1, :].rearrange("p (r c) -> p r c", c=64))

        if s == 0:
            tap("M2", M2)
            tap("S1M", S1M)
        # ---------------- exp: scaling-squaring ----------------
        outS1 = big.tile([128, NP_PAIR * 64], f32, tag="outS1")
        for b in range(NBATCH):
            cs = ds(b * WB, WB)
            X = S1M[:, cs]
            H = chain.tile([128, WB], f32, tag="expH")
            nc.vector.scalar_tensor_tensor(
                out=H, in0=X, scalar=1.0 / EXP_DEG, in1=IW["32"],
                op0=OP.mult, op1=OP.add)
            for k in range(EXP_DEG - 1, 0, -1):
                psx = ps_mid.tile([128, WB], mybir.dt.float32, tag="ps2")
                mm_pairs(psx, X, H)
                H2 = chain.tile([128, WB], f32, tag="expH")
                nc.vector.tensor_scalar_mul(H2, psx, 1.0 / k)
                nc.vector.tensor_add(H2, H2, IW["32"])
                H = H2
            for sq in range(EXP_SQ):
                psx = ps_mid.tile([128, WB], mybir.dt.float32, tag="ps2")
                mm_pairs(psx, H, H)
                if sq < EXP_SQ - 1:
                    H2 = chain.tile([128, WB], f32, tag="expH")
                    nc.vector.tensor_copy(out=H2, in_=psx)
                    H = H2
                else:
                    nc.vector.tensor_scalar_mul(outS1[:, cs], psx, C_NORM)

        o3 = out_ap[s].rearrange("(pr two) r c -> two r pr c", two=2)
        nc.sync.dma_start(
            out=o3[0], in_=outS1[0:64, :].rearrange("p (pr c) -> p pr c", c=64))
        nc.sync.dma_start(
            out=o3[1], in_=outS1[64:128, :].rearrange("p (pr c) -> p pr c", c=64))


def build(nsamp=NSAMP, num_devices=NCORES):
    import concourse.bacc as bacc
    import concourse.mybir as mybir
    import concourse.tile as tile

    nc = bacc.Bacc("TRN2", target_bir_lowering=False, debug=False,
                   num_devices=num_devices)
    f32 = mybir.dt.float32
    x_ap = nc.dram_tensor("x", [nsamp, M, DIN, DIN], f32, kind="ExternalInput").ap()
    wq = nc.dram_tensor("wq", [DIN, DOUT], f32, kind="ExternalInput").ap()
    wk = nc.dram_tensor("wk", [DIN, DOUT], f32, kind="ExternalInput").ap()
    wv = nc.dram_tensor("wv", [DIN, DOUT], f32, kind="ExternalInput").ap()
    out = nc.dram_tensor("out", [nsamp, M, DOUT, DOUT], f32, kind="ExternalOutput").ap()

    tapspec = {}
    if DEBUG:
        tapspec = {
            "irep32": [128, 64], "initV": [128, NP_PAIR * 64],
            "flatV": [64, M * 64], "flatQ": [64, M * 64],
            "Et": [64, 64], "expS": [64, 64],
            "M2": [64, M * 64], "S1M": [128, NP_PAIR * 64],
        }
        for k in (3, 5, 7, 8, 11, 14, 17, 20):
            tapspec[f"chainYW{k}"] = [128, PAIR_BATCH * 128]
        tapspec["serE"] = [128, PAIR_BATCH * 64]
        tapspec["serE4"] = [128, PAIR_BATCH * 64]
        tapspec["serB0"] = [128, PAIR_BATCH * 64]
    taps = {k: nc.dram_tensor("tap_" + k, v, f32 if k != "flatQ" and k != "expS" else mybir.dt.float16,
                              kind="ExternalOutput").ap()
            for k, v in tapspec.items()}
    with tile.TileContext(nc) as tc, ExitStack() as ctx:
        emit_kernel(nc, tc, ctx, x_ap, wq, wk, wv, out, nsamp=nsamp, taps=taps)
    nc.compile()
    return nc


_CACHED = {}


def _get_nc(nsamp):
    from concourse.bass_interp import get_hw_module
    if nsamp not in _CACHED:
        nc = build(nsamp=nsamp)
        nc.m = get_hw_module(nc.m)
        _CACHED[nsamp] = nc
    return _CACHED[nsamp]


def kernel(x, Wq, Wk, Wv):
    from concourse.bass_utils import run_bass_kernel_spmd

    bs = x.shape[0]
    nsamp = bs // NCORES
    nc = _get_nc(nsamp)
    in_maps = []
    for c in range(NCORES):
        in_maps.append({
            "x": np.ascontiguousarray(x[c * nsamp:(c + 1) * nsamp], dtype=np.float32),
            "wq": np.ascontiguousarray(Wq, dtype=np.float32),
            "wk": np.ascontiguousarray(Wk, dtype=np.float32),
            "wv": np.ascontiguousarray(Wv, dtype=np.float32),
        })
    res = run_bass_kernel_spmd(nc, in_maps, list(range(NCORES)))
    outs = [res.results[c]["out"] for c in range(NCORES)]
    full = np.concatenate(outs, axis=0)
    return full.reshape(bs * M, DOUT, DOUT).astype(np.float32)


# revision 23
# speedup vs baseline: 1.6805x; 1.6731x over previous
"""Trainium2 Bass kernel for nn_AttentionManifold (SPD manifold attention).

For each of bs*m=2048 SPD matrices X (100x100): Q/K/V = W^T X W (64x64),
logQ/K/V = matrix log, log-Euclidean attention (Frobenius distances ->
scores -> softmax over K index), mixed = prob-weighted sum of logV,
out = matrix exp(mixed).

Matrix log via tuned Newton-Schulz sqrt chain (4 levels, R = (A/16)^(1/16),
log A = 16 log R + log16*I; the global log16*I terms cancel in the
attention distances and fold into a final *16 output scale), log R via a
degree-14 series (Paterson-Stockmeyer), exp via scaling-squaring (k=5,
degree-7 Taylor).  Q/K paths use fp16 matmuls (scores are insensitive);
V path, congruence mm1 and exp use fp32 matmuls.

Sharding: pure data parallelism, bs=32 -> 4 samples per NeuronCore.
"""
import numpy as np
from contextlib import ExitStack

C_NORM = 16.0
SCHED = [
    [(24.871321977, -35.245186442),
     (1.605560380, -0.024430481),
     (1.595838197, -0.060908024),
     (1.576384611, -0.143218467),
     (1.543497701, -0.291162661),
     (1.511244305, -0.443655343),
     (1.5, -0.5), (1.5, -0.5)],
    [(6.228647233, -6.864010667),
     (1.554009519, -0.242273245),
     (1.518749014, -0.406941447),
     (1.5, -0.5), (1.5, -0.5), (1.5, -0.5)],
    [(3.051424190, -2.460263319),
     (1.508484255, -0.457724181),
     (1.5, -0.5), (1.5, -0.5)],
    [(2.128257338, -1.230895381),
     (1.5, -0.5), (1.5, -0.5)],
]
EXP_DEG = 7
EXP_SQ = 5
DEBUG = False

BS, M, DIN, DOUT = 32, 64, 100, 64
NCORES = 8
NSAMP = BS // NCORES
NP_PAIR = M // 2
PAIR_BATCH = 4
NBATCH = NP_PAIR // PAIR_BATCH


def _flat_sched():
    out = []
    for steps in SCHED:
        for j, (a, b) in enumerate(steps):
            out.append((j == 0, a, b))
    return out


def emit_kernel(nc, tc, ctx, x_ap, wq_ap, wk_ap, wv_ap, out_ap, nsamp=NSAMP, taps=None):
    def tap(name, t):
        if taps is not None and name in taps:
            nc.sync.dma_start(out=taps[name], in_=t)
    import concourse.mybir as mybir
    from concourse.bass import ds, ts
    from concourse.masks import make_identity

    f32 = mybir.dt.float32
    f16 = mybir.dt.float16
    AX = mybir.AxisListType
    OP = mybir.AluOpType
    ACT = mybir.ActivationFunctionType
    WB = PAIR_BATCH * 64

    const = ctx.enter_context(tc.tile_pool(name="const", bufs=1))
    work = ctx.enter_context(tc.tile_pool(name="work", bufs=2))
    big = ctx.enter_context(tc.tile_pool(name="big", bufs=1))
    logs = ctx.enter_context(tc.tile_pool(name="logs", bufs=2))
    chain = ctx.enter_context(tc.tile_pool(name="chain", bufs=2))
    ser = ctx.enter_context(tc.tile_pool(name="ser", bufs=1))
    chainP = ctx.enter_context(tc.tile_pool(name="chainP", bufs=2))
    ps_small = ctx.enter_context(tc.tile_pool(name="ps_s", bufs=1, space="PSUM"))
    ps_big = ctx.enter_context(tc.tile_pool(name="ps_b", bufs=2, space="PSUM"))
    ps_mid = ctx.enter_context(tc.tile_pool(name="ps_m", bufs=1, space="PSUM"))

    # ---------------- constants ----------------
    W3 = const.tile([DIN, 3 * DOUT], f32)
    nc.sync.dma_start(out=W3[:, 0:DOUT], in_=wq_ap)
    nc.sync.dma_start(out=W3[:, DOUT:2 * DOUT], in_=wk_ap)
    nc.sync.dma_start(out=W3[:, 2 * DOUT:3 * DOUT], in_=wv_ap)
    WQh = const.tile([DIN, DOUT], f16)
    WKh = const.tile([DIN, DOUT], f16)
    nc.vector.tensor_copy(out=WQh, in_=W3[:, 0:DOUT])
    nc.vector.tensor_copy(out=WKh, in_=W3[:, DOUT:2 * DOUT])

    IREP16 = const.tile([128, 64], f16)
    IREP32 = const.tile([128, 64], f32)
    for t in (IREP16, IREP32):
        make_identity(nc, t[0:64, :])
        make_identity(nc, t[64:128, :])
    # widened identity / block-coefficient tiles [128, WB]
    IW = {}
    for dt_, rep, tag in ((f16, IREP16, "16"), (f32, IREP32, "32")):
        w = const.tile([128, WB], dt_, tag=f"IW{tag}")
        for p in range(PAIR_BATCH):
            nc.vector.tensor_copy(out=w[:, ts(p, 64)], in_=rep)
        IW[tag] = w
    IWD = {}
    for dt_, rep, tag in ((f16, IREP16, "16"), (f32, IREP32, "32")):
        w = const.tile([128, PAIR_BATCH * 128], dt_, tag=f"IWD{tag}")
        for p in range(2 * PAIR_BATCH):
            nc.vector.tensor_copy(out=w[:, ts(p, 64)], in_=rep)
        IWD[tag] = w
    cI = {}
    for tag in ("16", "32"):
        for k in (4, 8, 12):
            dt_ = f16 if tag == "16" else f32
            t = const.tile([128, WB], dt_, tag=f"cI{tag}_{k}")
            nc.vector.tensor_scalar_mul(t, IW[tag], 1.0 / k)
            cI[(tag, k)] = t

    ones_col = const.tile([64, 1], f32)
    nc.vector.memset(ones_col, 1.0)
    ones_col_h = const.tile([64, 1], f16)
    nc.vector.memset(ones_col_h, 32.0)      # folds the /32 exp prescale
    ones_row = const.tile([1, 64], f32)
    nc.vector.memset(ones_row, 1.0)
    bias_ln = const.tile([64, 1], f32)
    nc.vector.memset(bias_ln, 1.0 + 64e-6)
    bias_one = const.tile([64, 1], f32)
    nc.vector.memset(bias_one, 1.0)

    FS = _flat_sched()

    def mm_pairs(out_ps, lhs_t, rhs_t, ncols=64):
        for p in range(PAIR_BATCH):
            for h in (0, 1):
                nc.tensor.matmul(
                    out_ps[h * 64:h * 64 + 64, ts(p, ncols)],
                    lhs_t[h * 64:h * 64 + 64, ts(p, 64)],
                    rhs_t[h * 64:h * 64 + 64, ts(p, ncols)],
                    start=True, stop=True)

    def chain_and_series(init_t, dt_, tag, flat_t, b):
        # generator: yields after each NS step so Q/K/V emission interleaves
        # V runs level 1 in fp32 (ill-conditioned state), then fp16.
        irep = IW["16" if dt_ == f16 else "32"]
        ctag = "16" if dt_ == f16 else "32"
        if True:
            cs = ds(b * WB, WB)
            # state quad [Y | Yt | Z | Zt] per pair, 256 cols each
            SQ = chain.tile([128, PAIR_BATCH * 256], dt_, tag=f"SQ{tag}")
            sq4 = SQ.rearrange("p (n f c) -> p n f c", f=4, c=64)
            iv = init_t[:, cs].rearrange("p (n c) -> p n c", c=64)
            nc.vector.tensor_copy(out=sq4[:, :, 0, :], in_=iv)
            nc.vector.tensor_copy(out=sq4[:, :, 1, :], in_=iv)
            ir3 = irep.rearrange("p (n c) -> p n c", c=64)
            nc.vector.tensor_copy(out=sq4[:, :, 2, :], in_=ir3)
            nc.vector.tensor_copy(out=sq4[:, :, 3, :], in_=ir3)

            def qmm(out_ps, oslice, lhs4, li, rhs4, ri):
                for p in range(PAIR_BATCH):
                    for h in (0, 1):
                        nc.tensor.matmul(
                            out_ps[h * 64:h * 64 + 64, p * oslice[1] + oslice[0] * 64:
                                   p * oslice[1] + oslice[0] * 64 + 64],
                            lhs4[h * 64:h * 64 + 64, p * 256 + li * 64:p * 256 + li * 64 + 64],
                            rhs4[h * 64:h * 64 + 64, p * 256 + ri * 64:p * 256 + ri * 64 + 64] if ri is not None
                            else rhs4[h * 64:h * 64 + 64, ts(p, 64)],
                            start=True, stop=True)

            for k_idx, (lvl_start, al, be) in enumerate(FS):
                if lvl_start and k_idx == 8 and dt_ == f32:
                    # V-path precision drop: fp32 -> fp16 from level 2 on
                    dt_ = f16
                    irep = IW["16"]
                    ctag = "16"
                    ir3 = irep.rearrange("p (n c) -> p n c", c=64)
                    SQn = chain.tile([128, PAIR_BATCH * 256], dt_, tag=f"SQ{tag}")
                    sqn4 = SQn.rearrange("p (n f c) -> p n f c", f=4, c=64)
                    nc.vector.tensor_copy(out=sqn4[:, :, 0, :], in_=sq4[:, :, 0, :])
                    nc.vector.tensor_copy(out=sqn4[:, :, 1, :], in_=sq4[:, :, 1, :])
                    SQ, sq4 = SQn, sqn4
                if lvl_start and k_idx > 0:
                    nc.vector.tensor_copy(out=sq4[:, :, 2, :], in_=ir3)
                    nc.vector.tensor_copy(out=sq4[:, :, 3, :], in_=ir3)
                # W = Zt^T Y ; Wt = Y^T Zt
                psA = ps_big.tile([128, PAIR_BATCH * 128], mybir.dt.float32, tag="psA")
                qmm(psA, (0, 128), SQ, 3, SQ, 0)
                qmm(psA, (1, 128), SQ, 0, SQ, 3)
                Pb = chainP.tile([128, 2 * WB], dt_, tag=f"Pb{tag}")
                nc.vector.tensor_scalar_mul(Pb, psA, be)
                nc.vector.scalar_tensor_tensor(
                    out=Pb, in0=IWD[ctag], scalar=al,
                    in1=Pb, op0=OP.mult, op1=OP.add)
                # P = Pb[...,0], Pt = Pb[...,1]
                # Yn = Yt^T P ; Ytn = P^T Yt ; Zn = Pt^T Z ; Ztn = Z^T Pt
                psB = ps_big.tile([128, PAIR_BATCH * 128], mybir.dt.float32, tag="psB")
                for p in range(PAIR_BATCH):
                    for h in (0, 1):
                        hs = slice(h * 64, h * 64 + 64)
                        yt = SQ[hs, p * 256 + 64:p * 256 + 128]
                        pp = Pb[hs, p * 128:p * 128 + 64]
                        nc.tensor.matmul(psB[hs, p * 128:p * 128 + 64], yt, pp,
                                         start=True, stop=True)
                        nc.tensor.matmul(psB[hs, p * 128 + 64:p * 128 + 128], pp, yt,
                                         start=True, stop=True)
                psC = ps_big.tile([128, PAIR_BATCH * 128], mybir.dt.float32, tag="psC")
                for p in range(PAIR_BATCH):
                    for h in (0, 1):
                        hs = slice(h * 64, h * 64 + 64)
                        z = SQ[hs, p * 256 + 128:p * 256 + 192]
                        zt = SQ[hs, p * 256 + 192:p * 256 + 256]
                        pt = Pb[hs, p * 128 + 64:p * 128 + 128]
                        nc.tensor.matmul(psC[hs, p * 128:p * 128 + 64], pt, z,
                                         start=True, stop=True)
                        nc.tensor.matmul(psC[hs, p * 128 + 64:p * 128 + 128], z, pt,
                                         start=True, stop=True)
                SQ2 = chain.tile([128, PAIR_BATCH * 256], dt_, tag=f"SQ{tag}")
                sq24 = SQ2.rearrange("p (n f c) -> p n f c", f=4, c=64)
                psBr = psB.rearrange("p (n f c) -> p n f c", f=2, c=64)
                psCr = psC.rearrange("p (n f c) -> p n f c", f=2, c=64)
                nc.scalar.activation(out=sq24[:, :, 0:2, :], in_=psBr,
                                     func=ACT.Copy, bias=0.0, scale=1.0)
                nc.scalar.activation(out=sq24[:, :, 2:4, :], in_=psCr,
                                     func=ACT.Copy, bias=0.0, scale=1.0)
                SQ, sq4 = SQ2, sq24
                yield
            # R = (Y + Yt)/2 ; E = I - R
            E = ser.tile([128, WB], dt_, tag=f"E{tag}")
            e3 = E.rearrange("p (n c) -> p n c", c=64)
            nc.vector.tensor_add(e3, sq4[:, :, 0, :], sq4[:, :, 1, :])
            nc.vector.scalar_tensor_tensor(
                out=e3, in0=e3, scalar=-0.5,
                in1=ir3, op0=OP.mult, op1=OP.add)
            if ctag == "32" and b == 0:
                tap("serE", E)
            psE = ps_mid.tile([128, WB], mybir.dt.float32, tag="ps2")
            mm_pairs(psE, E, E)
            E2 = ser.tile([128, WB], dt_, tag=f"E2{tag}")
            nc.vector.tensor_copy(out=E2, in_=psE)
            psE3 = ps_mid.tile([128, WB], mybir.dt.float32, tag="ps2")
            mm_pairs(psE3, E2, E)
            E3 = ser.tile([128, WB], dt_, tag=f"E3{tag}")
            nc.vector.tensor_copy(out=E3, in_=psE3)
            yield
            psE4 = ps_mid.tile([128, WB], mybir.dt.float32, tag="ps2")
            mm_pairs(psE4, E2, E2)
            E4 = ser.tile([128, WB], dt_, tag=f"E4{tag}")
            nc.vector.tensor_copy(out=E4, in_=psE4)
            if ctag == "32" and b == 0:
                tap("serE4", E4)
            B = ser.tile([128, WB], dt_, tag=f"B{tag}")
            nc.vector.scalar_tensor_tensor(out=B, in0=E, scalar=1.0 / 13, in1=cI[(ctag, 12)], op0=OP.mult, op1=OP.add)
            nc.vector.scalar_tensor_tensor(out=B, in0=E2, scalar=1.0 / 14, in1=B, op0=OP.mult, op1=OP.add)
            psH = ps_mid.tile([128, WB], mybir.dt.float32, tag="ps2")
            mm_pairs(psH, E4, B)
            H = ser.tile([128, WB], dt_, tag=f"B{tag}")
            nc.vector.scalar_tensor_tensor(out=H, in0=E, scalar=1.0 / 9, in1=cI[(ctag, 8)], op0=OP.mult, op1=OP.add)
            nc.vector.scalar_tensor_tensor(out=H, in0=E2, scalar=1.0 / 10, in1=H, op0=OP.mult, op1=OP.add)
            nc.vector.scalar_tensor_tensor(out=H, in0=E3, scalar=1.0 / 11, in1=H, op0=OP.mult, op1=OP.add)
            yield
            Hs = ser.tile([128, WB], dt_, tag=f"Hs{tag}")
            nc.vector.tensor_copy(out=Hs, in_=psH)
            nc.vector.tensor_add(H, H, Hs)
            psH2 = ps_mid.tile([128, WB], mybir.dt.float32, tag="ps2")
            mm_pairs(psH2, E4, H)
            H2 = ser.tile([128, WB], dt_, tag=f"B{tag}")
            nc.vector.scalar_tensor_tensor(out=H2, in0=E, scalar=1.0 / 5, in1=cI[(ctag, 4)], op0=OP.mult, op1=OP.add)
            nc.vector.scalar_tensor_tensor(out=H2, in0=E2, scalar=1.0 / 6, in1=H2, op0=OP.mult, op1=OP.add)
            nc.vector.scalar_tensor_tensor(out=H2, in0=E3, scalar=1.0 / 7, in1=H2, op0=OP.mult, op1=OP.add)
            Hs2 = ser.tile([128, WB], dt_, tag=f"Hs{tag}")
            nc.vector.tensor_copy(out=Hs2, in_=psH2)
            nc.vector.tensor_add(H2, H2, Hs2)
            psH3 = ps_mid.tile([128, WB], mybir.dt.float32, tag="ps2")
            mm_pairs(psH3, E4, H2)
            B0 = ser.tile([128, WB], dt_, tag=f"B{tag}")
            nc.vector.tensor_scalar_mul(B0, E2, 0.5)
            nc.vector.scalar_tensor_tensor(out=B0, in0=E3, scalar=1.0 / 3, in1=B0, op0=OP.mult, op1=OP.add)
            nc.vector.tensor_add(B0, B0, E)
            Hs3 = ser.tile([128, WB], dt_, tag=f"Hs{tag}")
            nc.vector.tensor_copy(out=Hs3, in_=psH3)
            nc.vector.tensor_add(B0, B0, Hs3)
            if ctag == "32" and b == 0:
                tap("serB0", B0)
            LS = logs.tile([128, WB], flat_t.dtype, tag=f"LS{tag}")
            nc.vector.tensor_scalar_mul(LS, B0, -16.0)
            flat3 = flat_t.rearrange("p (n two c) -> p n two c", two=2, c=64)
            nc.vector.tensor_copy(
                out=flat3[:, ds(b * PAIR_BATCH, PAIR_BATCH), 0, :],
                in_=LS[0:64, :].rearrange("p (n c) -> p n c", c=64))
            nc.gpsimd.dma_start(
                out=flat3[:, ds(b * PAIR_BATCH, PAIR_BATCH), 1, :],
                in_=LS[64:128, :].rearrange("p (n c) -> p n c", c=64))

    # ======================== per-sample pipeline ========================
    for s in range(nsamp):
        initQ = big.tile([128, NP_PAIR * 64], f16, tag="initQ")
        initK = big.tile([128, NP_PAIR * 64], f16, tag="initK")
        initV = big.tile([128, NP_PAIR * 64], f32, tag="initV")
        oddQ = big.tile([64, NP_PAIR * 64], f16, tag="scr8a")
        oddK = big.tile([64, NP_PAIR * 64], f16, tag="scr8b")
        oddV = big.tile([64, NP_PAIR * 64], f32, tag="scr8c")

        for it in range(M):
            if it % 16 == 0:
                xbuf = work.tile([DIN, 16 * DIN], f32, tag="xbuf")
                nc.sync.dma_start(
                    out=xbuf.rearrange("p (i c) -> p i c", c=DIN),
                    in_=x_ap[s, ds(it, 16)].rearrange("i p c -> p i c"))
            p1 = ps_mid.tile([DIN, 3 * DOUT], mybir.dt.float32, tag="ps2")
            nc.tensor.matmul(p1, xbuf[:, ts(it % 16, DIN)], W3, start=True, stop=True)
            P1qk = work.tile([DIN, 2 * DOUT], f16, tag="p1qk")
            nc.vector.tensor_copy(out=P1qk, in_=p1[:, 0:2 * DOUT])
            P1v = work.tile([DIN, DOUT], f32, tag="p1v")
            nc.vector.tensor_copy(out=P1v, in_=p1[:, 2 * DOUT:3 * DOUT])
            pqkv = ps_small.tile([64, 192], mybir.dt.float32, tag="small")
            nc.tensor.matmul(pqkv[:, 0:64], WQh, P1qk[:, 0:DOUT], start=True, stop=True)
            nc.tensor.matmul(pqkv[:, 64:128], WKh, P1qk[:, DOUT:2 * DOUT], start=True, stop=True)
            nc.tensor.matmul(pqkv[:, 128:192], W3[:, 2 * DOUT:3 * DOUT], P1v, start=True, stop=True)
            pr, h = it // 2, it % 2
            for ci, (init_t, odd_t) in enumerate(((initQ, oddQ), (initK, oddK), (initV, oddV))):
                src = pqkv[:, ci * 64:(ci + 1) * 64]
                if h == 0:
                    nc.vector.tensor_scalar_mul(init_t[0:64, ts(pr, 64)], src, 1.0 / C_NORM)
                else:
                    nc.vector.tensor_scalar_mul(odd_t[:, ts(pr, 64)], src, 1.0 / C_NORM)
        for init_t, odd_t in ((initQ, oddQ), (initK, oddK), (initV, oddV)):
            nc.gpsimd.dma_start(out=init_t[64:128, :], in_=odd_t)
        if s == 0:
            tap("irep32", IREP32)
            tap("initV", initV)

        flatQ = big.tile([64, M * 64], f16, tag="flatQ")
        flatK = big.tile([64, M * 64], f16, tag="flatK")
        flatV = big.tile([64, M * 64], f32, tag="f32scr")
        for b in range(NBATCH):
            gens = [chain_and_series(initQ, f16, "q", flatQ, b),
                    chain_and_series(initK, f16, "k", flatK, b),
                    chain_and_series(initV, f32, "v", flatV, b)]
            while gens:
                gens = [g for g in gens if next(g, StopIteration) is not StopIteration]
        if s == 0:
            tap("flatV", flatV)
            tap("flatQ", flatQ)

        # ---------------- attention ----------------
        partQ = work.tile([64, M], f32, tag="partQ")
        partK = work.tile([64, M], f32, tag="partK")
        for flat_t, part_t in ((flatQ, partQ), (flatK, partK)):
            sq = big.tile([64, M * 64], f32, tag="VF")
            nc.vector.tensor_mul(sq, flat_t, flat_t)
            nc.vector.tensor_reduce(
                out=part_t, in_=sq.rearrange("p (i c) -> p i c", c=64),
                axis=AX.X, op=OP.add)
        ps_qn = ps_small.tile([1, 64], mybir.dt.float32, tag="small")
        nc.tensor.matmul(ps_qn, ones_col, partQ, start=True, stop=True)
        qn_row = work.tile([1, 64], f32, tag="qnrow_sb")
        nc.vector.tensor_copy(out=qn_row, in_=ps_qn)
        ps_kn = ps_small.tile([64, 1], mybir.dt.float32, tag="small")
        nc.tensor.matmul(ps_kn, partK, ones_col, start=True, stop=True)
        kn_col = work.tile([64, 1], f32, tag="kncol_sb")
        nc.vector.tensor_copy(out=kn_col, in_=ps_kn)
        ps_qrep = ps_small.tile([64, 64], mybir.dt.float32, tag="small")
        nc.tensor.matmul(ps_qrep, ones_row, qn_row, start=True, stop=True)
        qrep = work.tile([64, 64], f32, tag="qrep_sb")
        nc.vector.tensor_copy(out=qrep, in_=ps_qrep)

        ps_cross = ps_small.tile([64, 64], mybir.dt.float32, tag="small")
        fQ3 = flatQ.rearrange("p (i c) -> p c i", c=64)
        fK3 = flatK.rearrange("p (i c) -> p c i", c=64)
        for c in range(64):
            nc.tensor.matmul(ps_cross, fK3[:, c, :], fQ3[:, c, :],
                             start=(c == 0), stop=(c == 63))
        cross_sb = work.tile([64, 64], f32, tag="cross_sb")
        nc.vector.tensor_copy(out=cross_sb, in_=ps_cross)
        Et = work.tile([64, 64], f32, tag="Et")
        nc.vector.scalar_tensor_tensor(out=Et, in0=cross_sb, scalar=-2.0,
                                       in1=qrep, op0=OP.mult, op1=OP.add)
        nc.vector.tensor_scalar(out=Et, in0=Et, scalar1=kn_col, scalar2=0.0,
                                op0=OP.add, op1=OP.max)
        lnE = work.tile([64, 64], f32, tag="lnE")
        nc.scalar.activation(out=lnE, in_=Et, func=ACT.Ln,
                             bias=bias_ln, scale=1.0)
        ln1 = work.tile([64, 64], f32, tag="ln1")
        nc.vector.tensor_scalar_add(ln1, lnE, 1.0)
        sc = work.tile([64, 64], f32, tag="sc")
        nc.vector.reciprocal(out=sc, in_=ln1)
        expS = work.tile([64, 64], f16, tag="expS")
        nc.scalar.activation(out=expS, in_=sc, func=ACT.Exp, bias=0.0, scale=1.0)
        if s == 0:
            tap("Et", Et)
            tap("expS", expS)
        ps_cs = ps_small.tile([64, 1], mybir.dt.float32, tag="small")
        nc.tensor.matmul(ps_cs, expS, ones_col_h, start=True, stop=True)
        inv = work.tile([64, 1], f32, tag="inv")
        nc.vector.reciprocal(out=inv, in_=ps_cs)

        VF = big.tile([64, M * 64], f32, tag="VF")
        VF3 = VF.rearrange("p (r c) -> p r c", c=64)
        for r in range(64):
            nc.gpsimd.dma_start(
                out=VF3[:, r:r + 1, :],
                in_=flatV[r:r + 1, :].rearrange("p (i c) -> p i c", c=64))
        expS32 = work.tile([64, 64], f32, tag="expS32")
        nc.vector.tensor_copy(out=expS32, in_=expS)
        M2 = big.tile([64, M * 64], f32, tag="f32scr")
        for ch in range(8):
            ps_m2 = ps_small.tile([64, 512], mybir.dt.float32, tag="small")
            nc.tensor.matmul(ps_m2, expS32, VF[:, ts(ch, 512)], start=True, stop=True)
            nc.vector.tensor_scalar_mul(M2[:, ts(ch, 512)], ps_m2, inv)

        S1M = big.tile([128, NP_PAIR * 64], f32, tag="scr8c")
        for j in range(M):
            pr, h = j // 2, j % 2
            nc.gpsimd.dma_start(
                out=S1M[h * 64:h * 64 + 64, ts(pr, 64)].rearrange("p (o c) -> p o c", o=1),
                in_=M2[j:j + 1, :].rearrange("p (r c) -> p r c", c=64))

        if s == 0:
            tap("M2", M2)
            tap("S1M", S1M)
        # ---------------- exp: scaling-squaring ----------------
        outS1 = big.tile([128, NP_PAIR * 64], f32, tag="outS1")
        for b in range(NBATCH):
            cs = ds(b * WB, WB)
            X = S1M[:, cs]
            H = chain.tile([128, WB], f32, tag="expH")
            nc.vector.scalar_tensor_tensor(
                out=H, in0=X, scalar=1.0 / EXP_DEG, in1=IW["32"],
                op0=OP.mult, op1=OP.add)
            for k in range(EXP_DEG - 1, 0, -1):
                psx = ps_mid.tile([128, WB], mybir.dt.float32, tag="ps2")
                mm_pairs(psx, X, H)
                H2 = chain.tile([128, WB], f32, tag="expH")
                nc.vector.tensor_scalar_mul(H2, psx, 1.0 / k)
                nc.vector.tensor_add(H2, H2, IW["32"])
                H = H2
            for sq in range(EXP_SQ):
                psx = ps_mid.tile([128, WB], mybir.dt.float32, tag="ps2")
                mm_pairs(psx, H, H)
                if sq < EXP_SQ - 1:
                    H2 = chain.tile([128, WB], f32, tag="expH")
                    nc.vector.tensor_copy(out=H2, in_=psx)
                    H = H2
                else:
                    nc.vector.tensor_scalar_mul(outS1[:, cs], psx, C_NORM)

        o3 = out_ap[s].rearrange("(pr two) r c -> two r pr c", two=2)
        nc.sync.dma_start(
            out=o3[0], in_=outS1[0:64, :].rearrange("p (pr c) -> p pr c", c=64))
        nc.sync.dma_start(
            out=o3[1], in_=outS1[64:128, :].rearrange("p (pr c) -> p pr c", c=64))


def build(nsamp=NSAMP, num_devices=NCORES):
    import concourse.bacc as bacc
    import concourse.mybir as mybir
    import concourse.tile as tile

    nc = bacc.Bacc("TRN2", target_bir_lowering=False, debug=False,
                   num_devices=num_devices)
    f32 = mybir.dt.float32
    x_ap = nc.dram_tensor("x", [nsamp, M, DIN, DIN], f32, kind="ExternalInput").ap()
    wq = nc.dram_tensor("wq", [DIN, DOUT], f32, kind="ExternalInput").ap()
    wk = nc.dram_tensor("wk", [DIN, DOUT], f32, kind="ExternalInput").ap()
    wv = nc.dram_tensor("wv", [DIN, DOUT], f32, kind="ExternalInput").ap()
    out = nc.dram_tensor("out", [nsamp, M, DOUT, DOUT], f32, kind="ExternalOutput").ap()

    tapspec = {}
    if DEBUG:
        tapspec = {
            "irep32": [128, 64], "initV": [128, NP_PAIR * 64],
            "flatV": [64, M * 64], "flatQ": [64, M * 64],
            "Et": [64, 64], "expS": [64, 64],
            "M2": [64, M * 64], "S1M": [128, NP_PAIR * 64],
        }
        for k in (3, 5, 7, 8, 11, 14, 17, 20):
            tapspec[f"chainYW{k}"] = [128, PAIR_BATCH * 128]
        tapspec["serE"] = [128, PAIR_BATCH * 64]
        tapspec["serE4"] = [128, PAIR_BATCH * 64]
        tapspec["serB0"] = [128, PAIR_BATCH * 64]
    taps = {k: nc.dram_tensor("tap_" + k, v, f32 if k != "flatQ" and k != "expS" else mybir.dt.float16,
                              kind="ExternalOutput").ap()
            for k, v in tapspec.items()}
    with tile.TileContext(nc) as tc, ExitStack() as ctx:
        emit_kernel(nc, tc, ctx, x_ap, wq, wk, wv, out, nsamp=nsamp, taps=taps)
    nc.compile()
    return nc


_CACHED = {}


def _get_nc(nsamp):
    from concourse.bass_interp import get_hw_module
    if nsamp not in _CACHED:
        nc = build(nsamp=nsamp)
        nc.m = get_hw_module(nc.m)
        _CACHED[nsamp] = nc
    return _CACHED[nsamp]


def kernel(x, Wq, Wk, Wv):
    from concourse.bass_utils import run_bass_kernel_spmd

    bs = x.shape[0]
    nsamp = bs // NCORES
    nc = _get_nc(nsamp)
    in_maps = []
    for c in range(NCORES):
        in_maps.append({
            "x": np.ascontiguousarray(x[c * nsamp:(c + 1) * nsamp], dtype=np.float32),
            "wq": np.ascontiguousarray(Wq, dtype=np.float32),
            "wk": np.ascontiguousarray(Wk, dtype=np.float32),
            "wv": np.ascontiguousarray(Wv, dtype=np.float32),
        })
    res = run_bass_kernel_spmd(nc, in_maps, list(range(NCORES)))
    outs = [res.results[c]["out"] for c in range(NCORES)]
    full = np.concatenate(outs, axis=0)
    return full.reshape(bs * M, DOUT, DOUT).astype(np.float32)


# revision 26
# speedup vs baseline: 1.8690x; 1.1122x over previous
"""Trainium2 Bass kernel for nn_AttentionManifold (SPD manifold attention).

For each of bs*m=2048 SPD matrices X (100x100): Q/K/V = W^T X W (64x64),
logQ/K/V = matrix log, log-Euclidean attention (Frobenius distances ->
scores -> softmax over K index), mixed = prob-weighted sum of logV,
out = matrix exp(mixed).

Matrix log via tuned Newton-Schulz sqrt chain (4 levels, R = (A/16)^(1/16),
log A = 16 log R + log16*I; the global log16*I terms cancel in the
attention distances and fold into a final *16 output scale), log R via a
degree-14 series (Paterson-Stockmeyer), exp via scaling-squaring (k=5,
degree-7 Taylor).  Q/K paths use fp16 matmuls (scores are insensitive);
V path, congruence mm1 and exp use fp32 matmuls.

Sharding: pure data parallelism, bs=32 -> 4 samples per NeuronCore.
"""
import numpy as np
from contextlib import ExitStack

C_NORM = 16.0
SCHED = [
    [(24.871321977, -35.245186442),
     (1.605560380, -0.024430481),
     (1.595838197, -0.060908024),
     (1.576384611, -0.143218467),
     (1.543497701, -0.291162661),
     (1.511244305, -0.443655343),
     (1.5, -0.5), (1.5, -0.5)],
    [(6.228647233, -6.864010667),
     (1.554009519, -0.242273245),
     (1.518749014, -0.406941447),
     (1.5, -0.5), (1.5, -0.5), (1.5, -0.5)],
    [(3.051424190, -2.460263319),
     (1.508484255, -0.457724181),
     (1.5, -0.5), (1.5, -0.5)],
    [(2.128257338, -1.230895381),
     (1.5, -0.5), (1.5, -0.5)],
]
EXP_DEG = 7
EXP_SQ = 5
DEBUG = False

BS, M, DIN, DOUT = 32, 64, 100, 64
NCORES = 8
NSAMP = BS // NCORES
NP_PAIR = M // 2
PAIR_BATCH = 4
NBATCH = NP_PAIR // PAIR_BATCH


def _flat_sched(nlevels=4):
    out = []
    for steps in SCHED[:nlevels]:
        for j, (a, b) in enumerate(steps):
            out.append((j == 0, a, b))
    return out


def emit_kernel(nc, tc, ctx, x_ap, wq_ap, wk_ap, wv_ap, out_ap, nsamp=NSAMP, taps=None):
    def tap(name, t):
        if taps is not None and name in taps:
            nc.sync.dma_start(out=taps[name], in_=t)
    import concourse.mybir as mybir
    from concourse.bass import ds, ts
    from concourse.masks import make_identity

    f32 = mybir.dt.float32
    f16 = mybir.dt.float16
    AX = mybir.AxisListType
    OP = mybir.AluOpType
    ACT = mybir.ActivationFunctionType
    WB = PAIR_BATCH * 64

    const = ctx.enter_context(tc.tile_pool(name="const", bufs=1))
    work = ctx.enter_context(tc.tile_pool(name="work", bufs=2))
    big = ctx.enter_context(tc.tile_pool(name="big", bufs=1))
    logs = ctx.enter_context(tc.tile_pool(name="logs", bufs=2))
    chain = ctx.enter_context(tc.tile_pool(name="chain", bufs=2))
    ser = ctx.enter_context(tc.tile_pool(name="ser", bufs=1))
    chainP = ctx.enter_context(tc.tile_pool(name="chainP", bufs=2))
    ps_small = ctx.enter_context(tc.tile_pool(name="ps_s", bufs=1, space="PSUM"))
    ps_big = ctx.enter_context(tc.tile_pool(name="ps_b", bufs=2, space="PSUM"))
    ps_mid = ctx.enter_context(tc.tile_pool(name="ps_m", bufs=1, space="PSUM"))

    # ---------------- constants ----------------
    W3 = const.tile([DIN, 3 * DOUT], f32)
    nc.sync.dma_start(out=W3[:, 0:DOUT], in_=wq_ap)
    nc.sync.dma_start(out=W3[:, DOUT:2 * DOUT], in_=wk_ap)
    nc.sync.dma_start(out=W3[:, 2 * DOUT:3 * DOUT], in_=wv_ap)
    WQh = const.tile([DIN, DOUT], f16)
    WKh = const.tile([DIN, DOUT], f16)
    nc.vector.tensor_copy(out=WQh, in_=W3[:, 0:DOUT])
    nc.vector.tensor_copy(out=WKh, in_=W3[:, DOUT:2 * DOUT])

    IREP16 = const.tile([128, 64], f16)
    IREP32 = const.tile([128, 64], f32)
    for t in (IREP16, IREP32):
        make_identity(nc, t[0:64, :])
        make_identity(nc, t[64:128, :])
    # widened identity / block-coefficient tiles [128, WB]
    IW = {}
    for dt_, rep, tag in ((f16, IREP16, "16"), (f32, IREP32, "32")):
        w = const.tile([128, WB], dt_, tag=f"IW{tag}")
        for p in range(PAIR_BATCH):
            nc.vector.tensor_copy(out=w[:, ts(p, 64)], in_=rep)
        IW[tag] = w
    IWD = {}
    for dt_, rep, tag in ((f16, IREP16, "16"), (f32, IREP32, "32")):
        w = const.tile([128, PAIR_BATCH * 128], dt_, tag=f"IWD{tag}")
        for p in range(2 * PAIR_BATCH):
            nc.vector.tensor_copy(out=w[:, ts(p, 64)], in_=rep)
        IWD[tag] = w
    cI = {}
    for tag in ("16", "32"):
        for k in (4, 8, 12):
            dt_ = f16 if tag == "16" else f32
            t = const.tile([128, WB], dt_, tag=f"cI{tag}_{k}")
            nc.vector.tensor_scalar_mul(t, IW[tag], 1.0 / k)
            cI[(tag, k)] = t

    ones_col = const.tile([64, 1], f32)
    nc.vector.memset(ones_col, 1.0)
    ones_col_h = const.tile([64, 1], f16)
    nc.vector.memset(ones_col_h, 32.0)      # folds the /32 exp prescale
    ones_row = const.tile([1, 64], f32)
    nc.vector.memset(ones_row, 1.0)
    bias_ln = const.tile([64, 1], f32)
    nc.vector.memset(bias_ln, 1.0 + 64e-6)
    bias_one = const.tile([64, 1], f32)
    nc.vector.memset(bias_one, 1.0)

    FS4 = _flat_sched(4)
    FS3 = _flat_sched(3)

    def mm_pairs(out_ps, lhs_t, rhs_t, ncols=64):
        for p in range(PAIR_BATCH):
            for h in (0, 1):
                nc.tensor.matmul(
                    out_ps[h * 64:h * 64 + 64, ts(p, ncols)],
                    lhs_t[h * 64:h * 64 + 64, ts(p, 64)],
                    rhs_t[h * 64:h * 64 + 64, ts(p, ncols)],
                    start=True, stop=True)

    def chain_and_series(init_t, dt_, tag, flat_t, b):
        # generator: yields after each NS step so Q/K/V emission interleaves
        # V runs level 1 in fp32 (ill-conditioned state), then fp16.
        # Q/K use 3 sqrt levels (log scale 8), V uses 4 (scale 16).
        FS = FS4 if dt_ == f32 else FS3
        lscale = -16.0 if dt_ == f32 else -8.0
        irep = IW["16" if dt_ == f16 else "32"]
        ctag = "16" if dt_ == f16 else "32"
        if True:
            cs = ds(b * WB, WB)
            # state quad [Y | Yt | Z | Zt] per pair, 256 cols each
            SQ = chain.tile([128, PAIR_BATCH * 256], dt_, tag=f"SQ{tag}")
            sq4 = SQ.rearrange("p (n f c) -> p n f c", f=4, c=64)
            iv = init_t[:, cs].rearrange("p (n c) -> p n c", c=64)
            nc.vector.tensor_copy(out=sq4[:, :, 0, :], in_=iv)
            nc.vector.tensor_copy(out=sq4[:, :, 1, :], in_=iv)
            ir3 = irep.rearrange("p (n c) -> p n c", c=64)

            def qmm(out_ps, oslice, lhs4, li, rhs4, ri):
                for p in range(PAIR_BATCH):
                    for h in (0, 1):
                        nc.tensor.matmul(
                            out_ps[h * 64:h * 64 + 64, p * oslice[1] + oslice[0] * 64:
                                   p * oslice[1] + oslice[0] * 64 + 64],
                            lhs4[h * 64:h * 64 + 64, p * 256 + li * 64:p * 256 + li * 64 + 64],
                            rhs4[h * 64:h * 64 + 64, p * 256 + ri * 64:p * 256 + ri * 64 + 64] if ri is not None
                            else rhs4[h * 64:h * 64 + 64, ts(p, 64)],
                            start=True, stop=True)

            for k_idx, (lvl_start, al, be) in enumerate(FS):
                if lvl_start and k_idx == 8 and dt_ == f32:
                    # V-path precision drop: fp32 -> fp16 from level 2 on
                    dt_ = f16
                    irep = IW["16"]
                    ctag = "16"
                    ir3 = irep.rearrange("p (n c) -> p n c", c=64)
                    SQn = chain.tile([128, PAIR_BATCH * 256], dt_, tag=f"SQ{tag}")
                    sqn4 = SQn.rearrange("p (n f c) -> p n f c", f=4, c=64)
                    nc.vector.tensor_copy(out=sqn4[:, :, 0, :], in_=sq4[:, :, 0, :])
                    nc.vector.tensor_copy(out=sqn4[:, :, 1, :], in_=sq4[:, :, 1, :])
                    SQ, sq4 = SQn, sqn4
                if lvl_start:
                    # Z = I here, so W = Y and the level-start step needs no
                    # W-matmuls: P = aI + b*Y (from SBUF), and Z' = P Z = P.
                    Pb = chainP.tile([128, 2 * WB], dt_, tag=f"Pb{tag}")
                    pb3 = Pb.rearrange("p (n f c) -> p n f c", f=2, c=64)
                    nc.scalar.activation(out=pb3, in_=sq4[:, :, 0:2, :],
                                         func=ACT.Copy, bias=0.0, scale=be)
                    nc.vector.scalar_tensor_tensor(
                        out=Pb, in0=IWD[ctag], scalar=al,
                        in1=Pb, op0=OP.mult, op1=OP.add)
                else:
                    # W = Zt^T Y ; Wt = Y^T Zt
                    psA = ps_big.tile([128, PAIR_BATCH * 128], mybir.dt.float32, tag="psA")
                    qmm(psA, (0, 128), SQ, 3, SQ, 0)
                    qmm(psA, (1, 128), SQ, 0, SQ, 3)
                    Pb = chainP.tile([128, 2 * WB], dt_, tag=f"Pb{tag}")
                    nc.scalar.activation(out=Pb, in_=psA, func=ACT.Copy,
                                         bias=0.0, scale=be)
                    nc.vector.scalar_tensor_tensor(
                        out=Pb, in0=IWD[ctag], scalar=al,
                        in1=Pb, op0=OP.mult, op1=OP.add)
                # P = Pb[...,0], Pt = Pb[...,1]
                # Yn = Yt^T P ; Ytn = P^T Yt ; Zn = Pt^T Z ; Ztn = Z^T Pt
                psB = ps_big.tile([128, PAIR_BATCH * 128], mybir.dt.float32, tag="psB")
                for p in range(PAIR_BATCH):
                    for h in (0, 1):
                        hs = slice(h * 64, h * 64 + 64)
                        yt = SQ[hs, p * 256 + 64:p * 256 + 128]
                        pp = Pb[hs, p * 128:p * 128 + 64]
                        nc.tensor.matmul(psB[hs, p * 128:p * 128 + 64], yt, pp,
                                         start=True, stop=True)
                        nc.tensor.matmul(psB[hs, p * 128 + 64:p * 128 + 128], pp, yt,
                                         start=True, stop=True)
                SQ2 = chain.tile([128, PAIR_BATCH * 256], dt_, tag=f"SQ{tag}")
                sq24 = SQ2.rearrange("p (n f c) -> p n f c", f=4, c=64)
                psBr = psB.rearrange("p (n f c) -> p n f c", f=2, c=64)
                nc.scalar.activation(out=sq24[:, :, 0:2, :], in_=psBr,
                                     func=ACT.Copy, bias=0.0, scale=1.0)
                if lvl_start:
                    nc.scalar.activation(out=sq24[:, :, 2:4, :],
                                         in_=Pb.rearrange("p (n f c) -> p n f c", f=2, c=64),
                                         func=ACT.Copy, bias=0.0, scale=1.0)
                else:
                    psC = ps_big.tile([128, PAIR_BATCH * 128], mybir.dt.float32, tag="psC")
                    for p in range(PAIR_BATCH):
                        for h in (0, 1):
                            hs = slice(h * 64, h * 64 + 64)
                            z = SQ[hs, p * 256 + 128:p * 256 + 192]
                            zt = SQ[hs, p * 256 + 192:p * 256 + 256]
                            pt = Pb[hs, p * 128 + 64:p * 128 + 128]
                            nc.tensor.matmul(psC[hs, p * 128:p * 128 + 64], pt, z,
                                             start=True, stop=True)
                            nc.tensor.matmul(psC[hs, p * 128 + 64:p * 128 + 128], z, pt,
                                             start=True, stop=True)
                    psCr = psC.rearrange("p (n f c) -> p n f c", f=2, c=64)
                    nc.scalar.activation(out=sq24[:, :, 2:4, :], in_=psCr,
                                         func=ACT.Copy, bias=0.0, scale=1.0)
                SQ, sq4 = SQ2, sq24
                yield
            # R = (Y + Yt)/2 ; E = I - R
            E = ser.tile([128, WB], dt_, tag=f"E{tag}")
            e3 = E.rearrange("p (n c) -> p n c", c=64)
            nc.vector.tensor_add(e3, sq4[:, :, 0, :], sq4[:, :, 1, :])
            nc.vector.scalar_tensor_tensor(
                out=e3, in0=e3, scalar=-0.5,
                in1=ir3, op0=OP.mult, op1=OP.add)
            if ctag == "32" and b == 0:
                tap("serE", E)
            psE = ps_mid.tile([128, WB], mybir.dt.float32, tag="ps2")
            mm_pairs(psE, E, E)
            E2 = ser.tile([128, WB], dt_, tag=f"E2{tag}")
            nc.vector.tensor_copy(out=E2, in_=psE)
            psE3 = ps_mid.tile([128, WB], mybir.dt.float32, tag="ps2")
            mm_pairs(psE3, E2, E)
            E3 = ser.tile([128, WB], dt_, tag=f"E3{tag}")
            nc.vector.tensor_copy(out=E3, in_=psE3)
            yield
            psE4 = ps_mid.tile([128, WB], mybir.dt.float32, tag="ps2")
            mm_pairs(psE4, E2, E2)
            E4 = ser.tile([128, WB], dt_, tag=f"E4{tag}")
            nc.vector.tensor_copy(out=E4, in_=psE4)
            if ctag == "32" and b == 0:
                tap("serE4", E4)
            B = ser.tile([128, WB], dt_, tag=f"B{tag}")
            nc.vector.scalar_tensor_tensor(out=B, in0=E, scalar=1.0 / 13, in1=cI[(ctag, 12)], op0=OP.mult, op1=OP.add)
            nc.vector.scalar_tensor_tensor(out=B, in0=E2, scalar=1.0 / 14, in1=B, op0=OP.mult, op1=OP.add)
            psH = ps_mid.tile([128, WB], mybir.dt.float32, tag="ps2")
            mm_pairs(psH, E4, B)
            H = ser.tile([128, WB], dt_, tag=f"B{tag}")
            nc.vector.scalar_tensor_tensor(out=H, in0=E, scalar=1.0 / 9, in1=cI[(ctag, 8)], op0=OP.mult, op1=OP.add)
            nc.vector.scalar_tensor_tensor(out=H, in0=E2, scalar=1.0 / 10, in1=H, op0=OP.mult, op1=OP.add)
            nc.vector.scalar_tensor_tensor(out=H, in0=E3, scalar=1.0 / 11, in1=H, op0=OP.mult, op1=OP.add)
            yield
            Hs = ser.tile([128, WB], dt_, tag=f"Hs{tag}")
            nc.vector.tensor_copy(out=Hs, in_=psH)
            nc.vector.tensor_add(H, H, Hs)
            psH2 = ps_mid.tile([128, WB], mybir.dt.float32, tag="ps2")
            mm_pairs(psH2, E4, H)
            H2 = ser.tile([128, WB], dt_, tag=f"B{tag}")
            nc.vector.scalar_tensor_tensor(out=H2, in0=E, scalar=1.0 / 5, in1=cI[(ctag, 4)], op0=OP.mult, op1=OP.add)
            nc.vector.scalar_tensor_tensor(out=H2, in0=E2, scalar=1.0 / 6, in1=H2, op0=OP.mult, op1=OP.add)
            nc.vector.scalar_tensor_tensor(out=H2, in0=E3, scalar=1.0 / 7, in1=H2, op0=OP.mult, op1=OP.add)
            Hs2 = ser.tile([128, WB], dt_, tag=f"Hs{tag}")
            nc.vector.tensor_copy(out=Hs2, in_=psH2)
            nc.vector.tensor_add(H2, H2, Hs2)
            psH3 = ps_mid.tile([128, WB], mybir.dt.float32, tag="ps2")
            mm_pairs(psH3, E4, H2)
            B0 = ser.tile([128, WB], dt_, tag=f"B{tag}")
            nc.vector.tensor_scalar_mul(B0, E2, 0.5)
            nc.vector.scalar_tensor_tensor(out=B0, in0=E3, scalar=1.0 / 3, in1=B0, op0=OP.mult, op1=OP.add)
            nc.vector.tensor_add(B0, B0, E)
            Hs3 = ser.tile([128, WB], dt_, tag=f"Hs{tag}")
            nc.vector.tensor_copy(out=Hs3, in_=psH3)
            nc.vector.tensor_add(B0, B0, Hs3)
            if ctag == "32" and b == 0:
                tap("serB0", B0)
            LS = logs.tile([128, WB], flat_t.dtype, tag=f"LS{tag}")
            nc.scalar.activation(out=LS, in_=B0, func=ACT.Copy,
                                 bias=0.0, scale=lscale)
            flat3 = flat_t.rearrange("p (n two c) -> p n two c", two=2, c=64)
            nc.vector.tensor_copy(
                out=flat3[:, ds(b * PAIR_BATCH, PAIR_BATCH), 0, :],
                in_=LS[0:64, :].rearrange("p (n c) -> p n c", c=64))
            nc.gpsimd.dma_start(
                out=flat3[:, ds(b * PAIR_BATCH, PAIR_BATCH), 1, :],
                in_=LS[64:128, :].rearrange("p (n c) -> p n c", c=64))

    # ======================== per-sample pipeline ========================
    for s in range(nsamp):
        initQ = big.tile([128, NP_PAIR * 64], f16, tag="initQ")
        initK = big.tile([128, NP_PAIR * 64], f16, tag="initK")
        initV = big.tile([128, NP_PAIR * 64], f32, tag="initV")
        oddQ = big.tile([64, NP_PAIR * 64], f16, tag="scr8a")
        oddK = big.tile([64, NP_PAIR * 64], f16, tag="scr8b")
        oddV = big.tile([64, NP_PAIR * 64], f32, tag="scr8c")

        for it in range(M):
            if it % 16 == 0:
                xbuf = work.tile([DIN, 16 * DIN], f32, tag="xbuf")
                nc.sync.dma_start(
                    out=xbuf.rearrange("p (i c) -> p i c", c=DIN),
                    in_=x_ap[s, ds(it, 16)].rearrange("i p c -> p i c"))
            p1 = ps_mid.tile([DIN, 3 * DOUT], mybir.dt.float32, tag="ps2")
            nc.tensor.matmul(p1, xbuf[:, ts(it % 16, DIN)], W3, start=True, stop=True)
            P1qk = work.tile([DIN, 2 * DOUT], f16, tag="p1qk")
            nc.vector.tensor_copy(out=P1qk, in_=p1[:, 0:2 * DOUT])
            P1v = work.tile([DIN, DOUT], f32, tag="p1v")
            nc.vector.tensor_copy(out=P1v, in_=p1[:, 2 * DOUT:3 * DOUT])
            pqkv = ps_small.tile([64, 192], mybir.dt.float32, tag="small")
            nc.tensor.matmul(pqkv[:, 0:64], WQh, P1qk[:, 0:DOUT], start=True, stop=True)
            nc.tensor.matmul(pqkv[:, 64:128], WKh, P1qk[:, DOUT:2 * DOUT], start=True, stop=True)
            nc.tensor.matmul(pqkv[:, 128:192], W3[:, 2 * DOUT:3 * DOUT], P1v, start=True, stop=True)
            pr, h = it // 2, it % 2
            for ci, (init_t, odd_t) in enumerate(((initQ, oddQ), (initK, oddK), (initV, oddV))):
                src = pqkv[:, ci * 64:(ci + 1) * 64]
                if h == 0:
                    nc.scalar.activation(out=init_t[0:64, ts(pr, 64)], in_=src,
                                         func=ACT.Copy, bias=0.0, scale=1.0 / C_NORM)
                else:
                    nc.scalar.activation(out=odd_t[:, ts(pr, 64)], in_=src,
                                         func=ACT.Copy, bias=0.0, scale=1.0 / C_NORM)
        for init_t, odd_t in ((initQ, oddQ), (initK, oddK), (initV, oddV)):
            nc.gpsimd.dma_start(out=init_t[64:128, :], in_=odd_t)
        if s == 0:
            tap("irep32", IREP32)
            tap("initV", initV)

        flatQ = big.tile([64, M * 64], f16, tag="flatQ")
        flatK = big.tile([64, M * 64], f16, tag="flatK")
        flatV = big.tile([64, M * 64], f32, tag="f32scr")
        for b in range(NBATCH):
            gens = [chain_and_series(initQ, f16, "q", flatQ, b),
                    chain_and_series(initK, f16, "k", flatK, b),
                    chain_and_series(initV, f32, "v", flatV, b)]
            while gens:
                gens = [g for g in gens if next(g, StopIteration) is not StopIteration]
        if s == 0:
            tap("flatV", flatV)
            tap("flatQ", flatQ)

        # ---------------- attention ----------------
        partQ = work.tile([64, M], f32, tag="partQ")
        partK = work.tile([64, M], f32, tag="partK")
        for flat_t, part_t in ((flatQ, partQ), (flatK, partK)):
            sq = big.tile([64, M * 64], f32, tag="VF")
            nc.vector.tensor_mul(sq, flat_t, flat_t)
            nc.vector.tensor_reduce(
                out=part_t, in_=sq.rearrange("p (i c) -> p i c", c=64),
                axis=AX.X, op=OP.add)
        ps_qn = ps_small.tile([1, 64], mybir.dt.float32, tag="small")
        nc.tensor.matmul(ps_qn, ones_col, partQ, start=True, stop=True)
        qn_row = work.tile([1, 64], f32, tag="qnrow_sb")
        nc.vector.tensor_copy(out=qn_row, in_=ps_qn)
        ps_kn = ps_small.tile([64, 1], mybir.dt.float32, tag="small")
        nc.tensor.matmul(ps_kn, partK, ones_col, start=True, stop=True)
        kn_col = work.tile([64, 1], f32, tag="kncol_sb")
        nc.vector.tensor_copy(out=kn_col, in_=ps_kn)
        ps_qrep = ps_small.tile([64, 64], mybir.dt.float32, tag="small")
        nc.tensor.matmul(ps_qrep, ones_row, qn_row, start=True, stop=True)
        qrep = work.tile([64, 64], f32, tag="qrep_sb")
        nc.vector.tensor_copy(out=qrep, in_=ps_qrep)

        ps_cross = ps_small.tile([64, 64], mybir.dt.float32, tag="small")
        fQ3 = flatQ.rearrange("p (i c) -> p c i", c=64)
        fK3 = flatK.rearrange("p (i c) -> p c i", c=64)
        for c in range(64):
            nc.tensor.matmul(ps_cross, fK3[:, c, :], fQ3[:, c, :],
                             start=(c == 0), stop=(c == 63))
        cross_sb = work.tile([64, 64], f32, tag="cross_sb")
        nc.vector.tensor_copy(out=cross_sb, in_=ps_cross)
        Et = work.tile([64, 64], f32, tag="Et")
        nc.vector.scalar_tensor_tensor(out=Et, in0=cross_sb, scalar=-2.0,
                                       in1=qrep, op0=OP.mult, op1=OP.add)
        nc.vector.tensor_scalar(out=Et, in0=Et, scalar1=kn_col, scalar2=0.0,
                                op0=OP.add, op1=OP.max)
        lnE = work.tile([64, 64], f32, tag="lnE")
        nc.scalar.activation(out=lnE, in_=Et, func=ACT.Ln,
                             bias=bias_ln, scale=1.0)
        ln1 = work.tile([64, 64], f32, tag="ln1")
        nc.vector.tensor_scalar_add(ln1, lnE, 1.0)
        sc = work.tile([64, 64], f32, tag="sc")
        nc.vector.reciprocal(out=sc, in_=ln1)
        expS = work.tile([64, 64], f16, tag="expS")
        nc.scalar.activation(out=expS, in_=sc, func=ACT.Exp, bias=0.0, scale=1.0)
        if s == 0:
            tap("Et", Et)
            tap("expS", expS)
        ps_cs = ps_small.tile([64, 1], mybir.dt.float32, tag="small")
        nc.tensor.matmul(ps_cs, expS, ones_col_h, start=True, stop=True)
        inv = work.tile([64, 1], f32, tag="inv")
        nc.vector.reciprocal(out=inv, in_=ps_cs)

        VF = big.tile([64, M * 64], f32, tag="VF")
        VF3 = VF.rearrange("p (r c) -> p r c", c=64)
        for r in range(64):
            nc.gpsimd.dma_start(
                out=VF3[:, r:r + 1, :],
                in_=flatV[r:r + 1, :].rearrange("p (i c) -> p i c", c=64))
        expS32 = work.tile([64, 64], f32, tag="expS32")
        nc.vector.tensor_copy(out=expS32, in_=expS)
        M2 = big.tile([64, M * 64], f32, tag="f32scr")
        for ch in range(8):
            ps_m2 = ps_small.tile([64, 512], mybir.dt.float32, tag="small")
            nc.tensor.matmul(ps_m2, expS32, VF[:, ts(ch, 512)], start=True, stop=True)
            nc.vector.tensor_scalar_mul(M2[:, ts(ch, 512)], ps_m2, inv)

        S1M = big.tile([128, NP_PAIR * 64], f32, tag="scr8c")
        for j in range(M):
            pr, h = j // 2, j % 2
            nc.gpsimd.dma_start(
                out=S1M[h * 64:h * 64 + 64, ts(pr, 64)].rearrange("p (o c) -> p o c", o=1),
                in_=M2[j:j + 1, :].rearrange("p (r c) -> p r c", c=64))

        if s == 0:
            tap("M2", M2)
            tap("S1M", S1M)
        # ---------------- exp: scaling-squaring ----------------
        outS1 = big.tile([128, NP_PAIR * 64], f32, tag="outS1")
        for b in range(NBATCH):
            cs = ds(b * WB, WB)
            X = S1M[:, cs]
            H = chain.tile([128, WB], f32, tag="expH")
            nc.vector.scalar_tensor_tensor(
                out=H, in0=X, scalar=1.0 / EXP_DEG, in1=IW["32"],
                op0=OP.mult, op1=OP.add)
            for k in range(EXP_DEG - 1, 0, -1):
                psx = ps_mid.tile([128, WB], mybir.dt.float32, tag="ps2")
                mm_pairs(psx, X, H)
                H2 = chain.tile([128, WB], f32, tag="expH")
                nc.vector.tensor_scalar_mul(H2, psx, 1.0 / k)
                nc.vector.tensor_add(H2, H2, IW["32"])
                H = H2
            for sq in range(EXP_SQ):
                psx = ps_mid.tile([128, WB], mybir.dt.float32, tag="ps2")
                mm_pairs(psx, H, H)
                if sq < EXP_SQ - 1:
                    H2 = chain.tile([128, WB], f32, tag="expH")
                    nc.vector.tensor_copy(out=H2, in_=psx)
                    H = H2
                else:
                    nc.vector.tensor_scalar_mul(outS1[:, cs], psx, C_NORM)

        o3 = out_ap[s].rearrange("(pr two) r c -> two r pr c", two=2)
        nc.sync.dma_start(
            out=o3[0], in_=outS1[0:64, :].rearrange("p (pr c) -> p pr c", c=64))
        nc.sync.dma_start(
            out=o3[1], in_=outS1[64:128, :].rearrange("p (pr c) -> p pr c", c=64))


def build(nsamp=NSAMP, num_devices=NCORES):
    import concourse.bacc as bacc
    import concourse.mybir as mybir
    import concourse.tile as tile

    nc = bacc.Bacc("TRN2", target_bir_lowering=False, debug=False,
                   num_devices=num_devices)
    f32 = mybir.dt.float32
    x_ap = nc.dram_tensor("x", [nsamp, M, DIN, DIN], f32, kind="ExternalInput").ap()
    wq = nc.dram_tensor("wq", [DIN, DOUT], f32, kind="ExternalInput").ap()
    wk = nc.dram_tensor("wk", [DIN, DOUT], f32, kind="ExternalInput").ap()
    wv = nc.dram_tensor("wv", [DIN, DOUT], f32, kind="ExternalInput").ap()
    out = nc.dram_tensor("out", [nsamp, M, DOUT, DOUT], f32, kind="ExternalOutput").ap()

    tapspec = {}
    if DEBUG:
        tapspec = {
            "irep32": [128, 64], "initV": [128, NP_PAIR * 64],
            "flatV": [64, M * 64], "flatQ": [64, M * 64],
            "Et": [64, 64], "expS": [64, 64],
            "M2": [64, M * 64], "S1M": [128, NP_PAIR * 64],
        }
        for k in (3, 5, 7, 8, 11, 14, 17, 20):
            tapspec[f"chainYW{k}"] = [128, PAIR_BATCH * 128]
        tapspec["serE"] = [128, PAIR_BATCH * 64]
        tapspec["serE4"] = [128, PAIR_BATCH * 64]
        tapspec["serB0"] = [128, PAIR_BATCH * 64]
    taps = {k: nc.dram_tensor("tap_" + k, v, f32 if k != "flatQ" and k != "expS" else mybir.dt.float16,
                              kind="ExternalOutput").ap()
            for k, v in tapspec.items()}
    with tile.TileContext(nc) as tc, ExitStack() as ctx:
        emit_kernel(nc, tc, ctx, x_ap, wq, wk, wv, out, nsamp=nsamp, taps=taps)
    nc.compile()
    return nc


_CACHED = {}


def _get_nc(nsamp):
    from concourse.bass_interp import get_hw_module
    if nsamp not in _CACHED:
        nc = build(nsamp=nsamp)
        nc.m = get_hw_module(nc.m)
        _CACHED[nsamp] = nc
    return _CACHED[nsamp]


def kernel(x, Wq, Wk, Wv):
    from concourse.bass_utils import run_bass_kernel_spmd

    bs = x.shape[0]
    nsamp = bs // NCORES
    nc = _get_nc(nsamp)
    in_maps = []
    for c in range(NCORES):
        in_maps.append({
            "x": np.ascontiguousarray(x[c * nsamp:(c + 1) * nsamp], dtype=np.float32),
            "wq": np.ascontiguousarray(Wq, dtype=np.float32),
            "wk": np.ascontiguousarray(Wk, dtype=np.float32),
            "wv": np.ascontiguousarray(Wv, dtype=np.float32),
        })
    res = run_bass_kernel_spmd(nc, in_maps, list(range(NCORES)))
    outs = [res.results[c]["out"] for c in range(NCORES)]
    full = np.concatenate(outs, axis=0)
    return full.reshape(bs * M, DOUT, DOUT).astype(np.float32)


# revision 27
# speedup vs baseline: 1.9345x; 1.0350x over previous
"""Trainium2 Bass kernel for nn_AttentionManifold (SPD manifold attention).

For each of bs*m=2048 SPD matrices X (100x100): Q/K/V = W^T X W (64x64),
logQ/K/V = matrix log, log-Euclidean attention (Frobenius distances ->
scores -> softmax over K index), mixed = prob-weighted sum of logV,
out = matrix exp(mixed).

Matrix log via tuned Newton-Schulz sqrt chain (4 levels, R = (A/16)^(1/16),
log A = 16 log R + log16*I; the global log16*I terms cancel in the
attention distances and fold into a final *16 output scale), log R via a
degree-14 series (Paterson-Stockmeyer), exp via scaling-squaring (k=5,
degree-7 Taylor).  Q/K paths use fp16 matmuls (scores are insensitive);
V path, congruence mm1 and exp use fp32 matmuls.

Sharding: pure data parallelism, bs=32 -> 4 samples per NeuronCore.
"""
import numpy as np
from contextlib import ExitStack

C_NORM = 16.0
SCHED = [
    [(24.871321977, -35.245186442),
     (1.605560380, -0.024430481),
     (1.595838197, -0.060908024),
     (1.576384611, -0.143218467),
     (1.543497701, -0.291162661),
     (1.511244305, -0.443655343),
     (1.5, -0.5), (1.5, -0.5)],
    [(6.228647233, -6.864010667),
     (1.554009519, -0.242273245),
     (1.518749014, -0.406941447),
     (1.5, -0.5), (1.5, -0.5), (1.5, -0.5)],
    [(3.051424190, -2.460263319),
     (1.508484255, -0.457724181),
     (1.5, -0.5), (1.5, -0.5)],
    [(2.128257338, -1.230895381),
     (1.5, -0.5), (1.5, -0.5)],
]
EXP_DEG = 7
EXP_SQ = 5
DEBUG = False

BS, M, DIN, DOUT = 32, 64, 100, 64
NCORES = 8
NSAMP = BS // NCORES
NP_PAIR = M // 2
PAIR_BATCH = 4
NBATCH = NP_PAIR // PAIR_BATCH


def _flat_sched(nlevels=4):
    out = []
    for steps in SCHED[:nlevels]:
        for j, (a, b) in enumerate(steps):
            out.append((j == 0, a, b))
    return out


def emit_kernel(nc, tc, ctx, x_ap, wq_ap, wk_ap, wv_ap, out_ap, nsamp=NSAMP, taps=None):
    def tap(name, t):
        if taps is not None and name in taps:
            nc.sync.dma_start(out=taps[name], in_=t)
    import concourse.mybir as mybir
    from concourse.bass import ds, ts
    from concourse.masks import make_identity

    f32 = mybir.dt.float32
    f16 = mybir.dt.float16
    AX = mybir.AxisListType
    OP = mybir.AluOpType
    ACT = mybir.ActivationFunctionType
    WB = PAIR_BATCH * 64

    const = ctx.enter_context(tc.tile_pool(name="const", bufs=1))
    work = ctx.enter_context(tc.tile_pool(name="work", bufs=2))
    big = ctx.enter_context(tc.tile_pool(name="big", bufs=1))
    init2 = ctx.enter_context(tc.tile_pool(name="init2", bufs=2))
    logs = ctx.enter_context(tc.tile_pool(name="logs", bufs=2))
    chain = ctx.enter_context(tc.tile_pool(name="chain", bufs=2))
    ser = ctx.enter_context(tc.tile_pool(name="ser", bufs=1))
    chainP = ctx.enter_context(tc.tile_pool(name="chainP", bufs=2))
    ps_small = ctx.enter_context(tc.tile_pool(name="ps_s", bufs=1, space="PSUM"))
    ps_big = ctx.enter_context(tc.tile_pool(name="ps_b", bufs=2, space="PSUM"))
    ps_mid = ctx.enter_context(tc.tile_pool(name="ps_m", bufs=1, space="PSUM"))

    # ---------------- constants ----------------
    W3 = const.tile([DIN, 3 * DOUT], f32)
    nc.sync.dma_start(out=W3[:, 0:DOUT], in_=wq_ap)
    nc.sync.dma_start(out=W3[:, DOUT:2 * DOUT], in_=wk_ap)
    nc.sync.dma_start(out=W3[:, 2 * DOUT:3 * DOUT], in_=wv_ap)
    WQh = const.tile([DIN, DOUT], f16)
    WKh = const.tile([DIN, DOUT], f16)
    nc.vector.tensor_copy(out=WQh, in_=W3[:, 0:DOUT])
    nc.vector.tensor_copy(out=WKh, in_=W3[:, DOUT:2 * DOUT])

    IREP16 = const.tile([128, 64], f16)
    IREP32 = const.tile([128, 64], f32)
    for t in (IREP16, IREP32):
        make_identity(nc, t[0:64, :])
        make_identity(nc, t[64:128, :])
    # widened identity / block-coefficient tiles [128, WB]
    IW = {}
    for dt_, rep, tag in ((f16, IREP16, "16"), (f32, IREP32, "32")):
        w = const.tile([128, WB], dt_, tag=f"IW{tag}")
        for p in range(PAIR_BATCH):
            nc.vector.tensor_copy(out=w[:, ts(p, 64)], in_=rep)
        IW[tag] = w
    IWD = {}
    for dt_, rep, tag in ((f16, IREP16, "16"), (f32, IREP32, "32")):
        w = const.tile([128, PAIR_BATCH * 128], dt_, tag=f"IWD{tag}")
        for p in range(2 * PAIR_BATCH):
            nc.vector.tensor_copy(out=w[:, ts(p, 64)], in_=rep)
        IWD[tag] = w
    cI = {}
    for tag in ("16", "32"):
        for k in (4, 8, 12):
            dt_ = f16 if tag == "16" else f32
            t = const.tile([128, WB], dt_, tag=f"cI{tag}_{k}")
            nc.vector.tensor_scalar_mul(t, IW[tag], 1.0 / k)
            cI[(tag, k)] = t

    ones_col = const.tile([64, 1], f32)
    nc.vector.memset(ones_col, 1.0)
    ones_col_h = const.tile([64, 1], f16)
    nc.vector.memset(ones_col_h, 32.0)      # folds the /32 exp prescale
    ones_row = const.tile([1, 64], f32)
    nc.vector.memset(ones_row, 1.0)
    bias_ln = const.tile([64, 1], f32)
    nc.vector.memset(bias_ln, 1.0 + 64e-6)
    bias_one = const.tile([64, 1], f32)
    nc.vector.memset(bias_one, 1.0)

    FS4 = _flat_sched(4)
    FS3 = _flat_sched(3)

    def mm_pairs(out_ps, lhs_t, rhs_t, ncols=64):
        for p in range(PAIR_BATCH):
            for h in (0, 1):
                nc.tensor.matmul(
                    out_ps[h * 64:h * 64 + 64, ts(p, ncols)],
                    lhs_t[h * 64:h * 64 + 64, ts(p, 64)],
                    rhs_t[h * 64:h * 64 + 64, ts(p, ncols)],
                    start=True, stop=True)

    def chain_and_series(init_t, dt_, tag, flat_t, b):
        # generator: yields after each NS step so Q/K/V emission interleaves
        # V runs level 1 in fp32 (ill-conditioned state), then fp16.
        # Q/K use 3 sqrt levels (log scale 8), V uses 4 (scale 16).
        FS = FS4 if dt_ == f32 else FS3
        lscale = -16.0 if dt_ == f32 else -8.0
        irep = IW["16" if dt_ == f16 else "32"]
        ctag = "16" if dt_ == f16 else "32"
        if True:
            cs = ds(b * WB, WB)
            # state quad [Y | Yt | Z | Zt] per pair, 256 cols each
            SQ = chain.tile([128, PAIR_BATCH * 256], dt_, tag=f"SQ{tag}")
            sq4 = SQ.rearrange("p (n f c) -> p n f c", f=4, c=64)
            iv = init_t[:, cs].rearrange("p (n c) -> p n c", c=64)
            nc.vector.tensor_copy(out=sq4[:, :, 0, :], in_=iv)
            nc.vector.tensor_copy(out=sq4[:, :, 1, :], in_=iv)
            ir3 = irep.rearrange("p (n c) -> p n c", c=64)

            def qmm(out_ps, oslice, lhs4, li, rhs4, ri):
                for p in range(PAIR_BATCH):
                    for h in (0, 1):
                        nc.tensor.matmul(
                            out_ps[h * 64:h * 64 + 64, p * oslice[1] + oslice[0] * 64:
                                   p * oslice[1] + oslice[0] * 64 + 64],
                            lhs4[h * 64:h * 64 + 64, p * 256 + li * 64:p * 256 + li * 64 + 64],
                            rhs4[h * 64:h * 64 + 64, p * 256 + ri * 64:p * 256 + ri * 64 + 64] if ri is not None
                            else rhs4[h * 64:h * 64 + 64, ts(p, 64)],
                            start=True, stop=True)

            for k_idx, (lvl_start, al, be) in enumerate(FS):
                if lvl_start and k_idx == 8 and dt_ == f32:
                    # V-path precision drop: fp32 -> fp16 from level 2 on
                    dt_ = f16
                    irep = IW["16"]
                    ctag = "16"
                    ir3 = irep.rearrange("p (n c) -> p n c", c=64)
                    SQn = chain.tile([128, PAIR_BATCH * 256], dt_, tag=f"SQ{tag}")
                    sqn4 = SQn.rearrange("p (n f c) -> p n f c", f=4, c=64)
                    nc.vector.tensor_copy(out=sqn4[:, :, 0, :], in_=sq4[:, :, 0, :])
                    nc.vector.tensor_copy(out=sqn4[:, :, 1, :], in_=sq4[:, :, 1, :])
                    SQ, sq4 = SQn, sqn4
                if lvl_start:
                    # Z = I here, so W = Y and the level-start step needs no
                    # W-matmuls: P = aI + b*Y (from SBUF), and Z' = P Z = P.
                    Pb = chainP.tile([128, 2 * WB], dt_, tag=f"Pb{tag}")
                    pb3 = Pb.rearrange("p (n f c) -> p n f c", f=2, c=64)
                    nc.scalar.activation(out=pb3, in_=sq4[:, :, 0:2, :],
                                         func=ACT.Copy, bias=0.0, scale=be)
                    nc.vector.scalar_tensor_tensor(
                        out=Pb, in0=IWD[ctag], scalar=al,
                        in1=Pb, op0=OP.mult, op1=OP.add)
                else:
                    # W = Zt^T Y ; Wt = Y^T Zt
                    psA = ps_big.tile([128, PAIR_BATCH * 128], mybir.dt.float32, tag="psA")
                    qmm(psA, (0, 128), SQ, 3, SQ, 0)
                    qmm(psA, (1, 128), SQ, 0, SQ, 3)
                    Pb = chainP.tile([128, 2 * WB], dt_, tag=f"Pb{tag}")
                    nc.scalar.activation(out=Pb, in_=psA, func=ACT.Copy,
                                         bias=0.0, scale=be)
                    nc.vector.scalar_tensor_tensor(
                        out=Pb, in0=IWD[ctag], scalar=al,
                        in1=Pb, op0=OP.mult, op1=OP.add)
                # P = Pb[...,0], Pt = Pb[...,1]
                # Yn = Yt^T P ; Ytn = P^T Yt ; Zn = Pt^T Z ; Ztn = Z^T Pt
                psB = ps_big.tile([128, PAIR_BATCH * 128], mybir.dt.float32, tag="psB")
                for p in range(PAIR_BATCH):
                    for h in (0, 1):
                        hs = slice(h * 64, h * 64 + 64)
                        yt = SQ[hs, p * 256 + 64:p * 256 + 128]
                        pp = Pb[hs, p * 128:p * 128 + 64]
                        nc.tensor.matmul(psB[hs, p * 128:p * 128 + 64], yt, pp,
                                         start=True, stop=True)
                        nc.tensor.matmul(psB[hs, p * 128 + 64:p * 128 + 128], pp, yt,
                                         start=True, stop=True)
                SQ2 = chain.tile([128, PAIR_BATCH * 256], dt_, tag=f"SQ{tag}")
                sq24 = SQ2.rearrange("p (n f c) -> p n f c", f=4, c=64)
                psBr = psB.rearrange("p (n f c) -> p n f c", f=2, c=64)
                nc.scalar.activation(out=sq24[:, :, 0:2, :], in_=psBr,
                                     func=ACT.Copy, bias=0.0, scale=1.0)
                if lvl_start:
                    nc.vector.tensor_copy(
                        out=sq24[:, :, 2:4, :],
                        in_=Pb.rearrange("p (n f c) -> p n f c", f=2, c=64))
                else:
                    psC = ps_big.tile([128, PAIR_BATCH * 128], mybir.dt.float32, tag="psC")
                    for p in range(PAIR_BATCH):
                        for h in (0, 1):
                            hs = slice(h * 64, h * 64 + 64)
                            z = SQ[hs, p * 256 + 128:p * 256 + 192]
                            zt = SQ[hs, p * 256 + 192:p * 256 + 256]
                            pt = Pb[hs, p * 128 + 64:p * 128 + 128]
                            nc.tensor.matmul(psC[hs, p * 128:p * 128 + 64], pt, z,
                                             start=True, stop=True)
                            nc.tensor.matmul(psC[hs, p * 128 + 64:p * 128 + 128], z, pt,
                                             start=True, stop=True)
                    psCr = psC.rearrange("p (n f c) -> p n f c", f=2, c=64)
                    nc.vector.tensor_copy(out=sq24[:, :, 2:4, :], in_=psCr)
                SQ, sq4 = SQ2, sq24
                yield
            # R = (Y + Yt)/2 ; E = I - R
            E = ser.tile([128, WB], dt_, tag=f"E{tag}")
            e3 = E.rearrange("p (n c) -> p n c", c=64)
            nc.vector.tensor_add(e3, sq4[:, :, 0, :], sq4[:, :, 1, :])
            nc.vector.scalar_tensor_tensor(
                out=e3, in0=e3, scalar=-0.5,
                in1=ir3, op0=OP.mult, op1=OP.add)
            if ctag == "32" and b == 0:
                tap("serE", E)
            psE = ps_mid.tile([128, WB], mybir.dt.float32, tag="ps2")
            mm_pairs(psE, E, E)
            E2 = ser.tile([128, WB], dt_, tag=f"E2{tag}")
            nc.vector.tensor_copy(out=E2, in_=psE)
            psE3 = ps_mid.tile([128, WB], mybir.dt.float32, tag="ps2")
            mm_pairs(psE3, E2, E)
            E3 = ser.tile([128, WB], dt_, tag=f"E3{tag}")
            nc.vector.tensor_copy(out=E3, in_=psE3)
            yield
            psE4 = ps_mid.tile([128, WB], mybir.dt.float32, tag="ps2")
            mm_pairs(psE4, E2, E2)
            E4 = ser.tile([128, WB], dt_, tag=f"E4{tag}")
            nc.vector.tensor_copy(out=E4, in_=psE4)
            if ctag == "32" and b == 0:
                tap("serE4", E4)
            B = ser.tile([128, WB], dt_, tag=f"B{tag}")
            nc.vector.scalar_tensor_tensor(out=B, in0=E, scalar=1.0 / 13, in1=cI[(ctag, 12)], op0=OP.mult, op1=OP.add)
            nc.vector.scalar_tensor_tensor(out=B, in0=E2, scalar=1.0 / 14, in1=B, op0=OP.mult, op1=OP.add)
            psH = ps_mid.tile([128, WB], mybir.dt.float32, tag="ps2")
            mm_pairs(psH, E4, B)
            H = ser.tile([128, WB], dt_, tag=f"B{tag}")
            nc.vector.scalar_tensor_tensor(out=H, in0=E, scalar=1.0 / 9, in1=cI[(ctag, 8)], op0=OP.mult, op1=OP.add)
            nc.vector.scalar_tensor_tensor(out=H, in0=E2, scalar=1.0 / 10, in1=H, op0=OP.mult, op1=OP.add)
            nc.vector.scalar_tensor_tensor(out=H, in0=E3, scalar=1.0 / 11, in1=H, op0=OP.mult, op1=OP.add)
            yield
            Hs = ser.tile([128, WB], dt_, tag=f"Hs{tag}")
            nc.vector.tensor_copy(out=Hs, in_=psH)
            nc.vector.tensor_add(H, H, Hs)
            psH2 = ps_mid.tile([128, WB], mybir.dt.float32, tag="ps2")
            mm_pairs(psH2, E4, H)
            H2 = ser.tile([128, WB], dt_, tag=f"B{tag}")
            nc.vector.scalar_tensor_tensor(out=H2, in0=E, scalar=1.0 / 5, in1=cI[(ctag, 4)], op0=OP.mult, op1=OP.add)
            nc.vector.scalar_tensor_tensor(out=H2, in0=E2, scalar=1.0 / 6, in1=H2, op0=OP.mult, op1=OP.add)
            nc.vector.scalar_tensor_tensor(out=H2, in0=E3, scalar=1.0 / 7, in1=H2, op0=OP.mult, op1=OP.add)
            Hs2 = ser.tile([128, WB], dt_, tag=f"Hs{tag}")
            nc.vector.tensor_copy(out=Hs2, in_=psH2)
            nc.vector.tensor_add(H2, H2, Hs2)
            psH3 = ps_mid.tile([128, WB], mybir.dt.float32, tag="ps2")
            mm_pairs(psH3, E4, H2)
            B0 = ser.tile([128, WB], dt_, tag=f"B{tag}")
            nc.vector.tensor_scalar_mul(B0, E2, 0.5)
            nc.vector.scalar_tensor_tensor(out=B0, in0=E3, scalar=1.0 / 3, in1=B0, op0=OP.mult, op1=OP.add)
            nc.vector.tensor_add(B0, B0, E)
            Hs3 = ser.tile([128, WB], dt_, tag=f"Hs{tag}")
            nc.vector.tensor_copy(out=Hs3, in_=psH3)
            nc.vector.tensor_add(B0, B0, Hs3)
            if ctag == "32" and b == 0:
                tap("serB0", B0)
            LS = logs.tile([128, WB], flat_t.dtype, tag=f"LS{tag}")
            nc.scalar.activation(out=LS, in_=B0, func=ACT.Copy,
                                 bias=0.0, scale=lscale)
            flat3 = flat_t.rearrange("p (n two c) -> p n two c", two=2, c=64)
            nc.vector.tensor_copy(
                out=flat3[:, ds(b * PAIR_BATCH, PAIR_BATCH), 0, :],
                in_=LS[0:64, :].rearrange("p (n c) -> p n c", c=64))
            nc.gpsimd.dma_start(
                out=flat3[:, ds(b * PAIR_BATCH, PAIR_BATCH), 1, :],
                in_=LS[64:128, :].rearrange("p (n c) -> p n c", c=64))

    # ======================== per-sample pipeline ========================
    for s in range(nsamp):
        initQ = init2.tile([128, NP_PAIR * 64], f16, tag="initQ")
        initK = init2.tile([128, NP_PAIR * 64], f16, tag="initK")
        initV = init2.tile([128, NP_PAIR * 64], f32, tag="initV")
        oddQ = init2.tile([64, NP_PAIR * 64], f16, tag="oddQ")
        oddK = init2.tile([64, NP_PAIR * 64], f16, tag="oddK")
        oddV = init2.tile([64, NP_PAIR * 64], f32, tag="oddV")

        for it in range(M):
            if it % 16 == 0:
                xbuf = work.tile([DIN, 16 * DIN], f32, tag="xbuf")
                nc.sync.dma_start(
                    out=xbuf.rearrange("p (i c) -> p i c", c=DIN),
                    in_=x_ap[s, ds(it, 16)].rearrange("i p c -> p i c"))
            p1 = ps_mid.tile([DIN, 3 * DOUT], mybir.dt.float32, tag="ps2")
            nc.tensor.matmul(p1, xbuf[:, ts(it % 16, DIN)], W3, start=True, stop=True)
            P1qk = work.tile([DIN, 2 * DOUT], f16, tag="p1qk")
            nc.vector.tensor_copy(out=P1qk, in_=p1[:, 0:2 * DOUT])
            P1v = work.tile([DIN, DOUT], f32, tag="p1v")
            nc.vector.tensor_copy(out=P1v, in_=p1[:, 2 * DOUT:3 * DOUT])
            pqkv = ps_small.tile([64, 192], mybir.dt.float32, tag="small")
            nc.tensor.matmul(pqkv[:, 0:64], WQh, P1qk[:, 0:DOUT], start=True, stop=True)
            nc.tensor.matmul(pqkv[:, 64:128], WKh, P1qk[:, DOUT:2 * DOUT], start=True, stop=True)
            nc.tensor.matmul(pqkv[:, 128:192], W3[:, 2 * DOUT:3 * DOUT], P1v, start=True, stop=True)
            pr, h = it // 2, it % 2
            for ci, (init_t, odd_t) in enumerate(((initQ, oddQ), (initK, oddK), (initV, oddV))):
                src = pqkv[:, ci * 64:(ci + 1) * 64]
                if h == 0:
                    nc.scalar.activation(out=init_t[0:64, ts(pr, 64)], in_=src,
                                         func=ACT.Copy, bias=0.0, scale=1.0 / C_NORM)
                else:
                    nc.scalar.activation(out=odd_t[:, ts(pr, 64)], in_=src,
                                         func=ACT.Copy, bias=0.0, scale=1.0 / C_NORM)
        for init_t, odd_t in ((initQ, oddQ), (initK, oddK), (initV, oddV)):
            nc.gpsimd.dma_start(out=init_t[64:128, :], in_=odd_t)
        if s == 0:
            tap("irep32", IREP32)
            tap("initV", initV)

        flatQ = big.tile([64, M * 64], f16, tag="flatQ")
        flatK = big.tile([64, M * 64], f16, tag="flatK")
        flatV = big.tile([64, M * 64], f32, tag="f32scr")
        for b in range(NBATCH):
            gens = [chain_and_series(initQ, f16, "q", flatQ, b),
                    chain_and_series(initK, f16, "k", flatK, b),
                    chain_and_series(initV, f32, "v", flatV, b)]
            while gens:
                gens = [g for g in gens if next(g, StopIteration) is not StopIteration]
        if s == 0:
            tap("flatV", flatV)
            tap("flatQ", flatQ)

        # ---------------- attention ----------------
        partQ = work.tile([64, M], f32, tag="partQ")
        partK = work.tile([64, M], f32, tag="partK")
        for flat_t, part_t in ((flatQ, partQ), (flatK, partK)):
            sq = big.tile([64, M * 64], f32, tag="VF")
            nc.vector.tensor_mul(sq, flat_t, flat_t)
            nc.vector.tensor_reduce(
                out=part_t, in_=sq.rearrange("p (i c) -> p i c", c=64),
                axis=AX.X, op=OP.add)
        ps_qn = ps_small.tile([1, 64], mybir.dt.float32, tag="small")
        nc.tensor.matmul(ps_qn, ones_col, partQ, start=True, stop=True)
        qn_row = work.tile([1, 64], f32, tag="qnrow_sb")
        nc.vector.tensor_copy(out=qn_row, in_=ps_qn)
        ps_kn = ps_small.tile([64, 1], mybir.dt.float32, tag="small")
        nc.tensor.matmul(ps_kn, partK, ones_col, start=True, stop=True)
        kn_col = work.tile([64, 1], f32, tag="kncol_sb")
        nc.vector.tensor_copy(out=kn_col, in_=ps_kn)
        ps_qrep = ps_small.tile([64, 64], mybir.dt.float32, tag="small")
        nc.tensor.matmul(ps_qrep, ones_row, qn_row, start=True, stop=True)
        qrep = work.tile([64, 64], f32, tag="qrep_sb")
        nc.vector.tensor_copy(out=qrep, in_=ps_qrep)

        ps_cross = ps_small.tile([64, 64], mybir.dt.float32, tag="small")
        fQ3 = flatQ.rearrange("p (i c) -> p c i", c=64)
        fK3 = flatK.rearrange("p (i c) -> p c i", c=64)
        for c in range(64):
            nc.tensor.matmul(ps_cross, fK3[:, c, :], fQ3[:, c, :],
                             start=(c == 0), stop=(c == 63))
        cross_sb = work.tile([64, 64], f32, tag="cross_sb")
        nc.vector.tensor_copy(out=cross_sb, in_=ps_cross)
        Et = work.tile([64, 64], f32, tag="Et")
        nc.vector.scalar_tensor_tensor(out=Et, in0=cross_sb, scalar=-2.0,
                                       in1=qrep, op0=OP.mult, op1=OP.add)
        nc.vector.tensor_scalar(out=Et, in0=Et, scalar1=kn_col, scalar2=0.0,
                                op0=OP.add, op1=OP.max)
        lnE = work.tile([64, 64], f32, tag="lnE")
        nc.scalar.activation(out=lnE, in_=Et, func=ACT.Ln,
                             bias=bias_ln, scale=1.0)
        ln1 = work.tile([64, 64], f32, tag="ln1")
        nc.vector.tensor_scalar_add(ln1, lnE, 1.0)
        sc = work.tile([64, 64], f32, tag="sc")
        nc.vector.reciprocal(out=sc, in_=ln1)
        expS = work.tile([64, 64], f16, tag="expS")
        nc.scalar.activation(out=expS, in_=sc, func=ACT.Exp, bias=0.0, scale=1.0)
        if s == 0:
            tap("Et", Et)
            tap("expS", expS)
        ps_cs = ps_small.tile([64, 1], mybir.dt.float32, tag="small")
        nc.tensor.matmul(ps_cs, expS, ones_col_h, start=True, stop=True)
        inv = work.tile([64, 1], f32, tag="inv")
        nc.vector.reciprocal(out=inv, in_=ps_cs)

        VF = big.tile([64, M * 64], f32, tag="VF")
        VF3 = VF.rearrange("p (r c) -> p r c", c=64)
        for r in range(64):
            nc.gpsimd.dma_start(
                out=VF3[:, r:r + 1, :],
                in_=flatV[r:r + 1, :].rearrange("p (i c) -> p i c", c=64))
        expS32 = work.tile([64, 64], f32, tag="expS32")
        nc.vector.tensor_copy(out=expS32, in_=expS)
        M2 = big.tile([64, M * 64], f32, tag="f32scr")
        for ch in range(8):
            ps_m2 = ps_small.tile([64, 512], mybir.dt.float32, tag="small")
            nc.tensor.matmul(ps_m2, expS32, VF[:, ts(ch, 512)], start=True, stop=True)
            nc.vector.tensor_scalar_mul(M2[:, ts(ch, 512)], ps_m2, inv)

        S1M = big.tile([128, NP_PAIR * 64], f32, tag="scr8c")
        for j in range(M):
            pr, h = j // 2, j % 2
            nc.gpsimd.dma_start(
                out=S1M[h * 64:h * 64 + 64, ts(pr, 64)].rearrange("p (o c) -> p o c", o=1),
                in_=M2[j:j + 1, :].rearrange("p (r c) -> p r c", c=64))

        if s == 0:
            tap("M2", M2)
            tap("S1M", S1M)
        # ---------------- exp: scaling-squaring ----------------
        outS1 = big.tile([128, NP_PAIR * 64], f32, tag="outS1")
        for b in range(NBATCH):
            cs = ds(b * WB, WB)
            X = S1M[:, cs]
            H = chain.tile([128, WB], f32, tag="expH")
            nc.vector.scalar_tensor_tensor(
                out=H, in0=X, scalar=1.0 / EXP_DEG, in1=IW["32"],
                op0=OP.mult, op1=OP.add)
            for k in range(EXP_DEG - 1, 0, -1):
                psx = ps_mid.tile([128, WB], mybir.dt.float32, tag="ps2")
                mm_pairs(psx, X, H)
                H2 = chain.tile([128, WB], f32, tag="expH")
                nc.vector.tensor_scalar_mul(H2, psx, 1.0 / k)
                nc.vector.tensor_add(H2, H2, IW["32"])
                H = H2
            for sq in range(EXP_SQ):
                psx = ps_mid.tile([128, WB], mybir.dt.float32, tag="ps2")
                mm_pairs(psx, H, H)
                if sq < EXP_SQ - 1:
                    H2 = chain.tile([128, WB], f32, tag="expH")
                    nc.vector.tensor_copy(out=H2, in_=psx)
                    H = H2
                else:
                    nc.vector.tensor_scalar_mul(outS1[:, cs], psx, C_NORM)

        o3 = out_ap[s].rearrange("(pr two) r c -> two r pr c", two=2)
        nc.sync.dma_start(
            out=o3[0], in_=outS1[0:64, :].rearrange("p (pr c) -> p pr c", c=64))
        nc.sync.dma_start(
            out=o3[1], in_=outS1[64:128, :].rearrange("p (pr c) -> p pr c", c=64))


def build(nsamp=NSAMP, num_devices=NCORES):
    import concourse.bacc as bacc
    import concourse.mybir as mybir
    import concourse.tile as tile

    nc = bacc.Bacc("TRN2", target_bir_lowering=False, debug=False,
                   num_devices=num_devices)
    f32 = mybir.dt.float32
    x_ap = nc.dram_tensor("x", [nsamp, M, DIN, DIN], f32, kind="ExternalInput").ap()
    wq = nc.dram_tensor("wq", [DIN, DOUT], f32, kind="ExternalInput").ap()
    wk = nc.dram_tensor("wk", [DIN, DOUT], f32, kind="ExternalInput").ap()
    wv = nc.dram_tensor("wv", [DIN, DOUT], f32, kind="ExternalInput").ap()
    out = nc.dram_tensor("out", [nsamp, M, DOUT, DOUT], f32, kind="ExternalOutput").ap()

    tapspec = {}
    if DEBUG:
        tapspec = {
            "irep32": [128, 64], "initV": [128, NP_PAIR * 64],
            "flatV": [64, M * 64], "flatQ": [64, M * 64],
            "Et": [64, 64], "expS": [64, 64],
            "M2": [64, M * 64], "S1M": [128, NP_PAIR * 64],
        }
        for k in (3, 5, 7, 8, 11, 14, 17, 20):
            tapspec[f"chainYW{k}"] = [128, PAIR_BATCH * 128]
        tapspec["serE"] = [128, PAIR_BATCH * 64]
        tapspec["serE4"] = [128, PAIR_BATCH * 64]
        tapspec["serB0"] = [128, PAIR_BATCH * 64]
    taps = {k: nc.dram_tensor("tap_" + k, v, f32 if k != "flatQ" and k != "expS" else mybir.dt.float16,
                              kind="ExternalOutput").ap()
            for k, v in tapspec.items()}
    with tile.TileContext(nc) as tc, ExitStack() as ctx:
        emit_kernel(nc, tc, ctx, x_ap, wq, wk, wv, out, nsamp=nsamp, taps=taps)
    nc.compile()
    return nc


_CACHED = {}


def _get_nc(nsamp):
    from concourse.bass_interp import get_hw_module
    if nsamp not in _CACHED:
        nc = build(nsamp=nsamp)
        nc.m = get_hw_module(nc.m)
        _CACHED[nsamp] = nc
    return _CACHED[nsamp]


def kernel(x, Wq, Wk, Wv):
    from concourse.bass_utils import run_bass_kernel_spmd

    bs = x.shape[0]
    nsamp = bs // NCORES
    nc = _get_nc(nsamp)
    in_maps = []
    for c in range(NCORES):
        in_maps.append({
            "x": np.ascontiguousarray(x[c * nsamp:(c + 1) * nsamp], dtype=np.float32),
            "wq": np.ascontiguousarray(Wq, dtype=np.float32),
            "wk": np.ascontiguousarray(Wk, dtype=np.float32),
            "wv": np.ascontiguousarray(Wv, dtype=np.float32),
        })
    res = run_bass_kernel_spmd(nc, in_maps, list(range(NCORES)))
    outs = [res.results[c]["out"] for c in range(NCORES)]
    full = np.concatenate(outs, axis=0)
    return full.reshape(bs * M, DOUT, DOUT).astype(np.float32)


# revision 30
# speedup vs baseline: 1.9557x; 1.0109x over previous
"""Trainium2 Bass kernel for nn_AttentionManifold (SPD manifold attention).

For each of bs*m=2048 SPD matrices X (100x100): Q/K/V = W^T X W (64x64),
logQ/K/V = matrix log, log-Euclidean attention (Frobenius distances ->
scores -> softmax over K index), mixed = prob-weighted sum of logV,
out = matrix exp(mixed).

Matrix log via tuned Newton-Schulz sqrt chain (4 levels, R = (A/16)^(1/16),
log A = 16 log R + log16*I; the global log16*I terms cancel in the
attention distances and fold into a final *16 output scale), log R via a
degree-14 series (Paterson-Stockmeyer), exp via scaling-squaring (k=5,
degree-7 Taylor).  Q/K paths use fp16 matmuls (scores are insensitive);
V path, congruence mm1 and exp use fp32 matmuls.

Sharding: pure data parallelism, bs=32 -> 4 samples per NeuronCore.
"""
import numpy as np
from contextlib import ExitStack

C_NORM = 16.0
SCHED = [
    [(24.871321977, -35.245186442),
     (1.605560380, -0.024430481),
     (1.595838197, -0.060908024),
     (1.576384611, -0.143218467),
     (1.543497701, -0.291162661),
     (1.511244305, -0.443655343),
     (1.5, -0.5), (1.5, -0.5)],
    [(6.228647233, -6.864010667),
     (1.554009519, -0.242273245),
     (1.518749014, -0.406941447),
     (1.5, -0.5), (1.5, -0.5), (1.5, -0.5)],
    [(3.051424190, -2.460263319),
     (1.508484255, -0.457724181),
     (1.5, -0.5), (1.5, -0.5)],
    [(2.128257338, -1.230895381),
     (1.5, -0.5), (1.5, -0.5)],
]
EXP_DEG = 6
EXP_SQ = 5
DEBUG = False

BS, M, DIN, DOUT = 32, 64, 100, 64
NCORES = 8
NSAMP = BS // NCORES
NP_PAIR = M // 2
PAIR_BATCH = 4
NBATCH = NP_PAIR // PAIR_BATCH


def _flat_sched(nlevels=4):
    out = []
    for steps in SCHED[:nlevels]:
        for j, (a, b) in enumerate(steps):
            out.append((j == 0, a, b))
    return out


def emit_kernel(nc, tc, ctx, x_ap, wq_ap, wk_ap, wv_ap, out_ap, nsamp=NSAMP, taps=None):
    def tap(name, t):
        if taps is not None and name in taps:
            nc.sync.dma_start(out=taps[name], in_=t)
    import concourse.mybir as mybir
    from concourse.bass import ds, ts
    from concourse.masks import make_identity

    f32 = mybir.dt.float32
    f16 = mybir.dt.float16
    AX = mybir.AxisListType
    OP = mybir.AluOpType
    ACT = mybir.ActivationFunctionType
    WB = PAIR_BATCH * 64

    const = ctx.enter_context(tc.tile_pool(name="const", bufs=1))
    work = ctx.enter_context(tc.tile_pool(name="work", bufs=2))
    big = ctx.enter_context(tc.tile_pool(name="big", bufs=1))
    init2 = ctx.enter_context(tc.tile_pool(name="init2", bufs=2))
    logs = ctx.enter_context(tc.tile_pool(name="logs", bufs=2))
    chain = ctx.enter_context(tc.tile_pool(name="chain", bufs=2))
    ser = ctx.enter_context(tc.tile_pool(name="ser", bufs=1))
    chainP = ctx.enter_context(tc.tile_pool(name="chainP", bufs=2))
    ps_small = ctx.enter_context(tc.tile_pool(name="ps_s", bufs=1, space="PSUM"))
    ps_big = ctx.enter_context(tc.tile_pool(name="ps_b", bufs=2, space="PSUM"))
    ps_mid = ctx.enter_context(tc.tile_pool(name="ps_m", bufs=1, space="PSUM"))

    # ---------------- constants ----------------
    W3 = const.tile([DIN, 3 * DOUT], f32)
    nc.sync.dma_start(out=W3[:, 0:DOUT], in_=wq_ap)
    nc.sync.dma_start(out=W3[:, DOUT:2 * DOUT], in_=wk_ap)
    nc.sync.dma_start(out=W3[:, 2 * DOUT:3 * DOUT], in_=wv_ap)
    WQh = const.tile([DIN, DOUT], f16)
    WKh = const.tile([DIN, DOUT], f16)
    nc.vector.tensor_copy(out=WQh, in_=W3[:, 0:DOUT])
    nc.vector.tensor_copy(out=WKh, in_=W3[:, DOUT:2 * DOUT])

    IREP16 = const.tile([128, 64], f16)
    IREP32 = const.tile([128, 64], f32)
    for t in (IREP16, IREP32):
        make_identity(nc, t[0:64, :])
        make_identity(nc, t[64:128, :])
    # widened identity / block-coefficient tiles [128, WB]
    IW = {}
    for dt_, rep, tag in ((f16, IREP16, "16"), (f32, IREP32, "32")):
        w = const.tile([128, WB], dt_, tag=f"IW{tag}")
        for p in range(PAIR_BATCH):
            nc.vector.tensor_copy(out=w[:, ts(p, 64)], in_=rep)
        IW[tag] = w
    IWD = {}
    for dt_, rep, tag in ((f16, IREP16, "16"), (f32, IREP32, "32")):
        w = const.tile([128, PAIR_BATCH * 128], dt_, tag=f"IWD{tag}")
        for p in range(2 * PAIR_BATCH):
            nc.vector.tensor_copy(out=w[:, ts(p, 64)], in_=rep)
        IWD[tag] = w
    cI = {}
    for tag in ("16", "32"):
        for k in (4, 8, 12):
            dt_ = f16 if tag == "16" else f32
            t = const.tile([128, WB], dt_, tag=f"cI{tag}_{k}")
            nc.vector.tensor_scalar_mul(t, IW[tag], 1.0 / k)
            cI[(tag, k)] = t

    ones_col = const.tile([64, 1], f32)
    nc.vector.memset(ones_col, 1.0)
    ones_col_h = const.tile([64, 1], f16)
    nc.vector.memset(ones_col_h, 32.0)      # folds the /32 exp prescale
    ones_row = const.tile([1, 64], f32)
    nc.vector.memset(ones_row, 1.0)
    bias_ln = const.tile([64, 1], f32)
    nc.vector.memset(bias_ln, 1.0 + 64e-6)
    bias_one = const.tile([64, 1], f32)
    nc.vector.memset(bias_one, 1.0)

    FS4 = _flat_sched(4)
    FS3 = _flat_sched(3)

    def mm_pairs(out_ps, lhs_t, rhs_t, ncols=64):
        for p in range(PAIR_BATCH):
            for h in (0, 1):
                nc.tensor.matmul(
                    out_ps[h * 64:h * 64 + 64, ts(p, ncols)],
                    lhs_t[h * 64:h * 64 + 64, ts(p, 64)],
                    rhs_t[h * 64:h * 64 + 64, ts(p, ncols)],
                    start=True, stop=True)

    def chain_and_series(init_t, dt_, tag, flat_t, b):
        # generator: yields after each NS step so Q/K/V emission interleaves
        # V runs level 1 in fp32 (ill-conditioned state), then fp16.
        # Q/K use 3 sqrt levels (log scale 8), V uses 4 (scale 16).
        FS = FS4 if dt_ == f32 else FS3
        lscale = -16.0 if dt_ == f32 else -8.0
        irep = IW["16" if dt_ == f16 else "32"]
        ctag = "16" if dt_ == f16 else "32"
        if True:
            cs = ds(b * WB, WB)
            # state quad [Y | Yt | Z | Zt] per pair, 256 cols each
            SQ = chain.tile([128, PAIR_BATCH * 256], dt_, tag=f"SQ{tag}")
            sq4 = SQ.rearrange("p (n f c) -> p n f c", f=4, c=64)
            iv = init_t[:, cs].rearrange("p (n c) -> p n c", c=64)
            nc.vector.tensor_copy(out=sq4[:, :, 0, :], in_=iv)
            nc.vector.tensor_copy(out=sq4[:, :, 1, :], in_=iv)
            ir3 = irep.rearrange("p (n c) -> p n c", c=64)

            def qmm(out_ps, oslice, lhs4, li, rhs4, ri):
                for p in range(PAIR_BATCH):
                    for h in (0, 1):
                        nc.tensor.matmul(
                            out_ps[h * 64:h * 64 + 64, p * oslice[1] + oslice[0] * 64:
                                   p * oslice[1] + oslice[0] * 64 + 64],
                            lhs4[h * 64:h * 64 + 64, p * 256 + li * 64:p * 256 + li * 64 + 64],
                            rhs4[h * 64:h * 64 + 64, p * 256 + ri * 64:p * 256 + ri * 64 + 64] if ri is not None
                            else rhs4[h * 64:h * 64 + 64, ts(p, 64)],
                            start=True, stop=True)

            for k_idx, (lvl_start, al, be) in enumerate(FS):
                if lvl_start and k_idx == 8 and dt_ == f32:
                    # V-path precision drop: fp32 -> fp16 from level 2 on
                    dt_ = f16
                    irep = IW["16"]
                    ctag = "16"
                    ir3 = irep.rearrange("p (n c) -> p n c", c=64)
                    SQn = chain.tile([128, PAIR_BATCH * 256], dt_, tag=f"SQ{tag}")
                    sqn4 = SQn.rearrange("p (n f c) -> p n f c", f=4, c=64)
                    nc.vector.tensor_copy(out=sqn4[:, :, 0, :], in_=sq4[:, :, 0, :])
                    nc.vector.tensor_copy(out=sqn4[:, :, 1, :], in_=sq4[:, :, 1, :])
                    SQ, sq4 = SQn, sqn4
                if lvl_start:
                    # Z = I here, so W = Y and the level-start step needs no
                    # W-matmuls: P = aI + b*Y (from SBUF), and Z' = P Z = P.
                    Pb = chainP.tile([128, 2 * WB], dt_, tag=f"Pb{tag}")
                    pb3 = Pb.rearrange("p (n f c) -> p n f c", f=2, c=64)
                    nc.scalar.activation(out=pb3, in_=sq4[:, :, 0:2, :],
                                         func=ACT.Copy, bias=0.0, scale=be)
                    nc.vector.scalar_tensor_tensor(
                        out=Pb, in0=IWD[ctag], scalar=al,
                        in1=Pb, op0=OP.mult, op1=OP.add)
                else:
                    # W = Zt^T Y ; Wt = Y^T Zt
                    psA = ps_big.tile([128, PAIR_BATCH * 128], mybir.dt.float32, tag="psA")
                    qmm(psA, (0, 128), SQ, 3, SQ, 0)
                    qmm(psA, (1, 128), SQ, 0, SQ, 3)
                    Pb = chainP.tile([128, 2 * WB], dt_, tag=f"Pb{tag}")
                    nc.scalar.activation(out=Pb, in_=psA, func=ACT.Copy,
                                         bias=0.0, scale=be)
                    nc.vector.scalar_tensor_tensor(
                        out=Pb, in0=IWD[ctag], scalar=al,
                        in1=Pb, op0=OP.mult, op1=OP.add)
                # P = Pb[...,0], Pt = Pb[...,1]
                # Yn = Yt^T P ; Ytn = P^T Yt ; Zn = Pt^T Z ; Ztn = Z^T Pt
                psB = ps_big.tile([128, PAIR_BATCH * 128], mybir.dt.float32, tag="psB")
                for p in range(PAIR_BATCH):
                    for h in (0, 1):
                        hs = slice(h * 64, h * 64 + 64)
                        yt = SQ[hs, p * 256 + 64:p * 256 + 128]
                        pp = Pb[hs, p * 128:p * 128 + 64]
                        nc.tensor.matmul(psB[hs, p * 128:p * 128 + 64], yt, pp,
                                         start=True, stop=True)
                        nc.tensor.matmul(psB[hs, p * 128 + 64:p * 128 + 128], pp, yt,
                                         start=True, stop=True)
                SQ2 = chain.tile([128, PAIR_BATCH * 256], dt_, tag=f"SQ{tag}")
                sq24 = SQ2.rearrange("p (n f c) -> p n f c", f=4, c=64)
                psBr = psB.rearrange("p (n f c) -> p n f c", f=2, c=64)
                nc.scalar.activation(out=sq24[:, :, 0:2, :], in_=psBr,
                                     func=ACT.Copy, bias=0.0, scale=1.0)
                if lvl_start:
                    nc.vector.tensor_copy(
                        out=sq24[:, :, 2:4, :],
                        in_=Pb.rearrange("p (n f c) -> p n f c", f=2, c=64))
                else:
                    psC = ps_big.tile([128, PAIR_BATCH * 128], mybir.dt.float32, tag="psC")
                    for p in range(PAIR_BATCH):
                        for h in (0, 1):
                            hs = slice(h * 64, h * 64 + 64)
                            z = SQ[hs, p * 256 + 128:p * 256 + 192]
                            zt = SQ[hs, p * 256 + 192:p * 256 + 256]
                            pt = Pb[hs, p * 128 + 64:p * 128 + 128]
                            nc.tensor.matmul(psC[hs, p * 128:p * 128 + 64], pt, z,
                                             start=True, stop=True)
                            nc.tensor.matmul(psC[hs, p * 128 + 64:p * 128 + 128], z, pt,
                                             start=True, stop=True)
                    psCr = psC.rearrange("p (n f c) -> p n f c", f=2, c=64)
                    nc.vector.tensor_copy(out=sq24[:, :, 2:4, :], in_=psCr)
                SQ, sq4 = SQ2, sq24
                yield
            # R = (Y + Yt)/2 ; E = I - R
            E = ser.tile([128, WB], dt_, tag=f"E{tag}")
            e3 = E.rearrange("p (n c) -> p n c", c=64)
            nc.vector.tensor_add(e3, sq4[:, :, 0, :], sq4[:, :, 1, :])
            nc.vector.scalar_tensor_tensor(
                out=e3, in0=e3, scalar=-0.5,
                in1=ir3, op0=OP.mult, op1=OP.add)
            if ctag == "32" and b == 0:
                tap("serE", E)
            psE = ps_mid.tile([128, WB], mybir.dt.float32, tag="ps2")
            mm_pairs(psE, E, E)
            E2 = ser.tile([128, WB], dt_, tag=f"E2{tag}")
            nc.vector.tensor_copy(out=E2, in_=psE)
            psE3 = ps_mid.tile([128, WB], mybir.dt.float32, tag="ps2")
            mm_pairs(psE3, E2, E)
            E3 = ser.tile([128, WB], dt_, tag=f"E3{tag}")
            nc.vector.tensor_copy(out=E3, in_=psE3)
            yield
            psE4 = ps_mid.tile([128, WB], mybir.dt.float32, tag="ps2")
            mm_pairs(psE4, E2, E2)
            E4 = ser.tile([128, WB], dt_, tag=f"E4{tag}")
            nc.vector.tensor_copy(out=E4, in_=psE4)
            if ctag == "32" and b == 0:
                tap("serE4", E4)
            B = ser.tile([128, WB], dt_, tag=f"B{tag}")
            nc.vector.scalar_tensor_tensor(out=B, in0=E, scalar=1.0 / 13, in1=cI[(ctag, 12)], op0=OP.mult, op1=OP.add)
            nc.vector.scalar_tensor_tensor(out=B, in0=E2, scalar=1.0 / 14, in1=B, op0=OP.mult, op1=OP.add)
            psH = ps_mid.tile([128, WB], mybir.dt.float32, tag="ps2")
            mm_pairs(psH, E4, B)
            H = ser.tile([128, WB], dt_, tag=f"B{tag}")
            nc.vector.scalar_tensor_tensor(out=H, in0=E, scalar=1.0 / 9, in1=cI[(ctag, 8)], op0=OP.mult, op1=OP.add)
            nc.vector.scalar_tensor_tensor(out=H, in0=E2, scalar=1.0 / 10, in1=H, op0=OP.mult, op1=OP.add)
            nc.vector.scalar_tensor_tensor(out=H, in0=E3, scalar=1.0 / 11, in1=H, op0=OP.mult, op1=OP.add)
            yield
            Hs = ser.tile([128, WB], dt_, tag=f"Hs{tag}")
            nc.vector.tensor_copy(out=Hs, in_=psH)
            nc.vector.tensor_add(H, H, Hs)
            psH2 = ps_mid.tile([128, WB], mybir.dt.float32, tag="ps2")
            mm_pairs(psH2, E4, H)
            H2 = ser.tile([128, WB], dt_, tag=f"B{tag}")
            nc.vector.scalar_tensor_tensor(out=H2, in0=E, scalar=1.0 / 5, in1=cI[(ctag, 4)], op0=OP.mult, op1=OP.add)
            nc.vector.scalar_tensor_tensor(out=H2, in0=E2, scalar=1.0 / 6, in1=H2, op0=OP.mult, op1=OP.add)
            nc.vector.scalar_tensor_tensor(out=H2, in0=E3, scalar=1.0 / 7, in1=H2, op0=OP.mult, op1=OP.add)
            Hs2 = ser.tile([128, WB], dt_, tag=f"Hs{tag}")
            nc.vector.tensor_copy(out=Hs2, in_=psH2)
            nc.vector.tensor_add(H2, H2, Hs2)
            psH3 = ps_mid.tile([128, WB], mybir.dt.float32, tag="ps2")
            mm_pairs(psH3, E4, H2)
            B0 = ser.tile([128, WB], dt_, tag=f"B{tag}")
            nc.vector.tensor_scalar_mul(B0, E2, 0.5)
            nc.vector.scalar_tensor_tensor(out=B0, in0=E3, scalar=1.0 / 3, in1=B0, op0=OP.mult, op1=OP.add)
            nc.vector.tensor_add(B0, B0, E)
            Hs3 = ser.tile([128, WB], dt_, tag=f"Hs{tag}")
            nc.vector.tensor_copy(out=Hs3, in_=psH3)
            nc.vector.tensor_add(B0, B0, Hs3)
            if ctag == "32" and b == 0:
                tap("serB0", B0)
            LS = logs.tile([128, WB], flat_t.dtype, tag=f"LS{tag}")
            nc.scalar.activation(out=LS, in_=B0, func=ACT.Copy,
                                 bias=0.0, scale=lscale)
            flat3 = flat_t.rearrange("p (n two c) -> p n two c", two=2, c=64)
            nc.vector.tensor_copy(
                out=flat3[:, ds(b * PAIR_BATCH, PAIR_BATCH), 0, :],
                in_=LS[0:64, :].rearrange("p (n c) -> p n c", c=64))
            nc.gpsimd.dma_start(
                out=flat3[:, ds(b * PAIR_BATCH, PAIR_BATCH), 1, :],
                in_=LS[64:128, :].rearrange("p (n c) -> p n c", c=64))

    # ======================== per-sample pipeline ========================
    for s in range(nsamp):
        initQ = init2.tile([128, NP_PAIR * 64], f16, tag="initQ")
        initK = init2.tile([128, NP_PAIR * 64], f16, tag="initK")
        initV = init2.tile([128, NP_PAIR * 64], f32, tag="initV")
        oddQ = init2.tile([64, NP_PAIR * 64], f16, tag="oddQ")
        oddK = init2.tile([64, NP_PAIR * 64], f16, tag="oddK")
        oddV = init2.tile([64, NP_PAIR * 64], f32, tag="oddV")

        for it in range(M):
            if it % 16 == 0:
                xbuf = work.tile([DIN, 16 * DIN], f32, tag="xbuf")
                nc.sync.dma_start(
                    out=xbuf.rearrange("p (i c) -> p i c", c=DIN),
                    in_=x_ap[s, ds(it, 16)].rearrange("i p c -> p i c"))
            p1 = ps_mid.tile([DIN, 3 * DOUT], mybir.dt.float32, tag="ps2")
            nc.tensor.matmul(p1, xbuf[:, ts(it % 16, DIN)], W3, start=True, stop=True)
            P1qk = work.tile([DIN, 2 * DOUT], f16, tag="p1qk")
            nc.vector.tensor_copy(out=P1qk, in_=p1[:, 0:2 * DOUT])
            P1v = work.tile([DIN, DOUT], f32, tag="p1v")
            nc.vector.tensor_copy(out=P1v, in_=p1[:, 2 * DOUT:3 * DOUT])
            pqkv = ps_small.tile([64, 192], mybir.dt.float32, tag="small")
            nc.tensor.matmul(pqkv[:, 0:64], WQh, P1qk[:, 0:DOUT], start=True, stop=True)
            nc.tensor.matmul(pqkv[:, 64:128], WKh, P1qk[:, DOUT:2 * DOUT], start=True, stop=True)
            nc.tensor.matmul(pqkv[:, 128:192], W3[:, 2 * DOUT:3 * DOUT], P1v, start=True, stop=True)
            pr, h = it // 2, it % 2
            for ci, (init_t, odd_t) in enumerate(((initQ, oddQ), (initK, oddK), (initV, oddV))):
                src = pqkv[:, ci * 64:(ci + 1) * 64]
                if h == 0:
                    nc.scalar.activation(out=init_t[0:64, ts(pr, 64)], in_=src,
                                         func=ACT.Copy, bias=0.0, scale=1.0 / C_NORM)
                else:
                    nc.scalar.activation(out=odd_t[:, ts(pr, 64)], in_=src,
                                         func=ACT.Copy, bias=0.0, scale=1.0 / C_NORM)
        for init_t, odd_t in ((initQ, oddQ), (initK, oddK), (initV, oddV)):
            nc.gpsimd.dma_start(out=init_t[64:128, :], in_=odd_t)
        if s == 0:
            tap("irep32", IREP32)
            tap("initV", initV)

        flatQ = big.tile([64, M * 64], f16, tag="flatQ")
        flatK = big.tile([64, M * 64], f16, tag="flatK")
        flatV = big.tile([64, M * 64], f32, tag="f32scr")
        for b in range(NBATCH):
            gens = [chain_and_series(initQ, f16, "q", flatQ, b),
                    chain_and_series(initK, f16, "k", flatK, b),
                    chain_and_series(initV, f32, "v", flatV, b)]
            while gens:
                gens = [g for g in gens if next(g, StopIteration) is not StopIteration]
        if s == 0:
            tap("flatV", flatV)
            tap("flatQ", flatQ)

        # ---------------- attention ----------------
        partQ = work.tile([64, M], f32, tag="partQ")
        partK = work.tile([64, M], f32, tag="partK")
        for flat_t, part_t in ((flatQ, partQ), (flatK, partK)):
            sq = big.tile([64, M * 64], f32, tag="VF")
            nc.vector.tensor_mul(sq, flat_t, flat_t)
            nc.vector.tensor_reduce(
                out=part_t, in_=sq.rearrange("p (i c) -> p i c", c=64),
                axis=AX.X, op=OP.add)
        ps_qn = ps_small.tile([1, 64], mybir.dt.float32, tag="small")
        nc.tensor.matmul(ps_qn, ones_col, partQ, start=True, stop=True)
        qn_row = work.tile([1, 64], f32, tag="qnrow_sb")
        nc.vector.tensor_copy(out=qn_row, in_=ps_qn)
        ps_kn = ps_small.tile([64, 1], mybir.dt.float32, tag="small")
        nc.tensor.matmul(ps_kn, partK, ones_col, start=True, stop=True)
        kn_col = work.tile([64, 1], f32, tag="kncol_sb")
        nc.vector.tensor_copy(out=kn_col, in_=ps_kn)
        ps_qrep = ps_small.tile([64, 64], mybir.dt.float32, tag="small")
        nc.tensor.matmul(ps_qrep, ones_row, qn_row, start=True, stop=True)
        qrep = work.tile([64, 64], f32, tag="qrep_sb")
        nc.vector.tensor_copy(out=qrep, in_=ps_qrep)

        ps_cross = ps_small.tile([64, 64], mybir.dt.float32, tag="small")
        fQ3 = flatQ.rearrange("p (i c) -> p c i", c=64)
        fK3 = flatK.rearrange("p (i c) -> p c i", c=64)
        for c in range(64):
            nc.tensor.matmul(ps_cross, fK3[:, c, :], fQ3[:, c, :],
                             start=(c == 0), stop=(c == 63))
        cross_sb = work.tile([64, 64], f32, tag="cross_sb")
        nc.vector.tensor_copy(out=cross_sb, in_=ps_cross)
        Et = work.tile([64, 64], f32, tag="Et")
        nc.vector.scalar_tensor_tensor(out=Et, in0=cross_sb, scalar=-2.0,
                                       in1=qrep, op0=OP.mult, op1=OP.add)
        nc.vector.tensor_scalar(out=Et, in0=Et, scalar1=kn_col, scalar2=0.0,
                                op0=OP.add, op1=OP.max)
        lnE = work.tile([64, 64], f32, tag="lnE")
        nc.scalar.activation(out=lnE, in_=Et, func=ACT.Ln,
                             bias=bias_ln, scale=1.0)
        ln1 = work.tile([64, 64], f32, tag="ln1")
        nc.vector.tensor_scalar_add(ln1, lnE, 1.0)
        sc = work.tile([64, 64], f32, tag="sc")
        nc.vector.reciprocal(out=sc, in_=ln1)
        expS = work.tile([64, 64], f16, tag="expS")
        nc.scalar.activation(out=expS, in_=sc, func=ACT.Exp, bias=0.0, scale=1.0)
        if s == 0:
            tap("Et", Et)
            tap("expS", expS)
        ps_cs = ps_small.tile([64, 1], mybir.dt.float32, tag="small")
        nc.tensor.matmul(ps_cs, expS, ones_col_h, start=True, stop=True)
        inv = work.tile([64, 1], f32, tag="inv")
        nc.vector.reciprocal(out=inv, in_=ps_cs)

        VF = big.tile([64, M * 64], f32, tag="VF")
        VF3 = VF.rearrange("p (r c) -> p r c", c=64)
        for r in range(64):
            nc.gpsimd.dma_start(
                out=VF3[:, r:r + 1, :],
                in_=flatV[r:r + 1, :].rearrange("p (i c) -> p i c", c=64))
        expS32 = work.tile([64, 64], f32, tag="expS32")
        nc.vector.tensor_copy(out=expS32, in_=expS)
        M2 = big.tile([64, M * 64], f32, tag="f32scr")
        for ch in range(8):
            ps_m2 = ps_small.tile([64, 512], mybir.dt.float32, tag="small")
            nc.tensor.matmul(ps_m2, expS32, VF[:, ts(ch, 512)], start=True, stop=True)
            nc.vector.tensor_scalar_mul(M2[:, ts(ch, 512)], ps_m2, inv)

        S1M = big.tile([128, NP_PAIR * 64], f32, tag="scr8c")
        for j in range(M):
            pr, h = j // 2, j % 2
            nc.gpsimd.dma_start(
                out=S1M[h * 64:h * 64 + 64, ts(pr, 64)].rearrange("p (o c) -> p o c", o=1),
                in_=M2[j:j + 1, :].rearrange("p (r c) -> p r c", c=64))

        if s == 0:
            tap("M2", M2)
            tap("S1M", S1M)
        # ---------------- exp: scaling-squaring ----------------
        outS1 = big.tile([128, NP_PAIR * 64], f32, tag="outS1")
        for b in range(NBATCH):
            cs = ds(b * WB, WB)
            X = S1M[:, cs]
            H = chain.tile([128, WB], f32, tag="expH")
            nc.vector.scalar_tensor_tensor(
                out=H, in0=X, scalar=1.0 / EXP_DEG, in1=IW["32"],
                op0=OP.mult, op1=OP.add)
            for k in range(EXP_DEG - 1, 0, -1):
                psx = ps_mid.tile([128, WB], mybir.dt.float32, tag="ps2")
                mm_pairs(psx, X, H)
                H2 = chain.tile([128, WB], f32, tag="expH")
                nc.vector.tensor_scalar_mul(H2, psx, 1.0 / k)
                nc.vector.tensor_add(H2, H2, IW["32"])
                H = H2
            for sq in range(EXP_SQ):
                psx = ps_mid.tile([128, WB], mybir.dt.float32, tag="ps2")
                mm_pairs(psx, H, H)
                if sq < EXP_SQ - 1:
                    H2 = chain.tile([128, WB], f32, tag="expH")
                    nc.vector.tensor_copy(out=H2, in_=psx)
                    H = H2
                else:
                    nc.vector.tensor_scalar_mul(outS1[:, cs], psx, C_NORM)

        o3 = out_ap[s].rearrange("(pr two) r c -> two r pr c", two=2)
        nc.sync.dma_start(
            out=o3[0], in_=outS1[0:64, :].rearrange("p (pr c) -> p pr c", c=64))
        nc.sync.dma_start(
            out=o3[1], in_=outS1[64:128, :].rearrange("p (pr c) -> p pr c", c=64))


def build(nsamp=NSAMP, num_devices=NCORES):
    import concourse.bacc as bacc
    import concourse.mybir as mybir
    import concourse.tile as tile

    nc = bacc.Bacc("TRN2", target_bir_lowering=False, debug=False,
                   num_devices=num_devices)
    f32 = mybir.dt.float32
    x_ap = nc.dram_tensor("x", [nsamp, M, DIN, DIN], f32, kind="ExternalInput").ap()
    wq = nc.dram_tensor("wq", [DIN, DOUT], f32, kind="ExternalInput").ap()
    wk = nc.dram_tensor("wk", [DIN, DOUT], f32, kind="ExternalInput").ap()
    wv = nc.dram_tensor("wv", [DIN, DOUT], f32, kind="ExternalInput").ap()
    out = nc.dram_tensor("out", [nsamp, M, DOUT, DOUT], f32, kind="ExternalOutput").ap()

    tapspec = {}
    if DEBUG:
        tapspec = {
            "irep32": [128, 64], "initV": [128, NP_PAIR * 64],
            "flatV": [64, M * 64], "flatQ": [64, M * 64],
            "Et": [64, 64], "expS": [64, 64],
            "M2": [64, M * 64], "S1M": [128, NP_PAIR * 64],
        }
        for k in (3, 5, 7, 8, 11, 14, 17, 20):
            tapspec[f"chainYW{k}"] = [128, PAIR_BATCH * 128]
        tapspec["serE"] = [128, PAIR_BATCH * 64]
        tapspec["serE4"] = [128, PAIR_BATCH * 64]
        tapspec["serB0"] = [128, PAIR_BATCH * 64]
    taps = {k: nc.dram_tensor("tap_" + k, v, f32 if k != "flatQ" and k != "expS" else mybir.dt.float16,
                              kind="ExternalOutput").ap()
            for k, v in tapspec.items()}
    with tile.TileContext(nc) as tc, ExitStack() as ctx:
        emit_kernel(nc, tc, ctx, x_ap, wq, wk, wv, out, nsamp=nsamp, taps=taps)
    nc.compile()
    return nc


_CACHED = {}


def _get_nc(nsamp):
    from concourse.bass_interp import get_hw_module
    if nsamp not in _CACHED:
        nc = build(nsamp=nsamp)
        nc.m = get_hw_module(nc.m)
        _CACHED[nsamp] = nc
    return _CACHED[nsamp]


def kernel(x, Wq, Wk, Wv):
    from concourse.bass_utils import run_bass_kernel_spmd

    bs = x.shape[0]
    nsamp = bs // NCORES
    nc = _get_nc(nsamp)
    in_maps = []
    for c in range(NCORES):
        in_maps.append({
            "x": np.ascontiguousarray(x[c * nsamp:(c + 1) * nsamp], dtype=np.float32),
            "wq": np.ascontiguousarray(Wq, dtype=np.float32),
            "wk": np.ascontiguousarray(Wk, dtype=np.float32),
            "wv": np.ascontiguousarray(Wv, dtype=np.float32),
        })
    res = run_bass_kernel_spmd(nc, in_maps, list(range(NCORES)))
    outs = [res.results[c]["out"] for c in range(NCORES)]
    full = np.concatenate(outs, axis=0)
    return full.reshape(bs * M, DOUT, DOUT).astype(np.float32)


# revision 34
# speedup vs baseline: 1.9685x; 1.0066x over previous
"""Trainium2 Bass kernel for nn_AttentionManifold (SPD manifold attention).

For each of bs*m=2048 SPD matrices X (100x100): Q/K/V = W^T X W (64x64),
logQ/K/V = matrix log, log-Euclidean attention (Frobenius distances ->
scores -> softmax over K index), mixed = prob-weighted sum of logV,
out = matrix exp(mixed).

Matrix log via tuned Newton-Schulz sqrt chain (4 levels, R = (A/16)^(1/16),
log A = 16 log R + log16*I; the global log16*I terms cancel in the
attention distances and fold into a final *16 output scale), log R via a
degree-14 series (Paterson-Stockmeyer), exp via scaling-squaring (k=5,
degree-7 Taylor).  Q/K paths use fp16 matmuls (scores are insensitive);
V path, congruence mm1 and exp use fp32 matmuls.

Sharding: pure data parallelism, bs=32 -> 4 samples per NeuronCore.
"""
import numpy as np
from contextlib import ExitStack

C_NORM = 16.0
SCHED = [
    [(24.871321977, -35.245186442),
     (1.605560380, -0.024430481),
     (1.595838197, -0.060908024),
     (1.576384611, -0.143218467),
     (1.543497701, -0.291162661),
     (1.511244305, -0.443655343),
     (1.5, -0.5), (1.5, -0.5)],
    [(6.228647233, -6.864010667),
     (1.554009519, -0.242273245),
     (1.518749014, -0.406941447),
     (1.5, -0.5), (1.5, -0.5), (1.5, -0.5)],
    [(3.051424190, -2.460263319),
     (1.508484255, -0.457724181),
     (1.5, -0.5), (1.5, -0.5)],
    [(2.128257338, -1.230895381),
     (1.5, -0.5), (1.5, -0.5)],
]
EXP_DEG = 6
EXP_SQ = 5
DEBUG = False

BS, M, DIN, DOUT = 32, 64, 100, 64
NCORES = 8
NSAMP = BS // NCORES
NP_PAIR = M // 2
PAIR_BATCH = 4
NBATCH = NP_PAIR // PAIR_BATCH


def _flat_sched(nlevels=4):
    out = []
    for steps in SCHED[:nlevels]:
        for j, (a, b) in enumerate(steps):
            out.append((j == 0, a, b))
    return out


def emit_kernel(nc, tc, ctx, x_ap, wq_ap, wk_ap, wv_ap, out_ap, nsamp=NSAMP, taps=None):
    def tap(name, t):
        if taps is not None and name in taps:
            nc.sync.dma_start(out=taps[name], in_=t)
    import concourse.mybir as mybir
    from concourse.bass import ds, ts
    from concourse.masks import make_identity

    f32 = mybir.dt.float32
    f16 = mybir.dt.float16
    AX = mybir.AxisListType
    OP = mybir.AluOpType
    ACT = mybir.ActivationFunctionType
    WB = PAIR_BATCH * 64

    const = ctx.enter_context(tc.tile_pool(name="const", bufs=1))
    work = ctx.enter_context(tc.tile_pool(name="work", bufs=2))
    big = ctx.enter_context(tc.tile_pool(name="big", bufs=1))
    init2 = ctx.enter_context(tc.tile_pool(name="init2", bufs=2))
    logs = ctx.enter_context(tc.tile_pool(name="logs", bufs=2))
    chain = ctx.enter_context(tc.tile_pool(name="chain", bufs=3))
    ser = ctx.enter_context(tc.tile_pool(name="ser", bufs=1))
    chainP = ctx.enter_context(tc.tile_pool(name="chainP", bufs=2))
    ps_small = ctx.enter_context(tc.tile_pool(name="ps_s", bufs=1, space="PSUM"))
    ps_big = ctx.enter_context(tc.tile_pool(name="ps_b", bufs=2, space="PSUM"))
    ps_mid = ctx.enter_context(tc.tile_pool(name="ps_m", bufs=1, space="PSUM"))

    # ---------------- constants ----------------
    W3 = const.tile([DIN, 3 * DOUT], f32)
    nc.sync.dma_start(out=W3[:, 0:DOUT], in_=wq_ap)
    nc.sync.dma_start(out=W3[:, DOUT:2 * DOUT], in_=wk_ap)
    nc.sync.dma_start(out=W3[:, 2 * DOUT:3 * DOUT], in_=wv_ap)
    WQh = const.tile([DIN, DOUT], f16)
    WKh = const.tile([DIN, DOUT], f16)
    nc.vector.tensor_copy(out=WQh, in_=W3[:, 0:DOUT])
    nc.vector.tensor_copy(out=WKh, in_=W3[:, DOUT:2 * DOUT])

    IREP16 = const.tile([128, 64], f16)
    IREP32 = const.tile([128, 64], f32)
    for t in (IREP16, IREP32):
        make_identity(nc, t[0:64, :])
        make_identity(nc, t[64:128, :])
    # widened identity / block-coefficient tiles [128, WB]
    IW = {}
    for dt_, rep, tag in ((f16, IREP16, "16"), (f32, IREP32, "32")):
        w = const.tile([128, WB], dt_, tag=f"IW{tag}")
        for p in range(PAIR_BATCH):
            nc.vector.tensor_copy(out=w[:, ts(p, 64)], in_=rep)
        IW[tag] = w
    IWD = {}
    for dt_, rep, tag in ((f16, IREP16, "16"), (f32, IREP32, "32")):
        w = const.tile([128, PAIR_BATCH * 128], dt_, tag=f"IWD{tag}")
        for p in range(2 * PAIR_BATCH):
            nc.vector.tensor_copy(out=w[:, ts(p, 64)], in_=rep)
        IWD[tag] = w
    cI = {}
    for tag in ("16", "32"):
        for k in (4, 8, 12):
            dt_ = f16 if tag == "16" else f32
            t = const.tile([128, WB], dt_, tag=f"cI{tag}_{k}")
            nc.vector.tensor_scalar_mul(t, IW[tag], 1.0 / k)
            cI[(tag, k)] = t

    ones_col = const.tile([64, 1], f32)
    nc.vector.memset(ones_col, 1.0)
    ones_col_h = const.tile([64, 1], f16)
    nc.vector.memset(ones_col_h, 32.0)      # folds the /32 exp prescale
    ones_row = const.tile([1, 64], f32)
    nc.vector.memset(ones_row, 1.0)
    bias_ln = const.tile([64, 1], f32)
    nc.vector.memset(bias_ln, 1.0 + 64e-6)
    bias_one = const.tile([64, 1], f32)
    nc.vector.memset(bias_one, 1.0)

    FS4 = _flat_sched(4)
    FS3 = _flat_sched(3)

    def mm_pairs(out_ps, lhs_t, rhs_t, ncols=64):
        for p in range(PAIR_BATCH):
            for h in (0, 1):
                nc.tensor.matmul(
                    out_ps[h * 64:h * 64 + 64, ts(p, ncols)],
                    lhs_t[h * 64:h * 64 + 64, ts(p, 64)],
                    rhs_t[h * 64:h * 64 + 64, ts(p, ncols)],
                    start=True, stop=True)

    def chain_and_series(init_t, dt_, tag, flat_t, b):
        # generator: yields after each NS step so Q/K/V emission interleaves
        # V runs level 1 in fp32 (ill-conditioned state), then fp16.
        # Q/K use 3 sqrt levels (log scale 8), V uses 4 (scale 16).
        FS = FS4 if dt_ == f32 else FS3
        lscale = -16.0 if dt_ == f32 else -8.0
        irep = IW["16" if dt_ == f16 else "32"]
        ctag = "16" if dt_ == f16 else "32"
        if True:
            cs = ds(b * WB, WB)
            # state quad [Y | Yt | Z | Zt] per pair, 256 cols each
            SQ = chain.tile([128, PAIR_BATCH * 256], dt_, tag=f"SQ{tag}")
            sq4 = SQ.rearrange("p (n f c) -> p n f c", f=4, c=64)
            iv = init_t[:, cs].rearrange("p (n c) -> p n c", c=64)
            nc.vector.tensor_copy(out=sq4[:, :, 0, :], in_=iv)
            nc.vector.tensor_copy(out=sq4[:, :, 1, :], in_=iv)
            ir3 = irep.rearrange("p (n c) -> p n c", c=64)

            def qmm(out_ps, oslice, lhs4, li, rhs4, ri):
                for p in range(PAIR_BATCH):
                    for h in (0, 1):
                        nc.tensor.matmul(
                            out_ps[h * 64:h * 64 + 64, p * oslice[1] + oslice[0] * 64:
                                   p * oslice[1] + oslice[0] * 64 + 64],
                            lhs4[h * 64:h * 64 + 64, p * 256 + li * 64:p * 256 + li * 64 + 64],
                            rhs4[h * 64:h * 64 + 64, p * 256 + ri * 64:p * 256 + ri * 64 + 64] if ri is not None
                            else rhs4[h * 64:h * 64 + 64, ts(p, 64)],
                            start=True, stop=True)

            for k_idx, (lvl_start, al, be) in enumerate(FS):
                if lvl_start and k_idx == 8 and dt_ == f32:
                    # V-path precision drop: fp32 -> fp16 from level 2 on
                    dt_ = f16
                    irep = IW["16"]
                    ctag = "16"
                    ir3 = irep.rearrange("p (n c) -> p n c", c=64)
                    SQn = chain.tile([128, PAIR_BATCH * 256], dt_, tag=f"SQ{tag}")
                    sqn4 = SQn.rearrange("p (n f c) -> p n f c", f=4, c=64)
                    nc.vector.tensor_copy(out=sqn4[:, :, 0, :], in_=sq4[:, :, 0, :])
                    nc.vector.tensor_copy(out=sqn4[:, :, 1, :], in_=sq4[:, :, 1, :])
                    SQ, sq4 = SQn, sqn4
                if lvl_start:
                    # Z = I here, so W = Y and the level-start step needs no
                    # W-matmuls: P = aI + b*Y (from SBUF), and Z' = P Z = P.
                    Pb = chainP.tile([128, 2 * WB], dt_, tag=f"Pb{tag}")
                    pb3 = Pb.rearrange("p (n f c) -> p n f c", f=2, c=64)
                    nc.scalar.activation(out=pb3, in_=sq4[:, :, 0:2, :],
                                         func=ACT.Copy, bias=0.0, scale=be)
                    nc.vector.scalar_tensor_tensor(
                        out=Pb, in0=IWD[ctag], scalar=al,
                        in1=Pb, op0=OP.mult, op1=OP.add)
                else:
                    # W = Zt^T Y ; Wt = Y^T Zt
                    psA = ps_big.tile([128, PAIR_BATCH * 128], mybir.dt.float32, tag="psA")
                    qmm(psA, (0, 128), SQ, 3, SQ, 0)
                    qmm(psA, (1, 128), SQ, 0, SQ, 3)
                    Pb = chainP.tile([128, 2 * WB], dt_, tag=f"Pb{tag}")
                    nc.scalar.activation(out=Pb, in_=psA, func=ACT.Copy,
                                         bias=0.0, scale=be)
                    nc.vector.scalar_tensor_tensor(
                        out=Pb, in0=IWD[ctag], scalar=al,
                        in1=Pb, op0=OP.mult, op1=OP.add)
                # P = Pb[...,0], Pt = Pb[...,1]
                # Yn = Yt^T P ; Ytn = P^T Yt ; Zn = Pt^T Z ; Ztn = Z^T Pt
                psB = ps_big.tile([128, PAIR_BATCH * 128], mybir.dt.float32, tag="psB")
                for p in range(PAIR_BATCH):
                    for h in (0, 1):
                        hs = slice(h * 64, h * 64 + 64)
                        yt = SQ[hs, p * 256 + 64:p * 256 + 128]
                        pp = Pb[hs, p * 128:p * 128 + 64]
                        nc.tensor.matmul(psB[hs, p * 128:p * 128 + 64], yt, pp,
                                         start=True, stop=True)
                        nc.tensor.matmul(psB[hs, p * 128 + 64:p * 128 + 128], pp, yt,
                                         start=True, stop=True)
                SQ2 = chain.tile([128, PAIR_BATCH * 256], dt_, tag=f"SQ{tag}")
                sq24 = SQ2.rearrange("p (n f c) -> p n f c", f=4, c=64)
                psBr = psB.rearrange("p (n f c) -> p n f c", f=2, c=64)
                nc.scalar.activation(out=sq24[:, :, 0:2, :], in_=psBr,
                                     func=ACT.Copy, bias=0.0, scale=1.0)
                if lvl_start:
                    nc.vector.tensor_copy(
                        out=sq24[:, :, 2:4, :],
                        in_=Pb.rearrange("p (n f c) -> p n f c", f=2, c=64))
                else:
                    psC = ps_big.tile([128, PAIR_BATCH * 128], mybir.dt.float32, tag="psC")
                    for p in range(PAIR_BATCH):
                        for h in (0, 1):
                            hs = slice(h * 64, h * 64 + 64)
                            z = SQ[hs, p * 256 + 128:p * 256 + 192]
                            zt = SQ[hs, p * 256 + 192:p * 256 + 256]
                            pt = Pb[hs, p * 128 + 64:p * 128 + 128]
                            nc.tensor.matmul(psC[hs, p * 128:p * 128 + 64], pt, z,
                                             start=True, stop=True)
                            nc.tensor.matmul(psC[hs, p * 128 + 64:p * 128 + 128], z, pt,
                                             start=True, stop=True)
                    psCr = psC.rearrange("p (n f c) -> p n f c", f=2, c=64)
                    nc.vector.tensor_copy(out=sq24[:, :, 2:4, :], in_=psCr)
                SQ, sq4 = SQ2, sq24
                yield
            # R = (Y + Yt)/2 ; E = I - R
            E = ser.tile([128, WB], dt_, tag=f"E{tag}")
            e3 = E.rearrange("p (n c) -> p n c", c=64)
            nc.vector.tensor_add(e3, sq4[:, :, 0, :], sq4[:, :, 1, :])
            nc.vector.scalar_tensor_tensor(
                out=e3, in0=e3, scalar=-0.5,
                in1=ir3, op0=OP.mult, op1=OP.add)
            if ctag == "32" and b == 0:
                tap("serE", E)
            psE = ps_mid.tile([128, WB], mybir.dt.float32, tag="ps2")
            mm_pairs(psE, E, E)
            E2 = ser.tile([128, WB], dt_, tag=f"E2{tag}")
            nc.vector.tensor_copy(out=E2, in_=psE)
            psE3 = ps_mid.tile([128, WB], mybir.dt.float32, tag="ps2")
            mm_pairs(psE3, E2, E)
            E3 = ser.tile([128, WB], dt_, tag=f"E3{tag}")
            nc.vector.tensor_copy(out=E3, in_=psE3)
            yield
            psE4 = ps_mid.tile([128, WB], mybir.dt.float32, tag="ps2")
            mm_pairs(psE4, E2, E2)
            E4 = ser.tile([128, WB], dt_, tag=f"E4{tag}")
            nc.vector.tensor_copy(out=E4, in_=psE4)
            if ctag == "32" and b == 0:
                tap("serE4", E4)
            B = ser.tile([128, WB], dt_, tag=f"B{tag}")
            nc.vector.scalar_tensor_tensor(out=B, in0=E, scalar=1.0 / 13, in1=cI[(ctag, 12)], op0=OP.mult, op1=OP.add)
            nc.vector.scalar_tensor_tensor(out=B, in0=E2, scalar=1.0 / 14, in1=B, op0=OP.mult, op1=OP.add)
            psH = ps_mid.tile([128, WB], mybir.dt.float32, tag="ps2")
            mm_pairs(psH, E4, B)
            H = ser.tile([128, WB], dt_, tag=f"B{tag}")
            nc.vector.scalar_tensor_tensor(out=H, in0=E, scalar=1.0 / 9, in1=cI[(ctag, 8)], op0=OP.mult, op1=OP.add)
            nc.vector.scalar_tensor_tensor(out=H, in0=E2, scalar=1.0 / 10, in1=H, op0=OP.mult, op1=OP.add)
            nc.vector.scalar_tensor_tensor(out=H, in0=E3, scalar=1.0 / 11, in1=H, op0=OP.mult, op1=OP.add)
            yield
            Hs = ser.tile([128, WB], dt_, tag=f"Hs{tag}")
            nc.vector.tensor_copy(out=Hs, in_=psH)
            nc.vector.tensor_add(H, H, Hs)
            psH2 = ps_mid.tile([128, WB], mybir.dt.float32, tag="ps2")
            mm_pairs(psH2, E4, H)
            H2 = ser.tile([128, WB], dt_, tag=f"B{tag}")
            nc.vector.scalar_tensor_tensor(out=H2, in0=E, scalar=1.0 / 5, in1=cI[(ctag, 4)], op0=OP.mult, op1=OP.add)
            nc.vector.scalar_tensor_tensor(out=H2, in0=E2, scalar=1.0 / 6, in1=H2, op0=OP.mult, op1=OP.add)
            nc.vector.scalar_tensor_tensor(out=H2, in0=E3, scalar=1.0 / 7, in1=H2, op0=OP.mult, op1=OP.add)
            Hs2 = ser.tile([128, WB], dt_, tag=f"Hs{tag}")
            nc.vector.tensor_copy(out=Hs2, in_=psH2)
            nc.vector.tensor_add(H2, H2, Hs2)
            psH3 = ps_mid.tile([128, WB], mybir.dt.float32, tag="ps2")
            mm_pairs(psH3, E4, H2)
            B0 = ser.tile([128, WB], dt_, tag=f"B{tag}")
            nc.vector.tensor_scalar_mul(B0, E2, 0.5)
            nc.vector.scalar_tensor_tensor(out=B0, in0=E3, scalar=1.0 / 3, in1=B0, op0=OP.mult, op1=OP.add)
            nc.vector.tensor_add(B0, B0, E)
            Hs3 = ser.tile([128, WB], dt_, tag=f"Hs{tag}")
            nc.vector.tensor_copy(out=Hs3, in_=psH3)
            nc.vector.tensor_add(B0, B0, Hs3)
            if ctag == "32" and b == 0:
                tap("serB0", B0)
            LS = logs.tile([128, WB], flat_t.dtype, tag=f"LS{tag}")
            nc.scalar.activation(out=LS, in_=B0, func=ACT.Copy,
                                 bias=0.0, scale=lscale)
            flat3 = flat_t.rearrange("p (n two c) -> p n two c", two=2, c=64)
            nc.vector.tensor_copy(
                out=flat3[:, ds(b * PAIR_BATCH, PAIR_BATCH), 0, :],
                in_=LS[0:64, :].rearrange("p (n c) -> p n c", c=64))
            nc.gpsimd.dma_start(
                out=flat3[:, ds(b * PAIR_BATCH, PAIR_BATCH), 1, :],
                in_=LS[64:128, :].rearrange("p (n c) -> p n c", c=64))

    # ======================== per-sample pipeline ========================
    for s in range(nsamp):
        initQ = init2.tile([128, NP_PAIR * 64], f16, tag="initQ")
        initK = init2.tile([128, NP_PAIR * 64], f16, tag="initK")
        initV = init2.tile([128, NP_PAIR * 64], f32, tag="initV")
        oddQ = init2.tile([64, NP_PAIR * 64], f16, tag="oddQ")
        oddK = init2.tile([64, NP_PAIR * 64], f16, tag="oddK")
        oddV = init2.tile([64, NP_PAIR * 64], f32, tag="oddV")

        for it in range(M):
            if it % 16 == 0:
                xbuf = work.tile([DIN, 16 * DIN], f32, tag="xbuf")
                nc.sync.dma_start(
                    out=xbuf.rearrange("p (i c) -> p i c", c=DIN),
                    in_=x_ap[s, ds(it, 16)].rearrange("i p c -> p i c"))
            p1 = ps_mid.tile([DIN, 3 * DOUT], mybir.dt.float32, tag="ps2")
            nc.tensor.matmul(p1, xbuf[:, ts(it % 16, DIN)], W3, start=True, stop=True)
            P1qk = work.tile([DIN, 2 * DOUT], f16, tag="p1qk")
            nc.vector.tensor_copy(out=P1qk, in_=p1[:, 0:2 * DOUT])
            P1v = work.tile([DIN, DOUT], f32, tag="p1v")
            nc.vector.tensor_copy(out=P1v, in_=p1[:, 2 * DOUT:3 * DOUT])
            pqkv = ps_small.tile([64, 192], mybir.dt.float32, tag="small")
            nc.tensor.matmul(pqkv[:, 0:64], WQh, P1qk[:, 0:DOUT], start=True, stop=True)
            nc.tensor.matmul(pqkv[:, 64:128], WKh, P1qk[:, DOUT:2 * DOUT], start=True, stop=True)
            nc.tensor.matmul(pqkv[:, 128:192], W3[:, 2 * DOUT:3 * DOUT], P1v, start=True, stop=True)
            pr, h = it // 2, it % 2
            for ci, (init_t, odd_t) in enumerate(((initQ, oddQ), (initK, oddK), (initV, oddV))):
                src = pqkv[:, ci * 64:(ci + 1) * 64]
                if h == 0:
                    nc.scalar.activation(out=init_t[0:64, ts(pr, 64)], in_=src,
                                         func=ACT.Copy, bias=0.0, scale=1.0 / C_NORM)
                else:
                    nc.scalar.activation(out=odd_t[:, ts(pr, 64)], in_=src,
                                         func=ACT.Copy, bias=0.0, scale=1.0 / C_NORM)
        for init_t, odd_t in ((initQ, oddQ), (initK, oddK), (initV, oddV)):
            nc.gpsimd.dma_start(out=init_t[64:128, :], in_=odd_t)
        if s == 0:
            tap("irep32", IREP32)
            tap("initV", initV)

        flatQ = big.tile([64, M * 64], f16, tag="flatQ")
        flatK = big.tile([64, M * 64], f16, tag="flatK")
        flatV = big.tile([64, M * 64], f32, tag="f32scr")
        for b in range(NBATCH):
            gens = [chain_and_series(initQ, f16, "q", flatQ, b),
                    chain_and_series(initK, f16, "k", flatK, b),
                    chain_and_series(initV, f32, "v", flatV, b)]
            while gens:
                gens = [g for g in gens if next(g, StopIteration) is not StopIteration]
        if s == 0:
            tap("flatV", flatV)
            tap("flatQ", flatQ)

        # ---------------- attention ----------------
        partQ = work.tile([64, M], f32, tag="partQ")
        partK = work.tile([64, M], f32, tag="partK")
        for flat_t, part_t in ((flatQ, partQ), (flatK, partK)):
            sq = big.tile([64, M * 64], f32, tag="VF")
            nc.vector.tensor_mul(sq, flat_t, flat_t)
            nc.vector.tensor_reduce(
                out=part_t, in_=sq.rearrange("p (i c) -> p i c", c=64),
                axis=AX.X, op=OP.add)
        ps_qn = ps_small.tile([1, 64], mybir.dt.float32, tag="small")
        nc.tensor.matmul(ps_qn, ones_col, partQ, start=True, stop=True)
        qn_row = work.tile([1, 64], f32, tag="qnrow_sb")
        nc.vector.tensor_copy(out=qn_row, in_=ps_qn)
        ps_kn = ps_small.tile([64, 1], mybir.dt.float32, tag="small")
        nc.tensor.matmul(ps_kn, partK, ones_col, start=True, stop=True)
        kn_col = work.tile([64, 1], f32, tag="kncol_sb")
        nc.vector.tensor_copy(out=kn_col, in_=ps_kn)
        ps_qrep = ps_small.tile([64, 64], mybir.dt.float32, tag="small")
        nc.tensor.matmul(ps_qrep, ones_row, qn_row, start=True, stop=True)
        qrep = work.tile([64, 64], f32, tag="qrep_sb")
        nc.vector.tensor_copy(out=qrep, in_=ps_qrep)

        ps_cross = ps_small.tile([64, 64], mybir.dt.float32, tag="small")
        fQ3 = flatQ.rearrange("p (i c) -> p c i", c=64)
        fK3 = flatK.rearrange("p (i c) -> p c i", c=64)
        for c in range(64):
            nc.tensor.matmul(ps_cross, fK3[:, c, :], fQ3[:, c, :],
                             start=(c == 0), stop=(c == 63))
        cross_sb = work.tile([64, 64], f32, tag="cross_sb")
        nc.vector.tensor_copy(out=cross_sb, in_=ps_cross)
        Et = work.tile([64, 64], f32, tag="Et")
        nc.vector.scalar_tensor_tensor(out=Et, in0=cross_sb, scalar=-2.0,
                                       in1=qrep, op0=OP.mult, op1=OP.add)
        nc.vector.tensor_scalar(out=Et, in0=Et, scalar1=kn_col, scalar2=0.0,
                                op0=OP.add, op1=OP.max)
        lnE = work.tile([64, 64], f32, tag="lnE")
        nc.scalar.activation(out=lnE, in_=Et, func=ACT.Ln,
                             bias=bias_ln, scale=1.0)
        ln1 = work.tile([64, 64], f32, tag="ln1")
        nc.vector.tensor_scalar_add(ln1, lnE, 1.0)
        sc = work.tile([64, 64], f32, tag="sc")
        nc.vector.reciprocal(out=sc, in_=ln1)
        expS = work.tile([64, 64], f16, tag="expS")
        nc.scalar.activation(out=expS, in_=sc, func=ACT.Exp, bias=0.0, scale=1.0)
        if s == 0:
            tap("Et", Et)
            tap("expS", expS)
        ps_cs = ps_small.tile([64, 1], mybir.dt.float32, tag="small")
        nc.tensor.matmul(ps_cs, expS, ones_col_h, start=True, stop=True)
        inv = work.tile([64, 1], f32, tag="inv")
        nc.vector.reciprocal(out=inv, in_=ps_cs)

        VF = big.tile([64, M * 64], f32, tag="VF")
        VF3 = VF.rearrange("p (r c) -> p r c", c=64)
        for r in range(64):
            nc.gpsimd.dma_start(
                out=VF3[:, r:r + 1, :],
                in_=flatV[r:r + 1, :].rearrange("p (i c) -> p i c", c=64))
        expS32 = work.tile([64, 64], f32, tag="expS32")
        nc.vector.tensor_copy(out=expS32, in_=expS)
        M2 = big.tile([64, M * 64], f32, tag="f32scr")
        for ch in range(8):
            ps_m2 = ps_small.tile([64, 512], mybir.dt.float32, tag="small")
            nc.tensor.matmul(ps_m2, expS32, VF[:, ts(ch, 512)], start=True, stop=True)
            nc.vector.tensor_scalar_mul(M2[:, ts(ch, 512)], ps_m2, inv)

        S1M = big.tile([128, NP_PAIR * 64], f32, tag="scr8c")
        for j in range(M):
            pr, h = j // 2, j % 2
            nc.gpsimd.dma_start(
                out=S1M[h * 64:h * 64 + 64, ts(pr, 64)].rearrange("p (o c) -> p o c", o=1),
                in_=M2[j:j + 1, :].rearrange("p (r c) -> p r c", c=64))

        if s == 0:
            tap("M2", M2)
            tap("S1M", S1M)
        # ---------------- exp: scaling-squaring ----------------
        outS1 = big.tile([128, NP_PAIR * 64], f32, tag="outS1")
        for b in range(NBATCH):
            cs = ds(b * WB, WB)
            X = S1M[:, cs]
            H = chain.tile([128, WB], f32, tag="expH")
            nc.vector.scalar_tensor_tensor(
                out=H, in0=X, scalar=1.0 / EXP_DEG, in1=IW["32"],
                op0=OP.mult, op1=OP.add)
            for k in range(EXP_DEG - 1, 0, -1):
                psx = ps_mid.tile([128, WB], mybir.dt.float32, tag="ps2")
                mm_pairs(psx, X, H)
                H2 = chain.tile([128, WB], f32, tag="expH")
                nc.vector.tensor_scalar_mul(H2, psx, 1.0 / k)
                nc.vector.tensor_add(H2, H2, IW["32"])
                H = H2
            for sq in range(EXP_SQ):
                psx = ps_mid.tile([128, WB], mybir.dt.float32, tag="ps2")
                mm_pairs(psx, H, H)
                if sq < EXP_SQ - 1:
                    H2 = chain.tile([128, WB], f32, tag="expH")
                    nc.vector.tensor_copy(out=H2, in_=psx)
                    H = H2
                else:
                    nc.vector.tensor_scalar_mul(outS1[:, cs], psx, C_NORM)

        o3 = out_ap[s].rearrange("(pr two) r c -> two r pr c", two=2)
        nc.sync.dma_start(
            out=o3[0], in_=outS1[0:64, :].rearrange("p (pr c) -> p pr c", c=64))
        nc.sync.dma_start(
            out=o3[1], in_=outS1[64:128, :].rearrange("p (pr c) -> p pr c", c=64))


def build(nsamp=NSAMP, num_devices=NCORES):
    import concourse.bacc as bacc
    import concourse.mybir as mybir
    import concourse.tile as tile

    nc = bacc.Bacc("TRN2", target_bir_lowering=False, debug=False,
                   num_devices=num_devices)
    f32 = mybir.dt.float32
    x_ap = nc.dram_tensor("x", [nsamp, M, DIN, DIN], f32, kind="ExternalInput").ap()
    wq = nc.dram_tensor("wq", [DIN, DOUT], f32, kind="ExternalInput").ap()
    wk = nc.dram_tensor("wk", [DIN, DOUT], f32, kind="ExternalInput").ap()
    wv = nc.dram_tensor("wv", [DIN, DOUT], f32, kind="ExternalInput").ap()
    out = nc.dram_tensor("out", [nsamp, M, DOUT, DOUT], f32, kind="ExternalOutput").ap()

    tapspec = {}
    if DEBUG:
        tapspec = {
            "irep32": [128, 64], "initV": [128, NP_PAIR * 64],
            "flatV": [64, M * 64], "flatQ": [64, M * 64],
            "Et": [64, 64], "expS": [64, 64],
            "M2": [64, M * 64], "S1M": [128, NP_PAIR * 64],
        }
        for k in (3, 5, 7, 8, 11, 14, 17, 20):
            tapspec[f"chainYW{k}"] = [128, PAIR_BATCH * 128]
        tapspec["serE"] = [128, PAIR_BATCH * 64]
        tapspec["serE4"] = [128, PAIR_BATCH * 64]
        tapspec["serB0"] = [128, PAIR_BATCH * 64]
    taps = {k: nc.dram_tensor("tap_" + k, v, f32 if k != "flatQ" and k != "expS" else mybir.dt.float16,
                              kind="ExternalOutput").ap()
            for k, v in tapspec.items()}
    with tile.TileContext(nc) as tc, ExitStack() as ctx:
        emit_kernel(nc, tc, ctx, x_ap, wq, wk, wv, out, nsamp=nsamp, taps=taps)
    nc.compile()
    return nc


_CACHED = {}


def _get_nc(nsamp):
    from concourse.bass_interp import get_hw_module
    if nsamp not in _CACHED:
        nc = build(nsamp=nsamp)
        nc.m = get_hw_module(nc.m)
        _CACHED[nsamp] = nc
    return _CACHED[nsamp]


def kernel(x, Wq, Wk, Wv):
    from concourse.bass_utils import run_bass_kernel_spmd

    bs = x.shape[0]
    nsamp = bs // NCORES
    nc = _get_nc(nsamp)
    in_maps = []
    for c in range(NCORES):
        in_maps.append({
            "x": np.ascontiguousarray(x[c * nsamp:(c + 1) * nsamp], dtype=np.float32),
            "wq": np.ascontiguousarray(Wq, dtype=np.float32),
            "wk": np.ascontiguousarray(Wk, dtype=np.float32),
            "wv": np.ascontiguousarray(Wv, dtype=np.float32),
        })
    res = run_bass_kernel_spmd(nc, in_maps, list(range(NCORES)))
    outs = [res.results[c]["out"] for c in range(NCORES)]
    full = np.concatenate(outs, axis=0)
    return full.reshape(bs * M, DOUT, DOUT).astype(np.float32)
